# revision 1
# baseline (speedup 1.0000x reference)
"""Trainium2 Bass kernel for nn_AttentionBlock (GroupNorm + MHA + proj + residual).

Full inputs in, full output out. Sharding: 8 cores = 2 batches x 4 query-slices.
Each core: GroupNorm over its batch image (replicated within the batch group),
q projection for its 1024 queries, k/v projections over all 4096 keys,
per-head attention (S^T = k^T q formulation, softmax along the PSUM partition
axis via an appended ones-column in the PV matmul), output projection and
residual for its query slice. Host side only slices/rotates/concatenates.

All matmuls run in bf16 with fp32 PSUM accumulation; softmax logits stay fp32.
"""
import numpy as np

C = 512          # channels
N = 4096         # pixels (64*64)
NQ = 1024        # queries per core
H = 8            # heads
D = 64           # head dim
T = 4            # 128-channel chunks
W = NQ // 512    # query windows of 512
MT = N // 128    # key m-tiles of 128
NGROUPS = 8
EPS = 1e-5
GELEM = (C // NGROUPS) * N   # elements per norm group
MG = [3, 2] * 6 + [2]        # alternating m-tile group sizes (3+2 psum banks)

DEBUG = False                # adds intermediate-dump DRAM outputs

_COMPILED = None


def _emit(tc, io):
    import concourse.bass as bass
    from concourse import mybir, bass_isa
    from contextlib import ExitStack

    nc = tc.nc
    f32 = mybir.dt.float32
    bf16 = mybir.dt.bfloat16
    Alu = mybir.AluOpType
    Act = mybir.ActivationFunctionType

    xb, qkvw, qkvb, projw, projb, nw, nb, y = (
        io["xb"], io["qkvw"], io["qkvb"], io["projw"], io["projb"],
        io["nw"], io["nb"], io["y"])

    ctx = ExitStack()
    with ctx:
        # ---------------- pools ----------------
        # PSUM: big pool 2x(128,1536) [6 banks] shared by S-tiles, phase-3
        # accumulators and weight transposes; pv gets its own bank; bc/proj
        # share one more. 6+1+1 = 8 banks.
        left = ctx.enter_context(tc.tile_pool(name="left", bufs=1))
        psum_big = ctx.enter_context(tc.tile_pool(name="psum_big", bufs=1, space="PSUM"))
        psum_s2 = ctx.enter_context(tc.tile_pool(name="psum_s2", bufs=1, space="PSUM"))
        psum_pv = ctx.enter_context(tc.tile_pool(name="psum_pv", bufs=2, space="PSUM"))
        psum_acc = ctx.enter_context(tc.tile_pool(name="psum_acc", bufs=1, space="PSUM"))

        right_ctx = ExitStack()
        xf_pool = right_ctx.enter_context(
            tc.tile_pool(name="xf_pool", bufs=1, side="right"))
        wstg_pool = right_ctx.enter_context(
            tc.tile_pool(name="wstg_pool", bufs=4, side="right"))
        scr_pool = right_ctx.enter_context(
            tc.tile_pool(name="scr_pool", bufs=2, side="right"))

        # ---------------- persistent tiles ----------------
        xn = [left.tile([128, N], bf16, name=f"xn{t}", tag=f"xn{t}") for t in range(T)]
        ksb = [left.tile([128, N], bf16, name=f"ksb{t}", tag=f"ksb{t}") for t in range(T)]
        qsb = [left.tile([128, NQ], bf16, name=f"qsb{t}", tag=f"qsb{t}") for t in range(T)]
        wTq = [left.tile([128, 1536], bf16, name=f"wTq{t}", tag=f"wTq{t}") for t in range(T)]
        wTp = [left.tile([128, C], bf16, name=f"wTp{t}", tag=f"wTp{t}") for t in range(T)]
        vb_bc = left.tile([128, C], f32, name="vb_bc", tag="vb_bc")
        ones_row = left.tile([1, D], f32, name="ones_row", tag="ones_row")
        qb = [left.tile([128, 1], f32, name=f"qb{i}", tag=f"qb{i}") for i in range(8)]
        pb = [left.tile([128, 1], f32, name=f"pb{i}", tag=f"pb{i}") for i in range(T)]
        nwt = [left.tile([128, 1], f32, name=f"nwt{t}", tag=f"nwt{t}") for t in range(T)]
        nbt = [left.tile([128, 1], f32, name=f"nbt{t}", tag=f"nbt{t}") for t in range(T)]
        stat = [left.tile([128, 2], f32, name=f"stat{t}", tag=f"stat{t}") for t in range(T)]
        gstat = [left.tile([128, 2], f32, name=f"gstat{t}", tag=f"gstat{t}") for t in range(T)]

        # ---------------- input DMAs ----------------
        xf = [xf_pool.tile([128, N], f32, name=f"xf{t}", tag=f"xf{t}") for t in range(T)]
        for t in range(T):
            for c4 in range(4):   # split across DMA queues
                nc.sync.dma_start(
                    xf[t][:, 1024 * c4:1024 * (c4 + 1)],
                    xb[128 * t:128 * (t + 1), 1024 * c4:1024 * (c4 + 1)])
            nc.sync.dma_start(nwt[t][:, 0:1], nw[128 * t:128 * (t + 1)])
            nc.sync.dma_start(nbt[t][:, 0:1], nb[128 * t:128 * (t + 1)])
            nc.sync.dma_start(pb[t][:, 0:1], projb[128 * t:128 * (t + 1)])
        for i in range(8):
            nc.sync.dma_start(qb[i][:, 0:1], qkvb[128 * i:128 * (i + 1)])
        # v bias broadcast to 128 partitions (stride-0 partition read)
        nc.gpsimd.dma_start(
            out=vb_bc[:],
            in_=bass.AP(tensor=qkvb.tensor, offset=1024, ap=[[0, 128], [1, C]]))
        nc.vector.memset(ones_row[0:1, :], 1.0)

        # weights: natural-layout contiguous DMA, cast to bf16, transpose
        # 128x128 blocks on the PE (identity trick) into wTq/wTp.
        # identity + group-indicator matrices come in as constant inputs
        # (gpsimd ucode for iota/affine_select is unavailable here)
        ident = left.tile([128, 128], bf16, name="ident", tag="ident")
        nc.sync.dma_start(ident[:], io["cid"][:, :])
        ind = left.tile([128, 2], f32, name="ind", tag="ind")
        nc.sync.dma_start(ind[:], io["cind"][:, :])
        indT = left.tile([2, 128], f32, name="indT", tag="indT")
        nc.sync.dma_start(indT[0:2, :], io["cindT"][:, :])
        for i in range(12):   # qkv_w row-tiles
            wstg = wstg_pool.tile([128, C], f32, name="wstg", tag="wstg")
            nc.sync.dma_start(wstg[:], qkvw[128 * i:128 * (i + 1), :])
            wbf = wstg_pool.tile([128, C], bf16, name="wbf", tag="wbf")
            nc.vector.tensor_copy(wbf[:], wstg[:])
            for j in range(T):
                tp = psum_big.tile([128, 128], bf16, name="tp", tag="sbig")
                nc.tensor.transpose(tp[:], wbf[:, 128 * j:128 * (j + 1)], ident[:])
                nc.vector.tensor_copy(wTq[j][:, 128 * i:128 * (i + 1)], tp[:])
        for i in range(4):    # proj_w row-tiles
            wstg = wstg_pool.tile([128, C], f32, name="wstg", tag="wstg")
            nc.sync.dma_start(wstg[:], projw[128 * i:128 * (i + 1), :])
            wbf = wstg_pool.tile([128, C], bf16, name="wbf", tag="wbf")
            nc.vector.tensor_copy(wbf[:], wstg[:])
            for j in range(T):
                tp = psum_big.tile([128, 128], bf16, name="tp", tag="sbig")
                nc.tensor.transpose(tp[:], wbf[:, 128 * j:128 * (j + 1)], ident[:])
                nc.vector.tensor_copy(wTp[j][:, 128 * i:128 * (i + 1)], tp[:])

        # ---------------- phase 1: group stats ----------------
        for t in range(T):
            nc.vector.tensor_reduce(
                out=stat[t][:, 0:1], in_=xf[t][:], axis=mybir.AxisListType.X, op=Alu.add)
            sq_scr = scr_pool.tile([128, N], bf16, name="sq_scr", tag="sq_scr")
            nc.scalar.activation(
                sq_scr[:], xf[t][:], Act.Square, accum_out=stat[t][:, 1:2])
            # group-reduce over partitions via indicator matmuls:
            # gg[g,s] = sum_ch ind[ch,g]*stat[ch,s]; then broadcast back
            # per channel: gstat[ch,s] = sum_g indT[g,ch]*gg[g,s]
            gg_ps = psum_acc.tile([2, 2], f32, name="gg_ps", tag="acc")
            nc.tensor.matmul(gg_ps[0:2, :], ind[:, 0:2], stat[t][:, 0:2],
                             start=True, stop=True)
            gg_sb = left.tile([2, 2], f32, name=f"gg_sb{t}", tag=f"gg_sb{t}")
            nc.vector.tensor_copy(gg_sb[0:2, :], gg_ps[0:2, :])
            gb_ps = psum_acc.tile([128, 2], f32, name="gb_ps", tag="acc")
            nc.tensor.matmul(gb_ps[:, 0:2], indT[0:2, :], gg_sb[0:2, :],
                             start=True, stop=True)
            nc.vector.tensor_copy(gstat[t][:, 0:2], gb_ps[:, 0:2])
            # mean/var/rstd -> per-channel affine a,b
            mean_t = left.tile([128, 1], f32, name=f"mean{t}", tag=f"mean{t}")
            e2_t = left.tile([128, 1], f32, name=f"e2{t}", tag=f"e2{t}")
            var_t = left.tile([128, 1], f32, name=f"var{t}", tag=f"var{t}")
            std_t = left.tile([128, 1], f32, name=f"std{t}", tag=f"std{t}")
            a_t = left.tile([128, 1], f32, name=f"a{t}", tag=f"a{t}")
            b_t = left.tile([128, 1], f32, name=f"b{t}", tag=f"b{t}")
            inv = 1.0 / GELEM
            nc.vector.tensor_scalar(mean_t[:], gstat[t][:, 0:1], inv, None, Alu.mult)
            nc.vector.tensor_scalar(e2_t[:], gstat[t][:, 1:2], inv, None, Alu.mult)
            nc.vector.scalar_tensor_tensor(
                var_t[:], mean_t[:], -1.0, mean_t[:], Alu.mult, Alu.mult)
            nc.vector.scalar_tensor_tensor(
                var_t[:], e2_t[:], EPS, var_t[:], Alu.add, Alu.add)
            nc.scalar.activation(std_t[:], var_t[:], Act.Sqrt)
            nc.vector.reciprocal(a_t[:], std_t[:])
            nc.vector.tensor_tensor(a_t[:], a_t[:], nwt[t][:], Alu.mult)
            nc.vector.tensor_tensor(b_t[:], mean_t[:], a_t[:], Alu.mult)
            nc.vector.tensor_tensor(b_t[:], nbt[t][:], b_t[:], Alu.subtract)
            # phase 2: normalize + cast
            nc.vector.tensor_scalar(
                xn[t][:], xf[t][:], a_t[:, 0:1], b_t[:, 0:1], Alu.mult, Alu.add)
            if DEBUG:
                d = io["dbg_ab"]
                nc.sync.dma_start(d[128 * t:128 * t + 128, 0:1], a_t[:])
                nc.sync.dma_start(d[128 * t:128 * t + 128, 1:2], b_t[:])
                nc.sync.dma_start(d[128 * t:128 * t + 128, 2:3], stat[t][:, 0:1])
                nc.sync.dma_start(d[128 * t:128 * t + 128, 3:4], stat[t][:, 1:2])
                nc.sync.dma_start(d[128 * t:128 * t + 128, 4:5], gstat[t][:, 0:1])
                nc.sync.dma_start(d[128 * t:128 * t + 128, 5:6], gstat[t][:, 1:2])
                nc.sync.dma_start(io["dbg_xn"][128 * t:128 * t + 128, :], xn[t][:, 0:64])

        right_ctx.close()

        # ---------------- mid pools (reuse xf space) ----------------
        mid = ctx.enter_context(tc.tile_pool(name="mid", bufs=1))
        psb_pool = ctx.enter_context(tc.tile_pool(name="psb_pool", bufs=4))
        rec_pool = ctx.enter_context(tc.tile_pool(name="rec_pool", bufs=2))
        yo_pool = ctx.enter_context(tc.tile_pool(name="yo_pool", bufs=2))

        vT = mid.tile([128, MT * 520], bf16, name="vT", tag="vT")
        attn = [mid.tile([128, NQ], bf16, name=f"attn{t}", tag=f"attn{t}") for t in range(T)]
        xres = [mid.tile([128, NQ], f32, name=f"xres{t}", tag=f"xres{t}") for t in range(T)]
        for t in range(T):
            nc.sync.dma_start(xres[t][:], xb[128 * t:128 * (t + 1), 0:NQ])

        # ones columns of the augmented v^T (denominator trick)
        ones_view = vT[:].rearrange("p (m h x) -> p m h x", m=MT, x=65)[:, :, :, 64:65]
        nc.vector.memset(ones_view, 1.0)

        # ---------------- phase 3: projections ----------------
        # q: out rows 0..511 of qkv, only NQ query columns
        for i in range(T):
            for w in range(W):
                if (i * W + w) % 2 == 0:
                    qp = psum_big.tile([128, 512], f32, name="qp", tag="sbig")
                else:
                    qp = psum_s2.tile([128, 512], f32, name="qp2", tag="s2")
                for k in range(T):
                    nc.tensor.matmul(
                        qp[:], wTq[k][:, 128 * i:128 * i + 128],
                        xn[k][:, 512 * w:512 * w + 512],
                        start=(k == 0), stop=(k == T - 1))
                nc.vector.tensor_scalar(
                    qsb[i][:, 512 * w:512 * w + 512], qp[:], qb[i][:, 0:1], None, Alu.add)
        # k: out rows 512..1023, all N columns
        for i in range(T):
            for w in range(N // 512):
                if (i * 8 + w) % 2 == 0:
                    kp = psum_big.tile([128, 512], f32, name="kp", tag="sbig")
                else:
                    kp = psum_s2.tile([128, 512], f32, name="kp2", tag="s2")
                for k in range(T):
                    nc.tensor.matmul(
                        kp[:], wTq[k][:, 512 + 128 * i:512 + 128 * i + 128],
                        xn[k][:, 512 * w:512 * w + 512],
                        start=(k == 0), stop=(k == T - 1))
                nc.vector.tensor_scalar(
                    ksb[i][:, 512 * w:512 * w + 512], kp[:], qb[4 + i][:, 0:1], None, Alu.add)
        # vT: (m, 512) per m-tile, strided into the 65-column augmented layout
        for mt in range(MT):
            if mt % 2 == 0:
                vp = psum_big.tile([128, 512], f32, name="vp", tag="sbig")
            else:
                vp = psum_s2.tile([128, 512], f32, name="vp2", tag="s2")
            for k in range(T):
                nc.tensor.matmul(
                    vp[:], xn[k][:, 128 * mt:128 * mt + 128],
                    wTq[k][:, 1024:1536],
                    start=(k == 0), stop=(k == T - 1))
            dst = vT[:, 520 * mt:520 * mt + 520].rearrange(
                "p (h x) -> p h x", x=65)[:, :, 0:64]
            src = vp[:].rearrange("p (h x) -> p h x", x=64)
            vbv = vb_bc[:].rearrange("p (h x) -> p h x", x=64)
            nc.vector.tensor_tensor(dst, src, vbv, Alu.add)
            if DEBUG and mt == 0:
                nc.sync.dma_start(io["dbg_vt"][:], vT[:, 0:520])
                nc.sync.dma_start(io["dbg_k"][0:128, :], ksb[0][:, 0:64])
                nc.sync.dma_start(io["dbg_q"][0:128, :], qsb[0][:, 0:64])

        # ---------------- phase 4: attention ----------------
        for w in range(W):
            for h in range(H):
                kt, pr = h // 2, 64 * (h % 2)
                pv = psum_pv.tile([128, 512], f32, name="pv", tag="pv")
                mt = 0
                for gs in MG:
                    if gs == 3:
                        sp = psum_big.tile([128, 1536], f32, name="sp", tag="sbig")
                    else:
                        sp = psum_s2.tile([128, 1024], f32, name="sp2", tag="s2")
                    for j in range(gs):
                        nc.tensor.matmul(
                            sp[:, 512 * j:512 * j + 512],
                            ksb[kt][pr:pr + 64, 128 * (mt + j):128 * (mt + j) + 128],
                            qsb[kt][pr:pr + 64, 512 * w:512 * w + 512],
                            start=True, stop=True)
                    ps = psb_pool.tile([128, 1536], bf16, name="ps", tag="ps")
                    nc.scalar.activation(
                        ps[:, 0:512 * gs], sp[:, 0:512 * gs], Act.Exp, scale=0.125)
                    for j in range(gs):
                        m = mt + j
                        nc.tensor.matmul(
                            pv[0:65, :],
                            vT[:, 520 * m + 65 * h:520 * m + 65 * h + 65],
                            ps[:, 512 * j:512 * j + 512],
                            start=(m == 0), stop=(m == MT - 1))
                    mt += gs
                # NOTE: reciprocal_approx_* mis-handles nonzero partition
                # offsets on HW (reads partition 0), so stage the denominator
                # row at partition 0 first
                dnm = rec_pool.tile([1, 512], f32, name="dnm", tag="dnm")
                nc.vector.tensor_copy(dnm[0:1, :], pv[64:65, :])
                if DEBUG and w == 0:
                    dd2 = rec_pool.tile([1, 512], f32, name="dd2", tag="dd2")
                    nc.vector.tensor_copy(dd2[0:1, :], pv[0:1, :])
                    nc.sync.dma_start(io["dbg_den"][h:h + 1, :], dnm[0:1, :])
                    nc.sync.dma_start(io["dbg_pv"][h:h + 1, :], dd2[0:1, :])
                rec = rec_pool.tile([1, 512], f32, name="rec", tag="rec")
                rscr = rec_pool.tile([1, 512], f32, name="rscr", tag="rscr")
                nc.vector.reciprocal_approx_accurate(
                    rec[0:1, :], dnm[0:1, :], rscr[0:1, :])
                bc = psum_acc.tile([128, 512], f32, name="bc", tag="acc")
                nc.tensor.matmul(
                    bc[0:64, :], ones_row[0:1, 0:D],
                    rec[0:1, :], start=True, stop=True)
                bcs = rec_pool.tile([64, 512], f32, name="bcs", tag="bcs")
                nc.vector.tensor_copy(bcs[0:64, :], bc[0:64, :])
                nc.vector.tensor_tensor(
                    attn[kt][pr:pr + 64, 512 * w:512 * w + 512],
                    pv[0:64, :], bcs[0:64, :], Alu.mult)

            # ---------------- phase 5: proj + residual for this window ----
            for i in range(T):
                py = psum_acc.tile([128, 512], f32, name="py", tag="acc")
                # shares the 1-bank acc pool with bc; proj overlaps attention
                # of the next window only through this slot
                for k in range(T):
                    nc.tensor.matmul(
                        py[:], wTp[k][:, 128 * i:128 * i + 128],
                        attn[k][:, 512 * w:512 * w + 512],
                        start=(k == 0), stop=(k == T - 1))
                yo = yo_pool.tile([128, 512], f32, name="yo", tag="yo")
                nc.vector.scalar_tensor_tensor(
                    yo[:], py[:], pb[i][:, 0:1], xres[i][:, 512 * w:512 * w + 512],
                    Alu.add, Alu.add)
                nc.sync.dma_start(y[128 * i:128 * i + 128, 512 * w:512 * w + 512], yo[:])


def _build():
    import concourse.tile as tile
    from concourse import bacc, mybir

    nc = bacc.Bacc("TRN2", target_bir_lowering=False, debug=False)
    f32 = mybir.dt.float32
    io = {
        "xb": nc.dram_tensor("xb", [C, N], f32, kind="ExternalInput").ap(),
        "qkvw": nc.dram_tensor("qkvw", [3 * C, C], f32, kind="ExternalInput").ap(),
        "qkvb": nc.dram_tensor("qkvb", [3 * C], f32, kind="ExternalInput").ap(),
        "projw": nc.dram_tensor("projw", [C, C], f32, kind="ExternalInput").ap(),
        "projb": nc.dram_tensor("projb", [C], f32, kind="ExternalInput").ap(),
        "nw": nc.dram_tensor("nw", [C], f32, kind="ExternalInput").ap(),
        "nb": nc.dram_tensor("nb", [C], f32, kind="ExternalInput").ap(),
        "cid": nc.dram_tensor("cid", [128, 128], mybir.dt.bfloat16,
                              kind="ExternalInput").ap(),
        "cind": nc.dram_tensor("cind", [128, 2], f32, kind="ExternalInput").ap(),
        "cindT": nc.dram_tensor("cindT", [2, 128], f32, kind="ExternalInput").ap(),
        "y": nc.dram_tensor("y", [C, NQ], f32, kind="ExternalOutput").ap(),
    }
    if DEBUG:
        bf16 = mybir.dt.bfloat16
        io["dbg_ab"] = nc.dram_tensor("dbg_ab", [C, 8], f32, kind="ExternalOutput").ap()
        io["dbg_xn"] = nc.dram_tensor("dbg_xn", [C, 64], bf16, kind="ExternalOutput").ap()
        io["dbg_vt"] = nc.dram_tensor("dbg_vt", [128, 520], bf16, kind="ExternalOutput").ap()
        io["dbg_k"] = nc.dram_tensor("dbg_k", [C, 64], bf16, kind="ExternalOutput").ap()
        io["dbg_q"] = nc.dram_tensor("dbg_q", [C, 64], bf16, kind="ExternalOutput").ap()
        io["dbg_den"] = nc.dram_tensor("dbg_den", [8, 512], f32, kind="ExternalOutput").ap()
        io["dbg_pv"] = nc.dram_tensor("dbg_pv", [8, 512], f32, kind="ExternalOutput").ap()
    with tile.TileContext(nc) as tc:
        _emit(tc, io)
    nc.compile()
    return nc


def get_compiled():
    global _COMPILED
    if _COMPILED is None:
        _COMPILED = _build()
    return _COMPILED


def make_in_maps(x, norm_w, norm_b, qkv_w, qkv_b, proj_w, proj_b):
    import ml_dtypes

    xf = np.ascontiguousarray(np.asarray(x, np.float32)).reshape(2, C, N)
    ind = np.zeros((128, 2), np.float32)
    ind[0:64, 0] = 1.0
    ind[64:128, 1] = 1.0
    shared = {
        "cid": np.eye(128, dtype=ml_dtypes.bfloat16),
        "cind": ind,
        "cindT": np.ascontiguousarray(ind.T),
        "qkvw": np.ascontiguousarray(np.asarray(qkv_w, np.float32)),
        "qkvb": np.ascontiguousarray(np.asarray(qkv_b, np.float32)),
        "projw": np.ascontiguousarray(np.asarray(proj_w, np.float32)),
        "projb": np.ascontiguousarray(np.asarray(proj_b, np.float32)),
        "nw": np.ascontiguousarray(np.asarray(norm_w, np.float32)),
        "nb": np.ascontiguousarray(np.asarray(norm_b, np.float32)),
    }
    in_maps = []
    for core in range(8):
        bi, qs = core // 4, core % 4
        # rotate so this core's queries are always columns [0:NQ)
        xroll = np.concatenate(
            [xf[bi][:, qs * NQ:], xf[bi][:, :qs * NQ]], axis=1)
        m = dict(shared)
        m["xb"] = np.ascontiguousarray(xroll)
        in_maps.append(m)
    return in_maps


def assemble(results, x):
    y = np.zeros((2, C, N), np.float32)
    for core in range(8):
        bi, qs = core // 4, core % 4
        y[bi][:, qs * NQ:(qs + 1) * NQ] = results[core]["y"]
    return y.reshape(x.shape)


def kernel(x, norm_w, norm_b, qkv_w, qkv_b, proj_w, proj_b, **_ignored):
    from concourse import bass_utils

    nc = get_compiled()
    in_maps = make_in_maps(x, norm_w, norm_b, qkv_w, qkv_b, proj_w, proj_b)
    res = bass_utils.run_bass_kernel_spmd(nc, in_maps, core_ids=list(range(8)))
    return assemble(res.results, np.asarray(x))



# revision 3
# speedup vs baseline: 1.4338x; 1.4338x over previous
"""Trainium2 Bass kernel for nn_AttentionBlock (GroupNorm + MHA + proj + residual).

Full inputs in, full output out. Sharding: 8 cores = 2 batches x 4 query-slices.
Each core: GroupNorm over its batch image, q projection for its 1024 queries,
k/v projections over all 4096 keys, per-head attention (S^T = k^T q, softmax
along the PSUM partition axis via an appended ones-column in the PV matmul),
output projection and residual for its query slice.

v2 performance structure:
 - Warmup matmuls trip the PE HAM clock gate early (else everything runs at
   1.2 GHz instead of 2.4 GHz).
 - Heads are processed in pairs: head-even uses PE rows 0:63, head-odd rows
   64:127 (tile_position row groups) so the two S matmuls per key-tile run
   concurrently in the PE array and LDWEIGHTS overlaps streaming.
 - softmax exp alternates between ScalarE (real exp, even key-tiles) and
   VectorE (Schraudolph bit-trick exp -> bf16 bit pattern via an int16
   round, odd key-tiles), halving the exp bottleneck.
 - PV matmuls lag S by 2 key-tiles (3-deep PSUM ring) so the exp latency is
   off the PE critical path.

All matmuls run in bf16 with fp32 PSUM accumulation; softmax logits stay fp32.
"""
import numpy as np

C = 512          # channels
N = 4096         # pixels (64*64)
NQ = 1024        # queries per core
H = 8            # heads
D = 64           # head dim
T = 4            # 128-channel chunks
W = NQ // 512    # query windows of 512
MT = N // 128    # key m-tiles of 128
NGROUPS = 8
EPS = 1e-5
GELEM = (C // NGROUPS) * N   # elements per norm group

# Schraudolph fast-exp (bf16 bits via int16 round-to-nearest):
#   bits = round(raw * SCH_A + SCH_B);  bitcast(int16->bf16) ~ exp(0.125*raw)
# SCH_A = 0.125 * log2(e) * 128 ; SCH_B = 127*128 - 5.625 (max rel err 3.3%)
SCH_A = 23.083120654223414
SCH_B = 16250.375

N_WARM1 = 60     # warmup MMs before transposes (covers input DMA)
N_WARM2 = 14     # warmup MMs per groupnorm chunk
N_WARM3 = 24     # warmup MMs after groupnorm emission

_COMPILED = None


def _emit(tc, io):
    import concourse.bass as bass
    from concourse import mybir
    from contextlib import ExitStack

    nc = tc.nc
    f32 = mybir.dt.float32
    bf16 = mybir.dt.bfloat16
    i16 = mybir.dt.int16
    Alu = mybir.AluOpType
    Act = mybir.ActivationFunctionType

    xb, qkvw, qkvb, projw, projb, nw, nb, y = (
        io["xb"], io["qkvw"], io["qkvb"], io["projw"], io["projb"],
        io["nw"], io["nb"], io["y"])

    ctx = ExitStack()
    with ctx:
        # ---------------- pools ----------------
        # PSUM: sp ring 3x(128,1024) [6 banks] + pv 2x(128,512) [2 banks]
        left = ctx.enter_context(tc.tile_pool(name="left", bufs=1))
        psum_sp = ctx.enter_context(tc.tile_pool(name="psum_sp", bufs=3, space="PSUM"))
        psum_pv = ctx.enter_context(tc.tile_pool(name="psum_pv", bufs=2, space="PSUM"))

        right_ctx = ExitStack()
        xf_pool = right_ctx.enter_context(
            tc.tile_pool(name="xf_pool", bufs=1, side="right"))
        wstg_pool = right_ctx.enter_context(
            tc.tile_pool(name="wstg_pool", bufs=4, side="right"))
        scr_pool = right_ctx.enter_context(
            tc.tile_pool(name="scr_pool", bufs=2, side="right"))

        # ---------------- persistent tiles ----------------
        xn = [left.tile([128, N], bf16, name=f"xn{t}", tag=f"xn{t}") for t in range(T)]
        ksb = [left.tile([128, N], bf16, name=f"ksb{t}", tag=f"ksb{t}") for t in range(T)]
        qsb = [left.tile([128, NQ], bf16, name=f"qsb{t}", tag=f"qsb{t}") for t in range(T)]
        wTq = [left.tile([128, 1536], bf16, name=f"wTq{t}", tag=f"wTq{t}") for t in range(T)]
        wTp = [left.tile([128, C], bf16, name=f"wTp{t}", tag=f"wTp{t}") for t in range(T)]
        vb_bc = left.tile([128, C], f32, name="vb_bc", tag="vb_bc")
        ones_row = left.tile([1, D], f32, name="ones_row", tag="ones_row")
        qb = [left.tile([128, 1], f32, name=f"qb{i}", tag=f"qb{i}") for i in range(8)]
        pb = [left.tile([128, 1], f32, name=f"pb{i}", tag=f"pb{i}") for i in range(T)]
        nwt = [left.tile([128, 1], f32, name=f"nwt{t}", tag=f"nwt{t}") for t in range(T)]
        nbt = [left.tile([128, 1], f32, name=f"nbt{t}", tag=f"nbt{t}") for t in range(T)]
        stat = [left.tile([128, 2], f32, name=f"stat{t}", tag=f"stat{t}") for t in range(T)]
        gstat = [left.tile([128, 2], f32, name=f"gstat{t}", tag=f"gstat{t}") for t in range(T)]
        wu = left.tile([128, 512], bf16, name="wu", tag="wu")

        # ---------------- warmup: trip the HAM clock gate ----------------
        nc.vector.memset(wu[:], 0.125)
        wu_ps = psum_pv.tile([128, 512], f32, name="wu_ps", tag="pv")
        # preload the exp activation table while idle
        wu_exp = scr_pool.tile([1, 8], f32, name="wu_exp", tag="wu_exp")
        nc.scalar.activation(wu_exp[0:1, :], wu[0:1, 0:8], Act.Exp)

        def warm(n):
            for _ in range(n):
                nc.tensor.matmul(wu_ps[:], wu[:, 0:128], wu[:],
                                 start=True, stop=True)
        warm(N_WARM1)

        # ---------------- input DMAs ----------------
        xf = [xf_pool.tile([128, N], f32, name=f"xf{t}", tag=f"xf{t}") for t in range(T)]
        for t in range(T):
            for c4 in range(4):   # split across DMA queues
                nc.sync.dma_start(
                    xf[t][:, 1024 * c4:1024 * (c4 + 1)],
                    xb[128 * t:128 * (t + 1), 1024 * c4:1024 * (c4 + 1)])
            nc.sync.dma_start(nwt[t][:, 0:1], nw[128 * t:128 * (t + 1)])
            nc.sync.dma_start(nbt[t][:, 0:1], nb[128 * t:128 * (t + 1)])
            nc.sync.dma_start(pb[t][:, 0:1], projb[128 * t:128 * (t + 1)])
        for i in range(8):
            nc.sync.dma_start(qb[i][:, 0:1], qkvb[128 * i:128 * (i + 1)])
        # v bias broadcast to 128 partitions (stride-0 partition read)
        nc.gpsimd.dma_start(
            out=vb_bc[:],
            in_=bass.AP(tensor=qkvb.tensor, offset=1024, ap=[[0, 128], [1, C]]))
        nc.vector.memset(ones_row[0:1, :], 1.0)

        # weights: natural-layout contiguous DMA, cast to bf16, transpose
        # 128x128 blocks on the PE (identity trick) into wTq/wTp.
        ident = left.tile([128, 128], bf16, name="ident", tag="ident")
        nc.sync.dma_start(ident[:], io["cid"][:, :])
        ind = left.tile([128, 2], f32, name="ind", tag="ind")
        nc.sync.dma_start(ind[:], io["cind"][:, :])
        indT = left.tile([2, 128], f32, name="indT", tag="indT")
        nc.sync.dma_start(indT[0:2, :], io["cindT"][:, :])
        for i in range(12):   # qkv_w row-tiles
            wstg = wstg_pool.tile([128, C], f32, name="wstg", tag="wstg")
            nc.sync.dma_start(wstg[:], qkvw[128 * i:128 * (i + 1), :])
            wbf = wstg_pool.tile([128, C], bf16, name="wbf", tag="wbf")
            nc.vector.tensor_copy(wbf[:], wstg[:])
            for j in range(T):
                tp = psum_sp.tile([128, 1024], bf16, name="tp", tag="sp")
                nc.tensor.transpose(tp[:, 0:128], wbf[:, 128 * j:128 * (j + 1)], ident[:])
                nc.vector.tensor_copy(wTq[j][:, 128 * i:128 * (i + 1)], tp[:, 0:128])
        for i in range(4):    # proj_w row-tiles
            wstg = wstg_pool.tile([128, C], f32, name="wstg", tag="wstg")
            nc.sync.dma_start(wstg[:], projw[128 * i:128 * (i + 1), :])
            wbf = wstg_pool.tile([128, C], bf16, name="wbf", tag="wbf")
            nc.vector.tensor_copy(wbf[:], wstg[:])
            for j in range(T):
                tp = psum_sp.tile([128, 1024], bf16, name="tp", tag="sp")
                nc.tensor.transpose(tp[:, 0:128], wbf[:, 128 * j:128 * (j + 1)], ident[:])
                nc.vector.tensor_copy(wTp[j][:, 128 * i:128 * (i + 1)], tp[:, 0:128])

        # ---------------- phase 1: group stats + normalize ----------------
        for t in range(T):
            nc.vector.tensor_reduce(
                out=stat[t][:, 0:1], in_=xf[t][:], axis=mybir.AxisListType.X, op=Alu.add)
            sq_scr = scr_pool.tile([128, N], bf16, name="sq_scr", tag="sq_scr")
            nc.scalar.activation(
                sq_scr[:], xf[t][:], Act.Square, accum_out=stat[t][:, 1:2])
            # group-reduce over partitions via indicator matmuls
            gg_ps = psum_pv.tile([128, 512], f32, name="gg_ps", tag="pv")
            nc.tensor.matmul(gg_ps[0:2, 0:2], ind[:, 0:2], stat[t][:, 0:2],
                             start=True, stop=True)
            gg_sb = left.tile([2, 2], f32, name=f"gg_sb{t}", tag=f"gg_sb{t}")
            nc.vector.tensor_copy(gg_sb[0:2, :], gg_ps[0:2, 0:2])
            gb_ps = psum_pv.tile([128, 512], f32, name="gb_ps", tag="pv")
            nc.tensor.matmul(gb_ps[:, 0:2], indT[0:2, :], gg_sb[0:2, :],
                             start=True, stop=True)
            nc.vector.tensor_copy(gstat[t][:, 0:2], gb_ps[:, 0:2])
            # mean/var/rstd -> per-channel affine a,b
            mean_t = left.tile([128, 1], f32, name=f"mean{t}", tag=f"mean{t}")
            e2_t = left.tile([128, 1], f32, name=f"e2{t}", tag=f"e2{t}")
            var_t = left.tile([128, 1], f32, name=f"var{t}", tag=f"var{t}")
            std_t = left.tile([128, 1], f32, name=f"std{t}", tag=f"std{t}")
            a_t = left.tile([128, 1], f32, name=f"a{t}", tag=f"a{t}")
            b_t = left.tile([128, 1], f32, name=f"b{t}", tag=f"b{t}")
            inv = 1.0 / GELEM
            nc.vector.tensor_scalar(mean_t[:], gstat[t][:, 0:1], inv, None, Alu.mult)
            nc.vector.tensor_scalar(e2_t[:], gstat[t][:, 1:2], inv, None, Alu.mult)
            nc.vector.scalar_tensor_tensor(
                var_t[:], mean_t[:], -1.0, mean_t[:], Alu.mult, Alu.mult)
            nc.vector.scalar_tensor_tensor(
                var_t[:], e2_t[:], EPS, var_t[:], Alu.add, Alu.add)
            nc.scalar.activation(std_t[:], var_t[:], Act.Sqrt)
            nc.vector.reciprocal(a_t[:], std_t[:])
            nc.vector.tensor_tensor(a_t[:], a_t[:], nwt[t][:], Alu.mult)
            nc.vector.tensor_tensor(b_t[:], mean_t[:], a_t[:], Alu.mult)
            nc.vector.tensor_tensor(b_t[:], nbt[t][:], b_t[:], Alu.subtract)
            # normalize + cast
            nc.vector.tensor_scalar(
                xn[t][:], xf[t][:], a_t[:, 0:1], b_t[:, 0:1], Alu.mult, Alu.add)
            warm(N_WARM2)
        warm(N_WARM3)

        right_ctx.close()

        # ---------------- mid pools (reuse xf space) ----------------
        mid = ctx.enter_context(tc.tile_pool(name="mid", bufs=1))
        psa_pool = ctx.enter_context(tc.tile_pool(name="psa_pool", bufs=2))
        psb_pool = ctx.enter_context(tc.tile_pool(name="psb_pool", bufs=2))
        rec_pool = ctx.enter_context(tc.tile_pool(name="rec_pool", bufs=2))
        yo_pool = ctx.enter_context(tc.tile_pool(name="yo_pool", bufs=2))

        vT = mid.tile([128, MT * 520], bf16, name="vT", tag="vT")
        attn = [mid.tile([128, NQ], bf16, name=f"attn{t}", tag=f"attn{t}") for t in range(T)]
        xres = [mid.tile([128, NQ], f32, name=f"xres{t}", tag=f"xres{t}") for t in range(T)]
        for t in range(T):
            nc.sync.dma_start(xres[t][:], xb[128 * t:128 * (t + 1), 0:NQ])

        # ones columns of the augmented v^T (denominator trick)
        ones_view = vT[:].rearrange("p (m h x) -> p m h x", m=MT, x=65)[:, :, :, 64:65]
        nc.vector.memset(ones_view, 1.0)

        # ---------------- phase 3: projections ----------------
        # q: out rows 0..511 of qkv, only NQ query columns (2 windows per tile)
        for i in range(T):
            qp = psum_sp.tile([128, 1024], f32, name="qp", tag="sp")
            for w in range(W):
                for k in range(T):
                    nc.tensor.matmul(
                        qp[:, 512 * w:512 * w + 512],
                        wTq[k][:, 128 * i:128 * i + 128],
                        xn[k][:, 512 * w:512 * w + 512],
                        start=(k == 0), stop=(k == T - 1))
            nc.vector.tensor_scalar(
                qsb[i][:], qp[:], qb[i][:, 0:1], None, Alu.add)
        # k: out rows 512..1023, all N columns (2 windows per psum tile)
        for i in range(T):
            for w2 in range(4):
                kp = psum_sp.tile([128, 1024], f32, name="kp", tag="sp")
                for w in (2 * w2, 2 * w2 + 1):
                    for k in range(T):
                        nc.tensor.matmul(
                            kp[:, 512 * (w % 2):512 * (w % 2) + 512],
                            wTq[k][:, 512 + 128 * i:512 + 128 * i + 128],
                            xn[k][:, 512 * w:512 * w + 512],
                            start=(k == 0), stop=(k == T - 1))
                nc.vector.tensor_scalar(
                    ksb[i][:, 1024 * w2:1024 * (w2 + 1)], kp[:],
                    qb[4 + i][:, 0:1], None, Alu.add)
        # vT: (m, 512) per m-tile, two m-tiles per psum tile, strided into
        # the 65-column augmented layout
        for m2 in range(MT // 2):
            vp = psum_sp.tile([128, 1024], f32, name="vp", tag="sp")
            for mh in range(2):
                mt = 2 * m2 + mh
                for k in range(T):
                    nc.tensor.matmul(
                        vp[:, 512 * mh:512 * mh + 512],
                        xn[k][:, 128 * mt:128 * mt + 128],
                        wTq[k][:, 1024:1536],
                        start=(k == 0), stop=(k == T - 1))
            vbv = vb_bc[:].rearrange("p (h x) -> p h x", x=64)
            for mh in range(2):
                mt = 2 * m2 + mh
                dst = vT[:, 520 * mt:520 * mt + 520].rearrange(
                    "p (h x) -> p h x", x=65)[:, :, 0:64]
                src = vp[:, 512 * mh:512 * mh + 512].rearrange(
                    "p (h x) -> p h x", x=64)
                nc.vector.tensor_tensor(dst, src, vbv, Alu.add)

        # ---------------- phase 4: attention ----------------
        for w in range(W):
            for hp in range(4):
                h0, h1 = 2 * hp, 2 * hp + 1
                pv0 = psum_pv.tile([128, 512], f32, name="pv0", tag="pv")
                pv1 = psum_pv.tile([128, 512], f32, name="pv1", tag="pv")
                pend = []

                def flush_pv():
                    m, rhs0, rhs1 = pend.pop(0)
                    nc.tensor.matmul(
                        pv0[0:65, :], vT[:, 520 * m + 65 * h0:520 * m + 65 * h0 + 65],
                        rhs0, start=(m == 0), stop=(m == MT - 1))
                    nc.tensor.matmul(
                        pv1[0:65, :], vT[:, 520 * m + 65 * h1:520 * m + 65 * h1 + 65],
                        rhs1, start=(m == 0), stop=(m == MT - 1))

                for m in range(MT):
                    sp = psum_sp.tile([128, 1024], f32, name="sp", tag="sp")
                    # S pair: head-even on PE rows 0:63, head-odd on rows
                    # 64:127 -> the two matmuls run concurrently
                    nc.tensor.matmul(
                        sp[:, 0:512],
                        ksb[hp][0:64, 128 * m:128 * m + 128],
                        qsb[hp][0:64, 512 * w:512 * w + 512],
                        start=True, stop=True)
                    nc.tensor.matmul(
                        sp[:, 512:1024],
                        ksb[hp][64:128, 128 * m:128 * m + 128],
                        qsb[hp][64:128, 512 * w:512 * w + 512],
                        start=True, stop=True)
                    if m % 2 == 0:
                        ps = psa_pool.tile([128, 1024], bf16, name="psa", tag="psa")
                        nc.scalar.activation(ps[:], sp[:], Act.Exp, scale=0.125)
                        rhs0, rhs1 = ps[:, 0:512], ps[:, 512:1024]
                    else:
                        ps = psb_pool.tile([128, 1024], i16, name="psb", tag="psb")
                        nc.vector.tensor_scalar(
                            ps[:], sp[:], SCH_A, SCH_B, Alu.mult, Alu.add)
                        rhs0 = ps[:, 0:512].bitcast(mybir.dt.bfloat16)
                        rhs1 = ps[:, 512:1024].bitcast(mybir.dt.bfloat16)
                    pend.append((m, rhs0, rhs1))
                    if len(pend) >= 3:
                        flush_pv()
                while pend:
                    flush_pv()

                # per-head tails: denominator, reciprocal, broadcast, scale
                for h, pv in ((h0, pv0), (h1, pv1)):
                    kt, pr = hp, 64 * (h % 2)
                    # NOTE: reciprocal_approx_* mis-handles nonzero partition
                    # offsets on HW, so stage the denominator row at
                    # partition 0 first
                    dnm = rec_pool.tile([1, 512], f32, name="dnm", tag="dnm")
                    nc.vector.tensor_copy(dnm[0:1, :], pv[64:65, :])
                    rec = rec_pool.tile([1, 512], f32, name="rec", tag="rec")
                    rscr = rec_pool.tile([1, 512], f32, name="rscr", tag="rscr")
                    nc.vector.reciprocal_approx_accurate(
                        rec[0:1, :], dnm[0:1, :], rscr[0:1, :])
                    bc = psum_sp.tile([128, 1024], f32, name="bc", tag="sp")
                    nc.tensor.matmul(
                        bc[0:64, 0:512], ones_row[0:1, 0:D],
                        rec[0:1, :], start=True, stop=True)
                    bcs = rec_pool.tile([64, 512], f32, name="bcs", tag="bcs")
                    nc.scalar.copy(bcs[0:64, :], bc[0:64, 0:512])
                    nc.vector.tensor_tensor(
                        attn[kt][pr:pr + 64, 512 * w:512 * w + 512],
                        pv[0:64, :], bcs[0:64, :], Alu.mult)

            # ---------------- phase 5: proj + residual for this window ----
            for i in range(T):
                py = psum_pv.tile([128, 512], f32, name="py", tag="pv")
                for k in range(T):
                    nc.tensor.matmul(
                        py[:], wTp[k][:, 128 * i:128 * i + 128],
                        attn[k][:, 512 * w:512 * w + 512],
                        start=(k == 0), stop=(k == T - 1))
                yo = yo_pool.tile([128, 512], f32, name="yo", tag="yo")
                nc.vector.scalar_tensor_tensor(
                    yo[:], py[:], pb[i][:, 0:1], xres[i][:, 512 * w:512 * w + 512],
                    Alu.add, Alu.add)
                nc.sync.dma_start(y[128 * i:128 * i + 128, 512 * w:512 * w + 512], yo[:])


def _build():
    import concourse.tile as tile
    from concourse import bacc, mybir

    nc = bacc.Bacc("TRN2", target_bir_lowering=False, debug=False)
    f32 = mybir.dt.float32
    io = {
        "xb": nc.dram_tensor("xb", [C, N], f32, kind="ExternalInput").ap(),
        "qkvw": nc.dram_tensor("qkvw", [3 * C, C], f32, kind="ExternalInput").ap(),
        "qkvb": nc.dram_tensor("qkvb", [3 * C], f32, kind="ExternalInput").ap(),
        "projw": nc.dram_tensor("projw", [C, C], f32, kind="ExternalInput").ap(),
        "projb": nc.dram_tensor("projb", [C], f32, kind="ExternalInput").ap(),
        "nw": nc.dram_tensor("nw", [C], f32, kind="ExternalInput").ap(),
        "nb": nc.dram_tensor("nb", [C], f32, kind="ExternalInput").ap(),
        "cid": nc.dram_tensor("cid", [128, 128], mybir.dt.bfloat16,
                              kind="ExternalInput").ap(),
        "cind": nc.dram_tensor("cind", [128, 2], f32, kind="ExternalInput").ap(),
        "cindT": nc.dram_tensor("cindT", [2, 128], f32, kind="ExternalInput").ap(),
        "y": nc.dram_tensor("y", [C, NQ], f32, kind="ExternalOutput").ap(),
    }
    with tile.TileContext(nc) as tc:
        _emit(tc, io)
    nc.compile()
    return nc


def get_compiled():
    global _COMPILED
    if _COMPILED is None:
        _COMPILED = _build()
    return _COMPILED


def make_in_maps(x, norm_w, norm_b, qkv_w, qkv_b, proj_w, proj_b):
    import ml_dtypes

    xf = np.ascontiguousarray(np.asarray(x, np.float32)).reshape(2, C, N)
    ind = np.zeros((128, 2), np.float32)
    ind[0:64, 0] = 1.0
    ind[64:128, 1] = 1.0
    shared = {
        "cid": np.eye(128, dtype=ml_dtypes.bfloat16),
        "cind": ind,
        "cindT": np.ascontiguousarray(ind.T),
        "qkvw": np.ascontiguousarray(np.asarray(qkv_w, np.float32)),
        "qkvb": np.ascontiguousarray(np.asarray(qkv_b, np.float32)),
        "projw": np.ascontiguousarray(np.asarray(proj_w, np.float32)),
        "projb": np.ascontiguousarray(np.asarray(proj_b, np.float32)),
        "nw": np.ascontiguousarray(np.asarray(norm_w, np.float32)),
        "nb": np.ascontiguousarray(np.asarray(norm_b, np.float32)),
    }
    in_maps = []
    for core in range(8):
        bi, qs = core // 4, core % 4
        # rotate so this core's queries are always columns [0:NQ)
        xroll = np.concatenate(
            [xf[bi][:, qs * NQ:], xf[bi][:, :qs * NQ]], axis=1)
        m = dict(shared)
        m["xb"] = np.ascontiguousarray(xroll)
        in_maps.append(m)
    return in_maps


def assemble(results, x):
    y = np.zeros((2, C, N), np.float32)
    for core in range(8):
        bi, qs = core // 4, core % 4
        y[bi][:, qs * NQ:(qs + 1) * NQ] = results[core]["y"]
    return y.reshape(x.shape)


def kernel(x, norm_w, norm_b, qkv_w, qkv_b, proj_w, proj_b, **_ignored):
    from concourse import bass_utils

    nc = get_compiled()
    in_maps = make_in_maps(x, norm_w, norm_b, qkv_w, qkv_b, proj_w, proj_b)
    res = bass_utils.run_bass_kernel_spmd(nc, in_maps, core_ids=list(range(8)))
    return assemble(res.results, np.asarray(x))


# revision 13
# speedup vs baseline: 1.8854x; 1.3150x over previous
"""Trainium2 Bass kernel for nn_AttentionBlock (GroupNorm + MHA + proj + residual).

Full inputs in, full output out. Sharding: 8 cores = 2 batches x 4 query-slices.
Each core: GroupNorm over its batch image, q projection for its 1024 queries,
k/v projections over all 4096 keys, per-head attention (S^T = k^T q, softmax
along the PSUM partition axis via an appended ones-column in the PV matmul),
output projection and residual for its query slice.

v2 performance structure:
 - Warmup matmuls trip the PE HAM clock gate early (else everything runs at
   1.2 GHz instead of 2.4 GHz).
 - Heads are processed in pairs: head-even uses PE rows 0:63, head-odd rows
   64:127 (tile_position row groups) so the two S matmuls per key-tile run
   concurrently in the PE array and LDWEIGHTS overlaps streaming.
 - softmax exp alternates between ScalarE (real exp, even key-tiles) and
   VectorE (Schraudolph bit-trick exp -> bf16 bit pattern via an int16
   round, odd key-tiles), halving the exp bottleneck.
 - PV matmuls lag S by 2 key-tiles (3-deep PSUM ring) so the exp latency is
   off the PE critical path.

All matmuls run in bf16 with fp32 PSUM accumulation; softmax logits stay fp32.
"""
import numpy as np

C = 512          # channels
N = 4096         # pixels (64*64)
NQ = 1024        # queries per core
H = 8            # heads
D = 64           # head dim
T = 4            # 128-channel chunks
W = NQ // 512    # query windows of 512
MT = N // 128    # key m-tiles of 128
NGROUPS = 8
EPS = 1e-5
GELEM = (C // NGROUPS) * N   # elements per norm group

# Schraudolph fast-exp (fp8e5m2 bits via int8 round-to-nearest):
#   bits = round(raw * SA5 + SB5);  bitcast(int8->fp8e5) ~ exp(0.125*raw)
# SA5 = 0.125 * log2(e) * 4 ; SB5 = 15*4 - 0.21875 (max rel err 11.7%,
# same order as the direct e5m2 quantization of a true exp)
SA5 = 0.7213475204444817
SB5 = 59.78125

N_WARM1 = 60     # warmup MMs before transposes (covers input DMA)
N_WARM2 = 14     # warmup MMs per groupnorm chunk
N_WARM3 = 24     # warmup MMs after groupnorm emission

_COMPILED = None


def _emit(tc, io):
    import concourse.bass as bass
    from concourse import mybir
    from contextlib import ExitStack

    nc = tc.nc
    f32 = mybir.dt.float32
    bf16 = mybir.dt.bfloat16
    i8 = mybir.dt.int8
    f8 = mybir.dt.float8e5
    Alu = mybir.AluOpType
    Act = mybir.ActivationFunctionType

    xb, qkvw, qkvb, projw, projb, nw, nb, y = (
        io["xb"], io["qkvw"], io["qkvb"], io["projw"], io["projb"],
        io["nw"], io["nb"], io["y"])

    ctx = ExitStack()
    with ctx:
        # ---------------- pools ----------------
        # PSUM: sp ring 3x(128,1024) [6 banks] + pv 2x(128,512) [2 banks]
        left = ctx.enter_context(tc.tile_pool(name="left", bufs=1))
        psum_sp = ctx.enter_context(tc.tile_pool(name="psum_sp", bufs=3, space="PSUM"))
        psum_pv = ctx.enter_context(tc.tile_pool(name="psum_pv", bufs=2, space="PSUM"))

        right_ctx = ExitStack()
        xf_pool = right_ctx.enter_context(
            tc.tile_pool(name="xf_pool", bufs=1, side="right"))
        wstg_pool = right_ctx.enter_context(
            tc.tile_pool(name="wstg_pool", bufs=4, side="right"))
        scr_pool = right_ctx.enter_context(
            tc.tile_pool(name="scr_pool", bufs=2, side="right"))

        # ---------------- persistent tiles ----------------
        xn = [left.tile([128, N], bf16, name=f"xn{t}", tag=f"xn{t}") for t in range(T)]
        ksb = [left.tile([128, N], bf16, name=f"ksb{t}", tag=f"ksb{t}") for t in range(T)]
        qsb = [left.tile([128, NQ], bf16, name=f"qsb{t}", tag=f"qsb{t}") for t in range(T)]
        wTq = [left.tile([128, 1536], bf16, name=f"wTq{t}", tag=f"wTq{t}") for t in range(T)]
        wTp = [left.tile([128, C], bf16, name=f"wTp{t}", tag=f"wTp{t}") for t in range(T)]
        vb_bc = left.tile([128, C], f32, name="vb_bc", tag="vb_bc")
        ones_row = left.tile([1, D], f32, name="ones_row", tag="ones_row")
        qb = [left.tile([128, 1], f32, name=f"qb{i}", tag=f"qb{i}") for i in range(8)]
        pb = [left.tile([128, 1], f32, name=f"pb{i}", tag=f"pb{i}") for i in range(T)]
        nwt = [left.tile([128, 1], f32, name=f"nwt{t}", tag=f"nwt{t}") for t in range(T)]
        nbt = [left.tile([128, 1], f32, name=f"nbt{t}", tag=f"nbt{t}") for t in range(T)]
        stat = [left.tile([128, 2], f32, name=f"stat{t}", tag=f"stat{t}") for t in range(T)]
        gstat = [left.tile([128, 2], f32, name=f"gstat{t}", tag=f"gstat{t}") for t in range(T)]
        wu = left.tile([128, 512], bf16, name="wu", tag="wu")

        # ---------------- warmup: trip the HAM clock gate ----------------
        nc.vector.memset(wu[:], 0.125)
        wu_ps = psum_pv.tile([128, 512], f32, name="wu_ps", tag="pv")
        # preload the exp activation table while idle
        wu_exp = scr_pool.tile([1, 8], f32, name="wu_exp", tag="wu_exp")
        nc.scalar.activation(wu_exp[0:1, :], wu[0:1, 0:8], Act.Exp)

        def warm(n):
            for _ in range(n):
                nc.tensor.matmul(wu_ps[:], wu[:, 0:128], wu[:],
                                 start=True, stop=True)
        warm(N_WARM1)

        # ---------------- input DMAs ----------------
        xf = [xf_pool.tile([128, N], f32, name=f"xf{t}", tag=f"xf{t}") for t in range(T)]
        for t in range(T):
            for c4 in range(4):   # split across DMA queues
                nc.sync.dma_start(
                    xf[t][:, 1024 * c4:1024 * (c4 + 1)],
                    xb[128 * t:128 * (t + 1), 1024 * c4:1024 * (c4 + 1)])
            nc.sync.dma_start(nwt[t][:, 0:1], nw[128 * t:128 * (t + 1)])
            nc.sync.dma_start(nbt[t][:, 0:1], nb[128 * t:128 * (t + 1)])
            nc.sync.dma_start(pb[t][:, 0:1], projb[128 * t:128 * (t + 1)])
        for i in range(8):
            nc.sync.dma_start(qb[i][:, 0:1], qkvb[128 * i:128 * (i + 1)])
        # v bias broadcast to 128 partitions (stride-0 partition read)
        nc.gpsimd.dma_start(
            out=vb_bc[:],
            in_=bass.AP(tensor=qkvb.tensor, offset=1024, ap=[[0, 128], [1, C]]))
        nc.vector.memset(ones_row[0:1, :], 1.0)

        # weights: natural-layout contiguous DMA, cast to bf16, transpose
        # 128x128 blocks on the PE (identity trick) into wTq/wTp.
        ident = left.tile([128, 128], bf16, name="ident", tag="ident")
        nc.sync.dma_start(ident[:], io["cid"][:, :])
        ind = left.tile([128, 2], f32, name="ind", tag="ind")
        nc.sync.dma_start(ind[:], io["cind"][:, :])
        indT = left.tile([2, 128], f32, name="indT", tag="indT")
        nc.sync.dma_start(indT[0:2, :], io["cindT"][:, :])
        for i in range(12):   # qkv_w row-tiles
            wstg = wstg_pool.tile([128, C], f32, name="wstg", tag="wstg")
            nc.sync.dma_start(wstg[:], qkvw[128 * i:128 * (i + 1), :])
            wbf = wstg_pool.tile([128, C], bf16, name="wbf", tag="wbf")
            nc.vector.tensor_copy(wbf[:], wstg[:])
            for j in range(T):
                tp = psum_sp.tile([128, 1024], bf16, name="tp", tag="sp")
                nc.tensor.transpose(tp[:, 0:128], wbf[:, 128 * j:128 * (j + 1)], ident[:])
                nc.vector.tensor_copy(wTq[j][:, 128 * i:128 * (i + 1)], tp[:, 0:128])
        for i in range(4):    # proj_w row-tiles
            wstg = wstg_pool.tile([128, C], f32, name="wstg", tag="wstg")
            nc.sync.dma_start(wstg[:], projw[128 * i:128 * (i + 1), :])
            wbf = wstg_pool.tile([128, C], bf16, name="wbf", tag="wbf")
            nc.vector.tensor_copy(wbf[:], wstg[:])
            for j in range(T):
                tp = psum_sp.tile([128, 1024], bf16, name="tp", tag="sp")
                nc.tensor.transpose(tp[:, 0:128], wbf[:, 128 * j:128 * (j + 1)], ident[:])
                nc.vector.tensor_copy(wTp[j][:, 128 * i:128 * (i + 1)], tp[:, 0:128])

        # ---------------- phase 1: group stats + normalize ----------------
        for t in range(T):
            nc.vector.tensor_reduce(
                out=stat[t][:, 0:1], in_=xf[t][:], axis=mybir.AxisListType.X, op=Alu.add)
            sq_scr = scr_pool.tile([128, N], bf16, name="sq_scr", tag="sq_scr")
            nc.scalar.activation(
                sq_scr[:], xf[t][:], Act.Square, accum_out=stat[t][:, 1:2])
            # group-reduce over partitions via indicator matmuls
            gg_ps = psum_pv.tile([128, 512], f32, name="gg_ps", tag="pv")
            nc.tensor.matmul(gg_ps[0:2, 0:2], ind[:, 0:2], stat[t][:, 0:2],
                             start=True, stop=True)
            gg_sb = left.tile([2, 2], f32, name=f"gg_sb{t}", tag=f"gg_sb{t}")
            nc.vector.tensor_copy(gg_sb[0:2, :], gg_ps[0:2, 0:2])
            gb_ps = psum_pv.tile([128, 512], f32, name="gb_ps", tag="pv")
            nc.tensor.matmul(gb_ps[:, 0:2], indT[0:2, :], gg_sb[0:2, :],
                             start=True, stop=True)
            nc.vector.tensor_copy(gstat[t][:, 0:2], gb_ps[:, 0:2])
            # mean/var/rstd -> per-channel affine a,b
            mean_t = left.tile([128, 1], f32, name=f"mean{t}", tag=f"mean{t}")
            e2_t = left.tile([128, 1], f32, name=f"e2{t}", tag=f"e2{t}")
            var_t = left.tile([128, 1], f32, name=f"var{t}", tag=f"var{t}")
            std_t = left.tile([128, 1], f32, name=f"std{t}", tag=f"std{t}")
            a_t = left.tile([128, 1], f32, name=f"a{t}", tag=f"a{t}")
            b_t = left.tile([128, 1], f32, name=f"b{t}", tag=f"b{t}")
            inv = 1.0 / GELEM
            nc.vector.tensor_scalar(mean_t[:], gstat[t][:, 0:1], inv, None, Alu.mult)
            nc.vector.tensor_scalar(e2_t[:], gstat[t][:, 1:2], inv, None, Alu.mult)
            nc.vector.scalar_tensor_tensor(
                var_t[:], mean_t[:], -1.0, mean_t[:], Alu.mult, Alu.mult)
            nc.vector.scalar_tensor_tensor(
                var_t[:], e2_t[:], EPS, var_t[:], Alu.add, Alu.add)
            nc.scalar.activation(std_t[:], var_t[:], Act.Sqrt)
            nc.vector.reciprocal(a_t[:], std_t[:])
            nc.vector.tensor_tensor(a_t[:], a_t[:], nwt[t][:], Alu.mult)
            nc.vector.tensor_tensor(b_t[:], mean_t[:], a_t[:], Alu.mult)
            nc.vector.tensor_tensor(b_t[:], nbt[t][:], b_t[:], Alu.subtract)
            # normalize + cast
            nc.vector.tensor_scalar(
                xn[t][:], xf[t][:], a_t[:, 0:1], b_t[:, 0:1], Alu.mult, Alu.add)
            warm(N_WARM2)
        warm(N_WARM3)

        right_ctx.close()

        # ---------------- mid pools (reuse xf space) ----------------
        mid = ctx.enter_context(tc.tile_pool(name="mid", bufs=1))
        ps8_pool = ctx.enter_context(tc.tile_pool(name="ps8_pool", bufs=4))
        rec_pool = ctx.enter_context(tc.tile_pool(name="rec_pool", bufs=2))
        yo_pool = ctx.enter_context(tc.tile_pool(name="yo_pool", bufs=2))

        # fp8e5 v^T for DoubleRow PV: per m-pair m2 and head h, 160 cols =
        # two 80-wide blocks (64 v + ones col @64 + 15 zero pad), one per
        # m-tile of the pair
        vT8 = mid.tile([128, (MT // 2) * 1280], f8, name="vT8", tag="vT8")
        attn = [mid.tile([128, NQ], bf16, name=f"attn{t}", tag=f"attn{t}") for t in range(T)]
        xres = [mid.tile([128, NQ], f32, name=f"xres{t}", tag=f"xres{t}") for t in range(T)]
        for t in range(T):
            nc.sync.dma_start(xres[t][:], xb[128 * t:128 * (t + 1), 0:NQ])

        # ones + zero-pad columns of the augmented v^T (denominator trick)
        v80 = vT8[:].rearrange("p (n x) -> p n x", x=80)
        nc.vector.memset(v80[:, :, 64:65], 1.0)
        nc.vector.memset(v80[:, :, 65:80], 0.0)

        # ---------------- phase 3: projections ----------------
        # q: out rows 0..511 of qkv, only NQ query columns (2 windows per tile)
        for i in range(T):
            qp = psum_sp.tile([128, 1024], f32, name="qp", tag="sp")
            for w in range(W):
                for k in range(T):
                    nc.tensor.matmul(
                        qp[:, 512 * w:512 * w + 512],
                        wTq[k][:, 128 * i:128 * i + 128],
                        xn[k][:, 512 * w:512 * w + 512],
                        start=(k == 0), stop=(k == T - 1))
            nc.vector.tensor_scalar(
                qsb[i][:], qp[:], qb[i][:, 0:1], None, Alu.add)
        # k: out rows 512..1023, all N columns (2 windows per psum tile)
        for i in range(T):
            for w2 in range(4):
                kp = psum_sp.tile([128, 1024], f32, name="kp", tag="sp")
                for w in (2 * w2, 2 * w2 + 1):
                    for k in range(T):
                        nc.tensor.matmul(
                            kp[:, 512 * (w % 2):512 * (w % 2) + 512],
                            wTq[k][:, 512 + 128 * i:512 + 128 * i + 128],
                            xn[k][:, 512 * w:512 * w + 512],
                            start=(k == 0), stop=(k == T - 1))
                nc.vector.tensor_scalar(
                    ksb[i][:, 1024 * w2:1024 * (w2 + 1)], kp[:],
                    qb[4 + i][:, 0:1], None, Alu.add)
        # vT: (m, 512) per m-tile, two m-tiles per psum tile, strided into
        # the 65-column augmented layout
        for m2 in range(MT // 2):
            vp = psum_sp.tile([128, 1024], f32, name="vp", tag="sp")
            for mh in range(2):
                mt = 2 * m2 + mh
                for k in range(T):
                    nc.tensor.matmul(
                        vp[:, 512 * mh:512 * mh + 512],
                        xn[k][:, 128 * mt:128 * mt + 128],
                        wTq[k][:, 1024:1536],
                        start=(k == 0), stop=(k == T - 1))
            vbv = vb_bc[:].rearrange("p (h x) -> p h x", x=64)
            vdst = vT8[:, 1280 * m2:1280 * (m2 + 1)].rearrange(
                "p (h j x) -> p h j x", h=8, j=2, x=80)
            for mh in range(2):
                dst = vdst[:, :, mh, 0:64]
                src = vp[:, 512 * mh:512 * mh + 512].rearrange(
                    "p (h x) -> p h x", x=64)
                nc.vector.tensor_tensor(dst, src, vbv, Alu.add)

        # ---------------- phase 4: attention ----------------
        for w in range(W):
            for hp in range(4):
                h0, h1 = 2 * hp, 2 * hp + 1
                pv0 = psum_pv.tile([128, 512], f32, name="pv0", tag="pv")
                pv1 = psum_pv.tile([128, 512], f32, name="pv1", tag="pv")
                pend = []

                def flush_pv():
                    m2, ps8 = pend.pop(0)
                    st, sto = (m2 == 0), (m2 == MT // 2 - 1)
                    vb = 1280 * m2
                    for h, pv in ((0, pv0), (1, pv1)):
                        lhs = vT8[:, vb + 160 * (2 * hp + h):
                                  vb + 160 * (2 * hp + h) + 160].rearrange(
                            "p (j x) -> p j x", j=2)
                        rhs = ps8[:, 1024 * h:1024 * h + 1024].rearrange(
                            "p (j x) -> p j x", j=2)
                        nc.tensor.matmul(
                            pv[0:80, :], lhs, rhs, start=st, stop=sto,
                            perf_mode=mybir.MatmulPerfMode.DoubleRow)

                ps8 = None
                for m in range(MT):
                    sp = psum_sp.tile([128, 1024], f32, name="sp", tag="sp")
                    # S pair: head-even on PE rows 0:63, head-odd on rows
                    # 64:127 (concurrent row groups, shared LDW window)
                    nc.tensor.matmul(
                        sp[:, 0:512],
                        ksb[hp][0:64, 128 * m:128 * m + 128],
                        qsb[hp][0:64, 512 * w:512 * w + 512],
                        start=True, stop=True)
                    nc.tensor.matmul(
                        sp[:, 512:1024],
                        ksb[hp][64:128, 128 * m:128 * m + 128],
                        qsb[hp][64:128, 512 * w:512 * w + 512],
                        start=True, stop=True)
                    m2, mh = divmod(m, 2)
                    if mh == 0:
                        ps8 = ps8_pool.tile([128, 2048], f8, name="ps8", tag="ps8")
                    # exp of this m-tile for both heads into the fp8 pair
                    # tile: cols [1024h + 512mh : +512] = (head h, m-half mh)
                    outv = ps8[:].rearrange(
                        "p (H x) -> p H x", H=2)[:, :, 512 * mh:512 * mh + 512]
                    inv = sp[:].rearrange("p (h x) -> p h x", h=2)
                    # ACT handles even m-tiles plus one odd (engine balance);
                    # DVE does the rest via the int8 Schraudolph bit trick
                    if mh == 0 or m == 13:
                        nc.scalar.activation(outv, inv, Act.Exp, scale=0.125)
                    else:
                        nc.vector.tensor_scalar(
                            outv.bitcast(i8), inv, SA5, SB5, Alu.mult, Alu.add)
                    if mh == 1:
                        pend.append((m2, ps8))
                        if len(pend) >= 3:
                            flush_pv()
                while pend:
                    flush_pv()

                # tails: both denominators stacked on partition 0, one
                # reciprocal, two broadcast matmuls (out partitions 0:64
                # and 64:128), per-head scale
                dnq = rec_pool.tile([1, 1024], f32, name="dnq", tag="dnq")
                nc.scalar.copy(dnq[0:1, 0:512], pv0[64:65, :])
                nc.scalar.copy(dnq[0:1, 512:1024], pv1[64:65, :])
                rcq = rec_pool.tile([1, 1024], f32, name="rcq", tag="rcq")
                rscr = rec_pool.tile([1, 1024], f32, name="rscr", tag="rscr")
                nc.vector.reciprocal_approx_accurate(
                    rcq[0:1, :], dnq[0:1, :], rscr[0:1, :])
                bc = psum_sp.tile([128, 1024], f32, name="bc", tag="sp")
                nc.tensor.matmul(
                    bc[0:64, 0:512], ones_row[0:1, 0:D],
                    rcq[0:1, 0:512], start=True, stop=True)
                nc.tensor.matmul(
                    bc[64:128, 0:512], ones_row[0:1, 0:D],
                    rcq[0:1, 512:1024], start=True, stop=True)
                bcs0 = rec_pool.tile([64, 512], f32, name="bcs0", tag="bcs0")
                bcs1 = rec_pool.tile([64, 512], f32, name="bcs1", tag="bcs1")
                nc.scalar.copy(bcs0[0:64, :], bc[0:64, 0:512])
                nc.scalar.copy(bcs1[0:64, :], bc[64:128, 0:512])
                nc.vector.tensor_tensor(
                    attn[hp][0:64, 512 * w:512 * w + 512],
                    pv0[0:64, :], bcs0[0:64, :], Alu.mult)
                nc.vector.tensor_tensor(
                    attn[hp][64:128, 512 * w:512 * w + 512],
                    pv1[0:64, :], bcs1[0:64, :], Alu.mult)

            # ---------------- phase 5: proj + residual for this window ----
            for i in range(T):
                py = psum_pv.tile([128, 512], f32, name="py", tag="pv")
                for k in range(T):
                    nc.tensor.matmul(
                        py[:], wTp[k][:, 128 * i:128 * i + 128],
                        attn[k][:, 512 * w:512 * w + 512],
                        start=(k == 0), stop=(k == T - 1))
                yo = yo_pool.tile([128, 512], f32, name="yo", tag="yo")
                nc.vector.scalar_tensor_tensor(
                    yo[:], py[:], pb[i][:, 0:1], xres[i][:, 512 * w:512 * w + 512],
                    Alu.add, Alu.add)
                nc.sync.dma_start(y[128 * i:128 * i + 128, 512 * w:512 * w + 512], yo[:])


def _build():
    import concourse.tile as tile
    from concourse import bacc, mybir

    nc = bacc.Bacc("TRN2", target_bir_lowering=False, debug=False)
    f32 = mybir.dt.float32
    io = {
        "xb": nc.dram_tensor("xb", [C, N], f32, kind="ExternalInput").ap(),
        "qkvw": nc.dram_tensor("qkvw", [3 * C, C], f32, kind="ExternalInput").ap(),
        "qkvb": nc.dram_tensor("qkvb", [3 * C], f32, kind="ExternalInput").ap(),
        "projw": nc.dram_tensor("projw", [C, C], f32, kind="ExternalInput").ap(),
        "projb": nc.dram_tensor("projb", [C], f32, kind="ExternalInput").ap(),
        "nw": nc.dram_tensor("nw", [C], f32, kind="ExternalInput").ap(),
        "nb": nc.dram_tensor("nb", [C], f32, kind="ExternalInput").ap(),
        "cid": nc.dram_tensor("cid", [128, 128], mybir.dt.bfloat16,
                              kind="ExternalInput").ap(),
        "cind": nc.dram_tensor("cind", [128, 2], f32, kind="ExternalInput").ap(),
        "cindT": nc.dram_tensor("cindT", [2, 128], f32, kind="ExternalInput").ap(),
        "y": nc.dram_tensor("y", [C, NQ], f32, kind="ExternalOutput").ap(),
    }
    with tile.TileContext(nc) as tc:
        _emit(tc, io)
    nc.compile()
    return nc


def get_compiled():
    global _COMPILED
    if _COMPILED is None:
        _COMPILED = _build()
    return _COMPILED


def make_in_maps(x, norm_w, norm_b, qkv_w, qkv_b, proj_w, proj_b):
    import ml_dtypes

    xf = np.ascontiguousarray(np.asarray(x, np.float32)).reshape(2, C, N)
    ind = np.zeros((128, 2), np.float32)
    ind[0:64, 0] = 1.0
    ind[64:128, 1] = 1.0
    shared = {
        "cid": np.eye(128, dtype=ml_dtypes.bfloat16),
        "cind": ind,
        "cindT": np.ascontiguousarray(ind.T),
        "qkvw": np.ascontiguousarray(np.asarray(qkv_w, np.float32)),
        "qkvb": np.ascontiguousarray(np.asarray(qkv_b, np.float32)),
        "projw": np.ascontiguousarray(np.asarray(proj_w, np.float32)),
        "projb": np.ascontiguousarray(np.asarray(proj_b, np.float32)),
        "nw": np.ascontiguousarray(np.asarray(norm_w, np.float32)),
        "nb": np.ascontiguousarray(np.asarray(norm_b, np.float32)),
    }
    in_maps = []
    for core in range(8):
        bi, qs = core // 4, core % 4
        # rotate so this core's queries are always columns [0:NQ)
        xroll = np.concatenate(
            [xf[bi][:, qs * NQ:], xf[bi][:, :qs * NQ]], axis=1)
        m = dict(shared)
        m["xb"] = np.ascontiguousarray(xroll)
        in_maps.append(m)
    return in_maps


def assemble(results, x):
    y = np.zeros((2, C, N), np.float32)
    for core in range(8):
        bi, qs = core // 4, core % 4
        y[bi][:, qs * NQ:(qs + 1) * NQ] = results[core]["y"]
    return y.reshape(x.shape)


def kernel(x, norm_w, norm_b, qkv_w, qkv_b, proj_w, proj_b, **_ignored):
    from concourse import bass_utils

    nc = get_compiled()
    in_maps = make_in_maps(x, norm_w, norm_b, qkv_w, qkv_b, proj_w, proj_b)
    res = bass_utils.run_bass_kernel_spmd(nc, in_maps, core_ids=list(range(8)))
    return assemble(res.results, np.asarray(x))


# revision 20
# speedup vs baseline: 1.9736x; 1.0468x over previous
"""Trainium2 Bass kernel for nn_AttentionBlock (GroupNorm + MHA + proj + residual).

Full inputs in, full output out. Sharding: 8 cores = 2 batches x 4 query-slices.
Each core: GroupNorm over its batch image, q projection for its 1024 queries,
k/v projections over all 4096 keys, per-head attention (S^T = k^T q, softmax
along the PSUM partition axis via an appended ones-column in the PV matmul),
output projection and residual for its query slice.

v2 performance structure:
 - Warmup matmuls trip the PE HAM clock gate early (else everything runs at
   1.2 GHz instead of 2.4 GHz).
 - Heads are processed in pairs: head-even uses PE rows 0:63, head-odd rows
   64:127 (tile_position row groups) so the two S matmuls per key-tile run
   concurrently in the PE array and LDWEIGHTS overlaps streaming.
 - softmax exp alternates between ScalarE (real exp, even key-tiles) and
   VectorE (Schraudolph bit-trick exp -> bf16 bit pattern via an int16
   round, odd key-tiles), halving the exp bottleneck.
 - PV matmuls lag S by 2 key-tiles (3-deep PSUM ring) so the exp latency is
   off the PE critical path.

All matmuls run in bf16 with fp32 PSUM accumulation; softmax logits stay fp32.
"""
import numpy as np

C = 512          # channels
N = 4096         # pixels (64*64)
NQ = 1024        # queries per core
H = 8            # heads
D = 64           # head dim
T = 4            # 128-channel chunks
W = NQ // 512    # query windows of 512
MT = N // 128    # key m-tiles of 128
NGROUPS = 8
EPS = 1e-5
GELEM = (C // NGROUPS) * N   # elements per norm group

# Schraudolph fast-exp (fp8e5m2 bits via int8 round-to-nearest):
#   bits = round(raw * SA5 + SB5);  bitcast(int8->fp8e5) ~ exp(0.125*raw)
# SA5 = 0.125 * log2(e) * 4 ; SB5 = 15*4 - 0.21875 (max rel err 11.7%,
# same order as the direct e5m2 quantization of a true exp)
SA5 = 0.7213475204444817
SB5 = 59.78125

N_WARM1 = 60     # warmup MMs before transposes (covers input DMA)
N_WARM2 = 20     # warmup MMs per groupnorm chunk
N_WARM3 = 60     # warmup MMs after groupnorm emission

_COMPILED = None


def _emit(tc, io):
    import concourse.bass as bass
    from concourse import mybir
    from contextlib import ExitStack

    nc = tc.nc
    f32 = mybir.dt.float32
    bf16 = mybir.dt.bfloat16
    i8 = mybir.dt.int8
    f8 = mybir.dt.float8e5
    Alu = mybir.AluOpType
    Act = mybir.ActivationFunctionType

    xb, qkvw, qkvb, projw, projb, nw, nb, y = (
        io["xb"], io["qkvw"], io["qkvb"], io["projw"], io["projb"],
        io["nw"], io["nb"], io["y"])

    ctx = ExitStack()
    with ctx:
        # ---------------- pools ----------------
        # PSUM: sp ring 3x(128,1024) [6 banks] + pv 2x(128,512) [2 banks]
        left = ctx.enter_context(tc.tile_pool(name="left", bufs=1))
        psum_sp = ctx.enter_context(tc.tile_pool(name="psum_sp", bufs=3, space="PSUM"))
        psum_pv = ctx.enter_context(tc.tile_pool(name="psum_pv", bufs=2, space="PSUM"))

        right_ctx = ExitStack()
        xf_pool = right_ctx.enter_context(
            tc.tile_pool(name="xf_pool", bufs=1, side="right"))
        wstg_pool = right_ctx.enter_context(
            tc.tile_pool(name="wstg_pool", bufs=4, side="right"))
        scr_pool = right_ctx.enter_context(
            tc.tile_pool(name="scr_pool", bufs=2, side="right"))

        # ---------------- persistent tiles ----------------
        # fp8 activations/weights for DoubleRow GEMMs, chunk-major layouts
        xn8 = left.tile([128, T * N], f8, name="xn8", tag="xn8")
        ksb = [left.tile([128, N], bf16, name=f"ksb{t}", tag=f"ksb{t}") for t in range(T)]
        qsb = [left.tile([128, NQ], bf16, name=f"qsb{t}", tag=f"qsb{t}") for t in range(T)]
        wTq8 = left.tile([128, T * 1536], f8, name="wTq8", tag="wTq8")
        wTp8 = left.tile([128, T * C], f8, name="wTp8", tag="wTp8")
        vb_bc = left.tile([128, C], f32, name="vb_bc", tag="vb_bc")
        ones_row = left.tile([1, D], f32, name="ones_row", tag="ones_row")
        qb = [left.tile([128, 1], f32, name=f"qb{i}", tag=f"qb{i}") for i in range(8)]
        pb = [left.tile([128, 1], f32, name=f"pb{i}", tag=f"pb{i}") for i in range(T)]
        nwt = [left.tile([128, 1], f32, name=f"nwt{t}", tag=f"nwt{t}") for t in range(T)]
        nbt = [left.tile([128, 1], f32, name=f"nbt{t}", tag=f"nbt{t}") for t in range(T)]
        stat = [left.tile([128, 2], f32, name=f"stat{t}", tag=f"stat{t}") for t in range(T)]
        gstat = [left.tile([128, 2], f32, name=f"gstat{t}", tag=f"gstat{t}") for t in range(T)]
        wu = left.tile([128, 512], bf16, name="wu", tag="wu")

        # ---------------- warmup: trip the HAM clock gate ----------------
        nc.vector.memset(wu[:], 0.125)
        wu_ps = psum_pv.tile([128, 512], f32, name="wu_ps", tag="pv")
        # preload the exp activation table while idle
        wu_exp = scr_pool.tile([1, 8], f32, name="wu_exp", tag="wu_exp")
        nc.scalar.activation(wu_exp[0:1, :], wu[0:1, 0:8], Act.Exp)

        def warm(n):
            for _ in range(n):
                nc.tensor.matmul(wu_ps[:], wu[:, 0:128], wu[:],
                                 start=True, stop=True)
        warm(N_WARM1)

        # ---------------- input DMAs ----------------
        xf = [xf_pool.tile([128, N], f32, name=f"xf{t}", tag=f"xf{t}") for t in range(T)]
        for t in range(T):
            for c4 in range(4):   # split across DMA queues
                nc.sync.dma_start(
                    xf[t][:, 1024 * c4:1024 * (c4 + 1)],
                    xb[128 * t:128 * (t + 1), 1024 * c4:1024 * (c4 + 1)])
            nc.sync.dma_start(nwt[t][:, 0:1], nw[128 * t:128 * (t + 1)])
            nc.sync.dma_start(nbt[t][:, 0:1], nb[128 * t:128 * (t + 1)])
            nc.sync.dma_start(pb[t][:, 0:1], projb[128 * t:128 * (t + 1)])
        for i in range(8):
            nc.sync.dma_start(qb[i][:, 0:1], qkvb[128 * i:128 * (i + 1)])
        # v bias broadcast to 128 partitions (stride-0 partition read)
        nc.gpsimd.dma_start(
            out=vb_bc[:],
            in_=bass.AP(tensor=qkvb.tensor, offset=1024, ap=[[0, 128], [1, C]]))
        nc.vector.memset(ones_row[0:1, :], 1.0)

        # weights: natural-layout contiguous DMA, cast to bf16, transpose
        # 128x128 blocks on the PE (identity trick) into wTq/wTp.
        ident = left.tile([128, 128], bf16, name="ident", tag="ident")
        nc.sync.dma_start(ident[:], io["cid"][:, :])
        ind = left.tile([128, 2], f32, name="ind", tag="ind")
        nc.sync.dma_start(ind[:], io["cind"][:, :])
        indT = left.tile([2, 128], f32, name="indT", tag="indT")
        nc.sync.dma_start(indT[0:2, :], io["cindT"][:, :])
        for i in range(12):   # qkv_w row-tiles
            wstg = wstg_pool.tile([128, C], f32, name="wstg", tag="wstg")
            nc.sync.dma_start(wstg[:], qkvw[128 * i:128 * (i + 1), :])
            wbf = wstg_pool.tile([128, C], bf16, name="wbf", tag="wbf")
            nc.vector.tensor_copy(wbf[:], wstg[:])
            for j in range(T):
                tp = psum_sp.tile([128, 1024], bf16, name="tp", tag="sp")
                nc.tensor.transpose(tp[:, 0:128], wbf[:, 128 * j:128 * (j + 1)], ident[:])
                nc.vector.tensor_copy(
                    wTq8[:, 1536 * j + 128 * i:1536 * j + 128 * (i + 1)], tp[:, 0:128])
        for i in range(4):    # proj_w row-tiles
            wstg = wstg_pool.tile([128, C], f32, name="wstg", tag="wstg")
            nc.sync.dma_start(wstg[:], projw[128 * i:128 * (i + 1), :])
            wbf = wstg_pool.tile([128, C], bf16, name="wbf", tag="wbf")
            nc.vector.tensor_copy(wbf[:], wstg[:])
            for j in range(T):
                tp = psum_sp.tile([128, 1024], bf16, name="tp", tag="sp")
                nc.tensor.transpose(tp[:, 0:128], wbf[:, 128 * j:128 * (j + 1)], ident[:])
                nc.vector.tensor_copy(
                    wTp8[:, C * j + 128 * i:C * j + 128 * (i + 1)], tp[:, 0:128])

        # ---------------- phase 1: group stats + normalize ----------------
        for t in range(T):
            nc.vector.tensor_reduce(
                out=stat[t][:, 0:1], in_=xf[t][:], axis=mybir.AxisListType.X, op=Alu.add)
            sq_scr = scr_pool.tile([128, N], bf16, name="sq_scr", tag="sq_scr")
            nc.scalar.activation(
                sq_scr[:], xf[t][:], Act.Square, accum_out=stat[t][:, 1:2])
            # group-reduce over partitions via indicator matmuls
            gg_ps = psum_pv.tile([128, 512], f32, name="gg_ps", tag="pv")
            nc.tensor.matmul(gg_ps[0:2, 0:2], ind[:, 0:2], stat[t][:, 0:2],
                             start=True, stop=True)
            gg_sb = left.tile([2, 2], f32, name=f"gg_sb{t}", tag=f"gg_sb{t}")
            nc.vector.tensor_copy(gg_sb[0:2, :], gg_ps[0:2, 0:2])
            gb_ps = psum_pv.tile([128, 512], f32, name="gb_ps", tag="pv")
            nc.tensor.matmul(gb_ps[:, 0:2], indT[0:2, :], gg_sb[0:2, :],
                             start=True, stop=True)
            nc.vector.tensor_copy(gstat[t][:, 0:2], gb_ps[:, 0:2])
            # mean/var/rstd -> per-channel affine a,b
            mean_t = left.tile([128, 1], f32, name=f"mean{t}", tag=f"mean{t}")
            e2_t = left.tile([128, 1], f32, name=f"e2{t}", tag=f"e2{t}")
            var_t = left.tile([128, 1], f32, name=f"var{t}", tag=f"var{t}")
            std_t = left.tile([128, 1], f32, name=f"std{t}", tag=f"std{t}")
            a_t = left.tile([128, 1], f32, name=f"a{t}", tag=f"a{t}")
            b_t = left.tile([128, 1], f32, name=f"b{t}", tag=f"b{t}")
            inv = 1.0 / GELEM
            nc.vector.tensor_scalar(mean_t[:], gstat[t][:, 0:1], inv, None, Alu.mult)
            nc.vector.tensor_scalar(e2_t[:], gstat[t][:, 1:2], inv, None, Alu.mult)
            nc.vector.scalar_tensor_tensor(
                var_t[:], mean_t[:], -1.0, mean_t[:], Alu.mult, Alu.mult)
            nc.vector.scalar_tensor_tensor(
                var_t[:], e2_t[:], EPS, var_t[:], Alu.add, Alu.add)
            nc.scalar.activation(std_t[:], var_t[:], Act.Sqrt)
            nc.vector.reciprocal(a_t[:], std_t[:])
            nc.vector.tensor_tensor(a_t[:], a_t[:], nwt[t][:], Alu.mult)
            nc.vector.tensor_tensor(b_t[:], mean_t[:], a_t[:], Alu.mult)
            nc.vector.tensor_tensor(b_t[:], nbt[t][:], b_t[:], Alu.subtract)
            # normalize + cast to fp8
            nc.vector.tensor_scalar(
                xn8[:, N * t:N * (t + 1)], xf[t][:],
                a_t[:, 0:1], b_t[:, 0:1], Alu.mult, Alu.add)
            warm(N_WARM2)
        warm(N_WARM3)

        right_ctx.close()

        # ---------------- mid pools (reuse xf space) ----------------
        mid = ctx.enter_context(tc.tile_pool(name="mid", bufs=1))
        ps8_pool = ctx.enter_context(tc.tile_pool(name="ps8_pool", bufs=4))
        rec_pool = ctx.enter_context(tc.tile_pool(name="rec_pool", bufs=2))
        yo_pool = ctx.enter_context(tc.tile_pool(name="yo_pool", bufs=2))

        # fp8e5 v^T for DoubleRow PV: per m-pair m2 and head h, 160 cols =
        # two 80-wide blocks (64 v + ones col @64 + 15 zero pad), one per
        # m-tile of the pair
        vT8 = mid.tile([128, (MT // 2) * 1280], f8, name="vT8", tag="vT8")
        attn8 = mid.tile([128, T * NQ], f8, name="attn8", tag="attn8")
        xres = [mid.tile([128, NQ], f32, name=f"xres{t}", tag=f"xres{t}") for t in range(T)]
        for t in range(T):
            nc.sync.dma_start(xres[t][:], xb[128 * t:128 * (t + 1), 0:NQ])

        # ones + zero-pad columns of the augmented v^T (denominator trick)
        v80 = vT8[:].rearrange("p (n x) -> p n x", x=80)
        nc.vector.memset(v80[:, :, 64:65], 1.0)
        nc.vector.memset(v80[:, :, 65:80], 0.0)

        # ---------------- phase 3: projections (fp8 DoubleRow) ----------
        DR = mybir.MatmulPerfMode.DoubleRow

        def wq_pair(pr, lo, hi):
            return wTq8[:, 3072 * pr:3072 * (pr + 1)].rearrange(
                "p (j x) -> p j x", j=2)[:, :, lo:hi]

        def xn_pair(pr, lo, hi):
            return xn8[:, 2 * N * pr:2 * N * (pr + 1)].rearrange(
                "p (j x) -> p j x", j=2)[:, :, lo:hi]

        # q: out rows 0..511 of qkv, only NQ query columns (2 windows per tile)
        for i in range(T):
            qp = psum_sp.tile([128, 1024], f32, name="qp", tag="sp")
            for w in range(W):
                for pr in range(2):
                    nc.tensor.matmul(
                        qp[:, 512 * w:512 * w + 512],
                        wq_pair(pr, 128 * i, 128 * i + 128),
                        xn_pair(pr, 512 * w, 512 * w + 512),
                        start=(pr == 0), stop=(pr == 1), perf_mode=DR)
            nc.vector.tensor_scalar(
                qsb[i][:], qp[:], qb[i][:, 0:1], None, Alu.add)
        # k: out rows 512..1023, all N columns (2 windows per psum tile)
        for i in range(T):
            for w2 in range(4):
                kp = psum_sp.tile([128, 1024], f32, name="kp", tag="sp")
                for w in (2 * w2, 2 * w2 + 1):
                    for pr in range(2):
                        nc.tensor.matmul(
                            kp[:, 512 * (w % 2):512 * (w % 2) + 512],
                            wq_pair(pr, 512 + 128 * i, 512 + 128 * i + 128),
                            xn_pair(pr, 512 * w, 512 * w + 512),
                            start=(pr == 0), stop=(pr == 1), perf_mode=DR)
                nc.vector.tensor_scalar(
                    ksb[i][:, 1024 * w2:1024 * (w2 + 1)], kp[:],
                    qb[4 + i][:, 0:1], None, Alu.add)
        # vT: (m, 512) per m-tile, two m-tiles per psum tile, strided into
        # the 80-column augmented fp8 layout
        for m2 in range(MT // 2):
            vp = psum_sp.tile([128, 1024], f32, name="vp", tag="sp")
            for mh in range(2):
                mt = 2 * m2 + mh
                for pr in range(2):
                    nc.tensor.matmul(
                        vp[:, 512 * mh:512 * mh + 512],
                        xn_pair(pr, 128 * mt, 128 * mt + 128),
                        wq_pair(pr, 1024, 1536),
                        start=(pr == 0), stop=(pr == 1), perf_mode=DR)
            vbv = vb_bc[:].rearrange("p (h x) -> p h x", x=64)
            vdst = vT8[:, 1280 * m2:1280 * (m2 + 1)].rearrange(
                "p (h j x) -> p h j x", h=8, j=2, x=80)
            for mh in range(2):
                dst = vdst[:, :, mh, 0:64]
                src = vp[:, 512 * mh:512 * mh + 512].rearrange(
                    "p (h x) -> p h x", x=64)
                nc.vector.tensor_tensor(dst, src, vbv, Alu.add)

        # ---------------- phase 4+5: attention, deferred tails, proj ----
        def emit_tail(tw, thp, tpv0, tpv1):
            # denominators stacked on partition 0, one reciprocal, two
            # broadcast matmuls (out partitions 0:64 / 64:128), per-head
            # scale into the fp8 attention tile
            dnq = rec_pool.tile([1, 1024], f32, name="dnq", tag="dnq")
            nc.scalar.copy(dnq[0:1, 0:512], tpv0[64:65, :])
            nc.scalar.copy(dnq[0:1, 512:1024], tpv1[64:65, :])
            rcq = rec_pool.tile([1, 1024], f32, name="rcq", tag="rcq")
            rscr = rec_pool.tile([1, 1024], f32, name="rscr", tag="rscr")
            nc.vector.reciprocal_approx_accurate(
                rcq[0:1, :], dnq[0:1, :], rscr[0:1, :])
            bc = psum_sp.tile([128, 1024], f32, name="bc", tag="sp")
            nc.tensor.matmul(
                bc[0:64, 0:512], ones_row[0:1, 0:D],
                rcq[0:1, 0:512], start=True, stop=True)
            nc.tensor.matmul(
                bc[64:128, 0:512], ones_row[0:1, 0:D],
                rcq[0:1, 512:1024], start=True, stop=True)
            bcs0 = rec_pool.tile([64, 512], f32, name="bcs0", tag="bcs0")
            bcs1 = rec_pool.tile([64, 512], f32, name="bcs1", tag="bcs1")
            nc.scalar.copy(bcs0[0:64, :], bc[0:64, 0:512])
            nc.scalar.copy(bcs1[0:64, :], bc[64:128, 0:512])
            nc.vector.tensor_tensor(
                attn8[0:64, NQ * thp + 512 * tw:NQ * thp + 512 * tw + 512],
                tpv0[0:64, :], bcs0[0:64, :], Alu.mult)
            nc.vector.tensor_tensor(
                attn8[64:128, NQ * thp + 512 * tw:NQ * thp + 512 * tw + 512],
                tpv1[0:64, :], bcs1[0:64, :], Alu.mult)

        def emit_proj(pw):
            # proj + residual for window pw (fp8 DoubleRow over attn8)
            for i in range(T):
                py = psum_sp.tile([128, 1024], f32, name="py", tag="sp")
                for pr in range(2):
                    rhs = attn8[:, 2 * NQ * pr:2 * NQ * (pr + 1)].rearrange(
                        "p (j x) -> p j x", j=2)[:, :, 512 * pw:512 * pw + 512]
                    lhs = wTp8[:, 1024 * pr:1024 * (pr + 1)].rearrange(
                        "p (j x) -> p j x", j=2)[:, :, 128 * i:128 * i + 128]
                    nc.tensor.matmul(
                        py[:, 0:512], lhs, rhs,
                        start=(pr == 0), stop=(pr == 1), perf_mode=DR)
                yo = yo_pool.tile([128, 512], f32, name="yo", tag="yo")
                nc.vector.scalar_tensor_tensor(
                    yo[:], py[:, 0:512], pb[i][:, 0:1],
                    xres[i][:, 512 * pw:512 * pw + 512], Alu.add, Alu.add)
                nc.sync.dma_start(
                    y[128 * i:128 * i + 128, 512 * pw:512 * pw + 512], yo[:])

        deferred = None       # (w, hp, pv0, pv1) of the previous unit
        proj_due = None       # window whose proj should be emitted next
        for w in range(W):
            for hp in range(4):
                h0, h1 = 2 * hp, 2 * hp + 1
                pv0 = psum_pv.tile([128, 512], f32, name="pv0", tag="pv")
                pv1 = psum_pv.tile([128, 512], f32, name="pv1", tag="pv")
                pend = []

                def flush_pv():
                    m2, ps8 = pend.pop(0)
                    st, sto = (m2 == 0), (m2 == MT // 2 - 1)
                    vb = 1280 * m2
                    for h, pv in ((0, pv0), (1, pv1)):
                        lhs = vT8[:, vb + 160 * (2 * hp + h):
                                  vb + 160 * (2 * hp + h) + 160].rearrange(
                            "p (j x) -> p j x", j=2)
                        rhs = ps8[:, 1024 * h:1024 * h + 1024].rearrange(
                            "p (j x) -> p j x", j=2)
                        nc.tensor.matmul(
                            pv[0:80, :], lhs, rhs, start=st, stop=sto,
                            perf_mode=DR)

                ps8 = None
                for m in range(MT):
                    sp = psum_sp.tile([128, 1024], f32, name="sp", tag="sp")
                    # S pair: head-even on PE rows 0:63, head-odd on rows
                    # 64:127 (concurrent row groups, shared LDW window)
                    nc.tensor.matmul(
                        sp[:, 0:512],
                        ksb[hp][0:64, 128 * m:128 * m + 128],
                        qsb[hp][0:64, 512 * w:512 * w + 512],
                        start=True, stop=True)
                    nc.tensor.matmul(
                        sp[:, 512:1024],
                        ksb[hp][64:128, 128 * m:128 * m + 128],
                        qsb[hp][64:128, 512 * w:512 * w + 512],
                        start=True, stop=True)
                    m2, mh = divmod(m, 2)
                    if mh == 0:
                        ps8 = ps8_pool.tile([128, 2048], f8, name="ps8", tag="ps8")
                    # exp of this m-tile for both heads into the fp8 pair
                    # tile: cols [1024h + 512mh : +512] = (head h, m-half mh)
                    outv = ps8[:].rearrange(
                        "p (H x) -> p H x", H=2)[:, :, 512 * mh:512 * mh + 512]
                    inv = sp[:].rearrange("p (h x) -> p h x", h=2)
                    # ACT handles even m-tiles plus one odd (engine balance);
                    # DVE does the rest via the int8 Schraudolph bit trick
                    if mh == 0 or m == 13:
                        nc.scalar.activation(outv, inv, Act.Exp, scale=0.125)
                    else:
                        nc.vector.tensor_scalar(
                            outv.bitcast(i8), inv, SA5, SB5, Alu.mult, Alu.add)
                    if mh == 1:
                        pend.append((m2, ps8))
                        if len(pend) >= 3:
                            flush_pv()
                    # previous unit's tail + any due proj, overlapped with
                    # this unit's m-loop (keeps the PE stream dense)
                    if m == 4 and deferred is not None:
                        emit_tail(*deferred)
                        deferred = None
                        if proj_due is not None:
                            emit_proj(proj_due)
                            proj_due = None
                while pend:
                    flush_pv()
                deferred = (w, hp, pv0, pv1)
                if hp == 3:
                    proj_due = w
        emit_tail(*deferred)
        emit_proj(proj_due)


def _build():
    import concourse.tile as tile
    from concourse import bacc, mybir

    nc = bacc.Bacc("TRN2", target_bir_lowering=False, debug=False)
    f32 = mybir.dt.float32
    io = {
        "xb": nc.dram_tensor("xb", [C, N], f32, kind="ExternalInput").ap(),
        "qkvw": nc.dram_tensor("qkvw", [3 * C, C], f32, kind="ExternalInput").ap(),
        "qkvb": nc.dram_tensor("qkvb", [3 * C], f32, kind="ExternalInput").ap(),
        "projw": nc.dram_tensor("projw", [C, C], f32, kind="ExternalInput").ap(),
        "projb": nc.dram_tensor("projb", [C], f32, kind="ExternalInput").ap(),
        "nw": nc.dram_tensor("nw", [C], f32, kind="ExternalInput").ap(),
        "nb": nc.dram_tensor("nb", [C], f32, kind="ExternalInput").ap(),
        "cid": nc.dram_tensor("cid", [128, 128], mybir.dt.bfloat16,
                              kind="ExternalInput").ap(),
        "cind": nc.dram_tensor("cind", [128, 2], f32, kind="ExternalInput").ap(),
        "cindT": nc.dram_tensor("cindT", [2, 128], f32, kind="ExternalInput").ap(),
        "y": nc.dram_tensor("y", [C, NQ], f32, kind="ExternalOutput").ap(),
    }
    with tile.TileContext(nc) as tc:
        _emit(tc, io)
    nc.compile()
    return nc


def get_compiled():
    global _COMPILED
    if _COMPILED is None:
        _COMPILED = _build()
    return _COMPILED


def make_in_maps(x, norm_w, norm_b, qkv_w, qkv_b, proj_w, proj_b):
    import ml_dtypes

    xf = np.ascontiguousarray(np.asarray(x, np.float32)).reshape(2, C, N)
    ind = np.zeros((128, 2), np.float32)
    ind[0:64, 0] = 1.0
    ind[64:128, 1] = 1.0
    shared = {
        "cid": np.eye(128, dtype=ml_dtypes.bfloat16),
        "cind": ind,
        "cindT": np.ascontiguousarray(ind.T),
        "qkvw": np.ascontiguousarray(np.asarray(qkv_w, np.float32)),
        "qkvb": np.ascontiguousarray(np.asarray(qkv_b, np.float32)),
        "projw": np.ascontiguousarray(np.asarray(proj_w, np.float32)),
        "projb": np.ascontiguousarray(np.asarray(proj_b, np.float32)),
        "nw": np.ascontiguousarray(np.asarray(norm_w, np.float32)),
        "nb": np.ascontiguousarray(np.asarray(norm_b, np.float32)),
    }
    in_maps = []
    for core in range(8):
        bi, qs = core // 4, core % 4
        # rotate so this core's queries are always columns [0:NQ)
        xroll = np.concatenate(
            [xf[bi][:, qs * NQ:], xf[bi][:, :qs * NQ]], axis=1)
        m = dict(shared)
        m["xb"] = np.ascontiguousarray(xroll)
        in_maps.append(m)
    return in_maps


def assemble(results, x):
    y = np.zeros((2, C, N), np.float32)
    for core in range(8):
        bi, qs = core // 4, core % 4
        y[bi][:, qs * NQ:(qs + 1) * NQ] = results[core]["y"]
    return y.reshape(x.shape)


def kernel(x, norm_w, norm_b, qkv_w, qkv_b, proj_w, proj_b, **_ignored):
    from concourse import bass_utils

    nc = get_compiled()
    in_maps = make_in_maps(x, norm_w, norm_b, qkv_w, qkv_b, proj_w, proj_b)
    res = bass_utils.run_bass_kernel_spmd(nc, in_maps, core_ids=list(range(8)))
    return assemble(res.results, np.asarray(x))


# revision 28
# speedup vs baseline: 2.0487x; 1.0381x over previous
"""Trainium2 Bass kernel for nn_AttentionBlock (GroupNorm + MHA + proj + residual).

Full inputs in, full output out. Sharding: 8 cores = 2 batches x 4 query-slices.
Each core: GroupNorm over its batch image, q projection for its 1024 queries,
k/v projections over all 4096 keys, per-head attention (S^T = k^T q, softmax
along the PSUM partition axis via an appended ones-column in the PV matmul),
output projection and residual for its query slice.

v2 performance structure:
 - Warmup matmuls trip the PE HAM clock gate early (else everything runs at
   1.2 GHz instead of 2.4 GHz).
 - Heads are processed in pairs: head-even uses PE rows 0:63, head-odd rows
   64:127 (tile_position row groups) so the two S matmuls per key-tile run
   concurrently in the PE array and LDWEIGHTS overlaps streaming.
 - softmax exp alternates between ScalarE (real exp, even key-tiles) and
   VectorE (Schraudolph bit-trick exp -> bf16 bit pattern via an int16
   round, odd key-tiles), halving the exp bottleneck.
 - PV matmuls lag S by 2 key-tiles (3-deep PSUM ring) so the exp latency is
   off the PE critical path.

All matmuls run in bf16 with fp32 PSUM accumulation; softmax logits stay fp32.
"""
import numpy as np

C = 512          # channels
N = 4096         # pixels (64*64)
NQ = 1024        # queries per core
H = 8            # heads
D = 64           # head dim
T = 4            # 128-channel chunks
W = NQ // 512    # query windows of 512
MT = N // 128    # key m-tiles of 128
NGROUPS = 8
EPS = 1e-5
GELEM = (C // NGROUPS) * N   # elements per norm group

# Schraudolph fast-exp (fp8e5m2 bits via int8 round-to-nearest):
#   bits = round(raw * SA5 + SB5);  bitcast(int8->fp8e5) ~ exp(0.125*raw)
# SA5 = 0.125 * log2(e) * 4 ; SB5 = 15*4 - 0.21875 (max rel err 11.7%,
# same order as the direct e5m2 quantization of a true exp)
SA5 = 0.7213475204444817
SB5 = 59.78125

N_WARM1 = 60     # warmup MMs before transposes (covers input DMA)
N_WARM2 = 20     # warmup MMs per groupnorm chunk
N_WARM3 = 60     # warmup MMs after groupnorm emission

_COMPILED = None


def _emit(tc, io):
    import concourse.bass as bass
    from concourse import mybir
    from contextlib import ExitStack

    nc = tc.nc
    f32 = mybir.dt.float32
    bf16 = mybir.dt.bfloat16
    i8 = mybir.dt.int8
    f8 = mybir.dt.float8e5
    Alu = mybir.AluOpType
    Act = mybir.ActivationFunctionType

    xb, qkvw, qkvb, projw, projb, nw, nb, y = (
        io["xb"], io["qkvw"], io["qkvb"], io["projw"], io["projb"],
        io["nw"], io["nb"], io["y"])

    ctx = ExitStack()
    with ctx:
        # ---------------- pools ----------------
        # PSUM: sp ring 3x(128,1024) [6 banks] + pv 2x(128,512) [2 banks]
        left = ctx.enter_context(tc.tile_pool(name="left", bufs=1))
        psum_sp = ctx.enter_context(tc.tile_pool(name="psum_sp", bufs=3, space="PSUM"))
        psum_pv = ctx.enter_context(tc.tile_pool(name="psum_pv", bufs=2, space="PSUM"))

        right_ctx = ExitStack()
        xf_pool = right_ctx.enter_context(
            tc.tile_pool(name="xf_pool", bufs=1, side="right"))
        wstg_pool = right_ctx.enter_context(
            tc.tile_pool(name="wstg_pool", bufs=4, side="right"))
        scr_pool = right_ctx.enter_context(
            tc.tile_pool(name="scr_pool", bufs=2, side="right"))

        # ---------------- persistent tiles ----------------
        # fp8 activations/weights for DoubleRow GEMMs, chunk-major layouts
        xn8 = left.tile([128, T * N], f8, name="xn8", tag="xn8")
        ksb = [left.tile([128, N], bf16, name=f"ksb{t}", tag=f"ksb{t}") for t in range(T)]
        qsb = [left.tile([128, NQ], bf16, name=f"qsb{t}", tag=f"qsb{t}") for t in range(T)]
        wTq8 = left.tile([128, T * 1536], f8, name="wTq8", tag="wTq8")
        wTp8 = left.tile([128, T * C], f8, name="wTp8", tag="wTp8")
        vb_bc = left.tile([128, C], f32, name="vb_bc", tag="vb_bc")
        ones_row = left.tile([1, D], f32, name="ones_row", tag="ones_row")
        qb = [left.tile([128, 1], f32, name=f"qb{i}", tag=f"qb{i}") for i in range(8)]
        pb = [left.tile([128, 1], f32, name=f"pb{i}", tag=f"pb{i}") for i in range(T)]
        nwt = [left.tile([128, 1], f32, name=f"nwt{t}", tag=f"nwt{t}") for t in range(T)]
        nbt = [left.tile([128, 1], f32, name=f"nbt{t}", tag=f"nbt{t}") for t in range(T)]
        stat = [left.tile([128, 2], f32, name=f"stat{t}", tag=f"stat{t}") for t in range(T)]
        gstat = [left.tile([128, 2], f32, name=f"gstat{t}", tag=f"gstat{t}") for t in range(T)]
        wu = left.tile([128, 512], bf16, name="wu", tag="wu")

        # ---------------- warmup: trip the HAM clock gate ----------------
        nc.vector.memset(wu[:], 0.125)
        wu_ps = psum_pv.tile([128, 512], f32, name="wu_ps", tag="pv")
        # preload the exp activation table while idle
        wu_exp = scr_pool.tile([1, 8], f32, name="wu_exp", tag="wu_exp")
        nc.scalar.activation(wu_exp[0:1, :], wu[0:1, 0:8], Act.Exp)

        def warm(n):
            for _ in range(n):
                nc.tensor.matmul(wu_ps[:], wu[:, 0:128], wu[:],
                                 start=True, stop=True)
        warm(N_WARM1)

        # ---------------- input DMAs ----------------
        xf = [xf_pool.tile([128, N], f32, name=f"xf{t}", tag=f"xf{t}") for t in range(T)]
        for t in range(T):
            for c4 in range(4):   # split across DMA queues
                nc.sync.dma_start(
                    xf[t][:, 1024 * c4:1024 * (c4 + 1)],
                    xb[128 * t:128 * (t + 1), 1024 * c4:1024 * (c4 + 1)])
            nc.sync.dma_start(nwt[t][:, 0:1], nw[128 * t:128 * (t + 1)])
            nc.sync.dma_start(nbt[t][:, 0:1], nb[128 * t:128 * (t + 1)])
            nc.sync.dma_start(pb[t][:, 0:1], projb[128 * t:128 * (t + 1)])
        for i in range(8):
            nc.sync.dma_start(qb[i][:, 0:1], qkvb[128 * i:128 * (i + 1)])
        # v bias broadcast to 128 partitions (stride-0 partition read)
        nc.gpsimd.dma_start(
            out=vb_bc[:],
            in_=bass.AP(tensor=qkvb.tensor, offset=1024, ap=[[0, 128], [1, C]]))
        nc.vector.memset(ones_row[0:1, :], 1.0)

        # weights: natural-layout contiguous DMA, cast to bf16, transpose
        # 128x128 blocks on the PE (identity trick) into wTq/wTp.
        ident = left.tile([128, 128], bf16, name="ident", tag="ident")
        nc.sync.dma_start(ident[:], io["cid"][:, :])
        ind = left.tile([128, 2], f32, name="ind", tag="ind")
        nc.sync.dma_start(ind[:], io["cind"][:, :])
        indT = left.tile([2, 128], f32, name="indT", tag="indT")
        nc.sync.dma_start(indT[0:2, :], io["cindT"][:, :])
        for i in range(12):   # qkv_w row-tiles
            wstg = wstg_pool.tile([128, C], f32, name="wstg", tag="wstg")
            nc.sync.dma_start(wstg[:], qkvw[128 * i:128 * (i + 1), :])
            wbf = wstg_pool.tile([128, C], bf16, name="wbf", tag="wbf")
            nc.vector.tensor_copy(wbf[:], wstg[:])
            for j in range(T):
                tp = psum_sp.tile([128, 1024], bf16, name="tp", tag="sp")
                nc.tensor.transpose(tp[:, 0:128], wbf[:, 128 * j:128 * (j + 1)], ident[:])
                nc.vector.tensor_copy(
                    wTq8[:, 1536 * j + 128 * i:1536 * j + 128 * (i + 1)], tp[:, 0:128])
        for i in range(4):    # proj_w row-tiles
            wstg = wstg_pool.tile([128, C], f32, name="wstg", tag="wstg")
            nc.sync.dma_start(wstg[:], projw[128 * i:128 * (i + 1), :])
            wbf = wstg_pool.tile([128, C], bf16, name="wbf", tag="wbf")
            nc.vector.tensor_copy(wbf[:], wstg[:])
            for j in range(T):
                tp = psum_sp.tile([128, 1024], bf16, name="tp", tag="sp")
                nc.tensor.transpose(tp[:, 0:128], wbf[:, 128 * j:128 * (j + 1)], ident[:])
                nc.vector.tensor_copy(
                    wTp8[:, C * j + 128 * i:C * j + 128 * (i + 1)], tp[:, 0:128])

        # ---------------- phase 1: group stats + normalize ----------------
        for t in range(T):
            nc.vector.tensor_reduce(
                out=stat[t][:, 0:1], in_=xf[t][:], axis=mybir.AxisListType.X, op=Alu.add)
            sq_scr = scr_pool.tile([128, N], bf16, name="sq_scr", tag="sq_scr")
            nc.scalar.activation(
                sq_scr[:], xf[t][:], Act.Square, accum_out=stat[t][:, 1:2])
            # group-reduce over partitions via indicator matmuls
            gg_ps = psum_pv.tile([128, 512], f32, name="gg_ps", tag="pv")
            nc.tensor.matmul(gg_ps[0:2, 0:2], ind[:, 0:2], stat[t][:, 0:2],
                             start=True, stop=True)
            gg_sb = left.tile([2, 2], f32, name=f"gg_sb{t}", tag=f"gg_sb{t}")
            nc.vector.tensor_copy(gg_sb[0:2, :], gg_ps[0:2, 0:2])
            gb_ps = psum_pv.tile([128, 512], f32, name="gb_ps", tag="pv")
            nc.tensor.matmul(gb_ps[:, 0:2], indT[0:2, :], gg_sb[0:2, :],
                             start=True, stop=True)
            nc.vector.tensor_copy(gstat[t][:, 0:2], gb_ps[:, 0:2])
            # mean/var/rstd -> per-channel affine a,b
            mean_t = left.tile([128, 1], f32, name=f"mean{t}", tag=f"mean{t}")
            e2_t = left.tile([128, 1], f32, name=f"e2{t}", tag=f"e2{t}")
            var_t = left.tile([128, 1], f32, name=f"var{t}", tag=f"var{t}")
            std_t = left.tile([128, 1], f32, name=f"std{t}", tag=f"std{t}")
            a_t = left.tile([128, 1], f32, name=f"a{t}", tag=f"a{t}")
            b_t = left.tile([128, 1], f32, name=f"b{t}", tag=f"b{t}")
            inv = 1.0 / GELEM
            nc.vector.tensor_scalar(mean_t[:], gstat[t][:, 0:1], inv, None, Alu.mult)
            nc.vector.tensor_scalar(e2_t[:], gstat[t][:, 1:2], inv, None, Alu.mult)
            nc.vector.scalar_tensor_tensor(
                var_t[:], mean_t[:], -1.0, mean_t[:], Alu.mult, Alu.mult)
            nc.vector.scalar_tensor_tensor(
                var_t[:], e2_t[:], EPS, var_t[:], Alu.add, Alu.add)
            nc.scalar.activation(std_t[:], var_t[:], Act.Sqrt)
            nc.vector.reciprocal(a_t[:], std_t[:])
            nc.vector.tensor_tensor(a_t[:], a_t[:], nwt[t][:], Alu.mult)
            nc.vector.tensor_tensor(b_t[:], mean_t[:], a_t[:], Alu.mult)
            nc.vector.tensor_tensor(b_t[:], nbt[t][:], b_t[:], Alu.subtract)
            # normalize + cast to fp8 (alternate engines to halve the chain)
            if t % 2 == 0:
                nc.scalar.activation(
                    xn8[:, N * t:N * (t + 1)], xf[t][:], Act.Identity,
                    bias=b_t[:, 0:1], scale=a_t[:, 0:1])
            else:
                nc.vector.tensor_scalar(
                    xn8[:, N * t:N * (t + 1)], xf[t][:],
                    a_t[:, 0:1], b_t[:, 0:1], Alu.mult, Alu.add)
            warm(N_WARM2)
        warm(N_WARM3)

        right_ctx.close()

        # ---------------- mid pools (reuse xf space) ----------------
        mid = ctx.enter_context(tc.tile_pool(name="mid", bufs=1))
        ps8_pool = ctx.enter_context(tc.tile_pool(name="ps8_pool", bufs=6))
        rec_pool = ctx.enter_context(tc.tile_pool(name="rec_pool", bufs=2))
        yo_pool = ctx.enter_context(tc.tile_pool(name="yo_pool", bufs=2))

        # fp8e5 v^T for DoubleRow PV: per m-pair m2 and head h, 160 cols =
        # two 80-wide blocks (64 v + ones col @64 + 15 zero pad), one per
        # m-tile of the pair
        vT8 = mid.tile([128, (MT // 2) * 1280], f8, name="vT8", tag="vT8")
        attn8 = mid.tile([128, T * NQ], f8, name="attn8", tag="attn8")
        xres = [mid.tile([128, NQ], f32, name=f"xres{t}", tag=f"xres{t}") for t in range(T)]
        for t in range(T):
            nc.sync.dma_start(xres[t][:], xb[128 * t:128 * (t + 1), 0:NQ])

        # ones + zero-pad columns of the augmented v^T (denominator trick)
        v80 = vT8[:].rearrange("p (n x) -> p n x", x=80)
        nc.vector.memset(v80[:, :, 64:65], 1.0)
        nc.vector.memset(v80[:, :, 65:80], 0.0)

        # ---------------- phase 3: projections (fp8 DoubleRow) ----------
        DR = mybir.MatmulPerfMode.DoubleRow

        def wq_pair(pr, lo, hi):
            return wTq8[:, 3072 * pr:3072 * (pr + 1)].rearrange(
                "p (j x) -> p j x", j=2)[:, :, lo:hi]

        def xn_pair(pr, lo, hi):
            return xn8[:, 2 * N * pr:2 * N * (pr + 1)].rearrange(
                "p (j x) -> p j x", j=2)[:, :, lo:hi]

        # q: out rows 0..511 of qkv, only NQ query columns (2 windows per tile)
        for i in range(T):
            qp = psum_sp.tile([128, 1024], f32, name="qp", tag="sp")
            for w in range(W):
                for pr in range(2):
                    nc.tensor.matmul(
                        qp[:, 512 * w:512 * w + 512],
                        wq_pair(pr, 128 * i, 128 * i + 128),
                        xn_pair(pr, 512 * w, 512 * w + 512),
                        start=(pr == 0), stop=(pr == 1), perf_mode=DR)
            if i % 2 == 0:
                nc.scalar.add(qsb[i][:], qp[:], qb[i][:, 0:1])
            else:
                nc.vector.tensor_scalar(
                    qsb[i][:], qp[:], qb[i][:, 0:1], None, Alu.add)
        # k: out rows 512..1023, all N columns (2 windows per psum tile)
        for i in range(T):
            for w2 in range(4):
                kp = psum_sp.tile([128, 1024], f32, name="kp", tag="sp")
                for w in (2 * w2, 2 * w2 + 1):
                    for pr in range(2):
                        nc.tensor.matmul(
                            kp[:, 512 * (w % 2):512 * (w % 2) + 512],
                            wq_pair(pr, 512 + 128 * i, 512 + 128 * i + 128),
                            xn_pair(pr, 512 * w, 512 * w + 512),
                            start=(pr == 0), stop=(pr == 1), perf_mode=DR)
                if w2 % 2 == 0:
                    nc.scalar.add(
                        ksb[i][:, 1024 * w2:1024 * (w2 + 1)], kp[:],
                        qb[4 + i][:, 0:1])
                else:
                    nc.vector.tensor_scalar(
                        ksb[i][:, 1024 * w2:1024 * (w2 + 1)], kp[:],
                        qb[4 + i][:, 0:1], None, Alu.add)
        # vT: (m, 512) per m-tile, two m-tiles per psum tile, strided into
        # the 80-column augmented fp8 layout
        for m2 in range(MT // 2):
            vp = psum_sp.tile([128, 1024], f32, name="vp", tag="sp")
            for mh in range(2):
                mt = 2 * m2 + mh
                for pr in range(2):
                    nc.tensor.matmul(
                        vp[:, 512 * mh:512 * mh + 512],
                        xn_pair(pr, 128 * mt, 128 * mt + 128),
                        wq_pair(pr, 1024, 1536),
                        start=(pr == 0), stop=(pr == 1), perf_mode=DR)
            vbv = vb_bc[:].rearrange("p (h x) -> p h x", x=64)
            vdst = vT8[:, 1280 * m2:1280 * (m2 + 1)].rearrange(
                "p (h j x) -> p h j x", h=8, j=2, x=80)
            for mh in range(2):
                dst = vdst[:, :, mh, 0:64]
                src = vp[:, 512 * mh:512 * mh + 512].rearrange(
                    "p (h x) -> p h x", x=64)
                nc.vector.tensor_tensor(dst, src, vbv, Alu.add)

        # ---------------- phase 4+5: attention, deferred tails, proj ----
        # tails are emitted in stages inside the NEXT unit's m-loop so the
        # PE stream never waits on the reciprocal chain
        def tail_stage1(tw, thp, tpv0, tpv1):
            # denominators stacked on partition 0 + one reciprocal
            dnq = rec_pool.tile([1, 1024], f32, name="dnq", tag="dnq")
            nc.scalar.copy(dnq[0:1, 0:512], tpv0[64:65, :])
            nc.scalar.copy(dnq[0:1, 512:1024], tpv1[64:65, :])
            rcq = rec_pool.tile([1, 1024], f32, name="rcq", tag="rcq")
            rscr = rec_pool.tile([1, 1024], f32, name="rscr", tag="rscr")
            nc.vector.reciprocal_approx_accurate(
                rcq[0:1, :], dnq[0:1, :], rscr[0:1, :])
            return rcq

        def tail_stage2(rcq):
            # partition-broadcast of the reciprocals via two PE matmuls
            # (emitted mid-next-unit, so the reciprocal chain is already done)
            bc = psum_sp.tile([128, 1024], f32, name="bc", tag="sp")
            nc.tensor.matmul(
                bc[0:64, 0:512], ones_row[0:1, 0:D],
                rcq[0:1, 0:512], start=True, stop=True)
            nc.tensor.matmul(
                bc[64:128, 0:512], ones_row[0:1, 0:D],
                rcq[0:1, 512:1024], start=True, stop=True)
            bcs0 = rec_pool.tile([64, 512], f32, name="bcs0", tag="bcs0")
            bcs1 = rec_pool.tile([64, 512], f32, name="bcs1", tag="bcs1")
            nc.scalar.copy(bcs0[0:64, :], bc[0:64, 0:512])
            nc.scalar.copy(bcs1[0:64, :], bc[64:128, 0:512])
            return bcs0, bcs1

        def tail_stage3(tw, thp, tpv0, tpv1, bcs0, bcs1):
            nc.vector.tensor_tensor(
                attn8[0:64, NQ * thp + 512 * tw:NQ * thp + 512 * tw + 512],
                tpv0[0:64, :], bcs0[0:64, :], Alu.mult)
            nc.vector.tensor_tensor(
                attn8[64:128, NQ * thp + 512 * tw:NQ * thp + 512 * tw + 512],
                tpv1[0:64, :], bcs1[0:64, :], Alu.mult)

        def emit_proj(pw):
            # proj + residual for window pw (fp8 DoubleRow over attn8)
            for i in range(T):
                py = psum_sp.tile([128, 1024], f32, name="py", tag="sp")
                for pr in range(2):
                    rhs = attn8[:, 2 * NQ * pr:2 * NQ * (pr + 1)].rearrange(
                        "p (j x) -> p j x", j=2)[:, :, 512 * pw:512 * pw + 512]
                    lhs = wTp8[:, 1024 * pr:1024 * (pr + 1)].rearrange(
                        "p (j x) -> p j x", j=2)[:, :, 128 * i:128 * i + 128]
                    nc.tensor.matmul(
                        py[:, 0:512], lhs, rhs,
                        start=(pr == 0), stop=(pr == 1), perf_mode=DR)
                yo = yo_pool.tile([128, 512], f32, name="yo", tag="yo")
                nc.vector.scalar_tensor_tensor(
                    yo[:], py[:, 0:512], pb[i][:, 0:1],
                    xres[i][:, 512 * pw:512 * pw + 512], Alu.add, Alu.add)
                nc.sync.dma_start(
                    y[128 * i:128 * i + 128, 512 * pw:512 * pw + 512], yo[:])

        deferred = None       # (w, hp, pv0, pv1) of the previous unit
        proj_due = None       # window whose proj should be emitted next
        for w in range(W):
            for hp in range(4):
                h0, h1 = 2 * hp, 2 * hp + 1
                pv0 = psum_pv.tile([128, 512], f32, name="pv0", tag="pv")
                pv1 = psum_pv.tile([128, 512], f32, name="pv1", tag="pv")
                pend = []

                def flush_pv():
                    m2, ps8 = pend.pop(0)
                    st, sto = (m2 == 0), (m2 == MT // 2 - 1)
                    vb = 1280 * m2
                    for h, pv in ((0, pv0), (1, pv1)):
                        lhs = vT8[:, vb + 160 * (2 * hp + h):
                                  vb + 160 * (2 * hp + h) + 160].rearrange(
                            "p (j x) -> p j x", j=2)
                        rhs = ps8[:, 1024 * h:1024 * h + 1024].rearrange(
                            "p (j x) -> p j x", j=2)
                        nc.tensor.matmul(
                            pv[0:80, :], lhs, rhs, start=st, stop=sto,
                            perf_mode=DR)

                ps8 = None
                for m in range(MT):
                    sp = psum_sp.tile([128, 1024], f32, name="sp", tag="sp")
                    # S pair: head-even on PE rows 0:63, head-odd on rows
                    # 64:127 (concurrent row groups, shared LDW window)
                    nc.tensor.matmul(
                        sp[:, 0:512],
                        ksb[hp][0:64, 128 * m:128 * m + 128],
                        qsb[hp][0:64, 512 * w:512 * w + 512],
                        start=True, stop=True)
                    nc.tensor.matmul(
                        sp[:, 512:1024],
                        ksb[hp][64:128, 128 * m:128 * m + 128],
                        qsb[hp][64:128, 512 * w:512 * w + 512],
                        start=True, stop=True)
                    m2, mh = divmod(m, 2)
                    if mh == 0:
                        ps8 = ps8_pool.tile([128, 2048], f8, name="ps8", tag="ps8")
                    # exp of this m-tile for both heads into the fp8 pair
                    # tile: cols [1024h + 512mh : +512] = (head h, m-half mh)
                    outv = ps8[:].rearrange(
                        "p (H x) -> p H x", H=2)[:, :, 512 * mh:512 * mh + 512]
                    inv = sp[:].rearrange("p (h x) -> p h x", h=2)
                    # ACT handles even m-tiles plus one odd (engine balance);
                    # DVE does the rest via the int8 Schraudolph bit trick
                    if mh == 0 or m == 13:
                        nc.scalar.activation(outv, inv, Act.Exp, scale=0.125)
                    else:
                        nc.vector.tensor_scalar(
                            outv.bitcast(i8), inv, SA5, SB5, Alu.mult, Alu.add)
                    if mh == 1:
                        pend.append((m2, ps8))
                        if len(pend) >= 5:
                            flush_pv()
                    # previous unit's tail stages + any due proj, overlapped
                    # with this unit's m-loop (keeps the PE stream dense)
                    if deferred is not None:
                        if m == 2:
                            t_rcq = tail_stage1(*deferred)
                        elif m == 5:
                            t_bcs = tail_stage2(t_rcq)
                        elif m == 8:
                            tail_stage3(*deferred, *t_bcs)
                            deferred = None
                    elif m == 10 and proj_due is not None:
                        emit_proj(proj_due)
                        proj_due = None
                while pend:
                    flush_pv()
                deferred = (w, hp, pv0, pv1)
                if hp == 3:
                    proj_due = w
        t_rcq = tail_stage1(*deferred)
        t_bcs = tail_stage2(t_rcq)
        tail_stage3(*deferred, *t_bcs)
        emit_proj(proj_due)


def _build():
    import concourse.tile as tile
    from concourse import bacc, mybir

    nc = bacc.Bacc("TRN2", target_bir_lowering=False, debug=False)
    f32 = mybir.dt.float32
    io = {
        "xb": nc.dram_tensor("xb", [C, N], f32, kind="ExternalInput").ap(),
        "qkvw": nc.dram_tensor("qkvw", [3 * C, C], f32, kind="ExternalInput").ap(),
        "qkvb": nc.dram_tensor("qkvb", [3 * C], f32, kind="ExternalInput").ap(),
        "projw": nc.dram_tensor("projw", [C, C], f32, kind="ExternalInput").ap(),
        "projb": nc.dram_tensor("projb", [C], f32, kind="ExternalInput").ap(),
        "nw": nc.dram_tensor("nw", [C], f32, kind="ExternalInput").ap(),
        "nb": nc.dram_tensor("nb", [C], f32, kind="ExternalInput").ap(),
        "cid": nc.dram_tensor("cid", [128, 128], mybir.dt.bfloat16,
                              kind="ExternalInput").ap(),
        "cind": nc.dram_tensor("cind", [128, 2], f32, kind="ExternalInput").ap(),
        "cindT": nc.dram_tensor("cindT", [2, 128], f32, kind="ExternalInput").ap(),
        "y": nc.dram_tensor("y", [C, NQ], f32, kind="ExternalOutput").ap(),
    }
    with tile.TileContext(nc) as tc:
        _emit(tc, io)
    nc.compile()
    return nc


def get_compiled():
    global _COMPILED
    if _COMPILED is None:
        _COMPILED = _build()
    return _COMPILED


def make_in_maps(x, norm_w, norm_b, qkv_w, qkv_b, proj_w, proj_b):
    import ml_dtypes

    xf = np.ascontiguousarray(np.asarray(x, np.float32)).reshape(2, C, N)
    ind = np.zeros((128, 2), np.float32)
    ind[0:64, 0] = 1.0
    ind[64:128, 1] = 1.0
    shared = {
        "cid": np.eye(128, dtype=ml_dtypes.bfloat16),
        "cind": ind,
        "cindT": np.ascontiguousarray(ind.T),
        "qkvw": np.ascontiguousarray(np.asarray(qkv_w, np.float32)),
        "qkvb": np.ascontiguousarray(np.asarray(qkv_b, np.float32)),
        "projw": np.ascontiguousarray(np.asarray(proj_w, np.float32)),
        "projb": np.ascontiguousarray(np.asarray(proj_b, np.float32)),
        "nw": np.ascontiguousarray(np.asarray(norm_w, np.float32)),
        "nb": np.ascontiguousarray(np.asarray(norm_b, np.float32)),
    }
    in_maps = []
    for core in range(8):
        bi, qs = core // 4, core % 4
        # rotate so this core's queries are always columns [0:NQ)
        xroll = np.concatenate(
            [xf[bi][:, qs * NQ:], xf[bi][:, :qs * NQ]], axis=1)
        m = dict(shared)
        m["xb"] = np.ascontiguousarray(xroll)
        in_maps.append(m)
    return in_maps


def assemble(results, x):
    y = np.zeros((2, C, N), np.float32)
    for core in range(8):
        bi, qs = core // 4, core % 4
        y[bi][:, qs * NQ:(qs + 1) * NQ] = results[core]["y"]
    return y.reshape(x.shape)


def kernel(x, norm_w, norm_b, qkv_w, qkv_b, proj_w, proj_b, **_ignored):
    from concourse import bass_utils

    nc = get_compiled()
    in_maps = make_in_maps(x, norm_w, norm_b, qkv_w, qkv_b, proj_w, proj_b)
    res = bass_utils.run_bass_kernel_spmd(nc, in_maps, core_ids=list(range(8)))
    return assemble(res.results, np.asarray(x))


# revision 32
# speedup vs baseline: 2.0557x; 1.0034x over previous
"""Trainium2 Bass kernel for nn_AttentionBlock (GroupNorm + MHA + proj + residual).

Full inputs in, full output out. Sharding: 8 cores = 2 batches x 4 query-slices.
Each core: GroupNorm over its batch image, q projection for its 1024 queries,
k/v projections over all 4096 keys, per-head attention (S^T = k^T q, softmax
along the PSUM partition axis via an appended ones-column in the PV matmul),
output projection and residual for its query slice.

v2 performance structure:
 - Warmup matmuls trip the PE HAM clock gate early (else everything runs at
   1.2 GHz instead of 2.4 GHz).
 - Heads are processed in pairs: head-even uses PE rows 0:63, head-odd rows
   64:127 (tile_position row groups) so the two S matmuls per key-tile run
   concurrently in the PE array and LDWEIGHTS overlaps streaming.
 - softmax exp alternates between ScalarE (real exp, even key-tiles) and
   VectorE (Schraudolph bit-trick exp -> bf16 bit pattern via an int16
   round, odd key-tiles), halving the exp bottleneck.
 - PV matmuls lag S by 2 key-tiles (3-deep PSUM ring) so the exp latency is
   off the PE critical path.

All matmuls run in bf16 with fp32 PSUM accumulation; softmax logits stay fp32.
"""
import numpy as np

C = 512          # channels
N = 4096         # pixels (64*64)
NQ = 1024        # queries per core
H = 8            # heads
D = 64           # head dim
T = 4            # 128-channel chunks
W = NQ // 512    # query windows of 512
MT = N // 128    # key m-tiles of 128
NGROUPS = 8
EPS = 1e-5
GELEM = (C // NGROUPS) * N   # elements per norm group

# Schraudolph fast-exp (fp8e5m2 bits via int8 round-to-nearest):
#   bits = round(raw * SA5 + SB5);  bitcast(int8->fp8e5) ~ exp(0.125*raw)
# SA5 = 0.125 * log2(e) * 4 ; SB5 = 15*4 - 0.21875 (max rel err 11.7%,
# same order as the direct e5m2 quantization of a true exp)
SA5 = 0.7213475204444817
SB5 = 59.78125

N_WARM1 = 60     # warmup MMs before transposes (covers input DMA)
N_WARM2 = 20     # warmup MMs per groupnorm chunk
N_WARM3 = 80     # warmup MMs after groupnorm emission

_COMPILED = None


def _emit(tc, io):
    import concourse.bass as bass
    from concourse import mybir
    from contextlib import ExitStack

    nc = tc.nc
    f32 = mybir.dt.float32
    bf16 = mybir.dt.bfloat16
    i8 = mybir.dt.int8
    f8 = mybir.dt.float8e5
    Alu = mybir.AluOpType
    Act = mybir.ActivationFunctionType

    xb, qkvw, qkvb, projw, projb, nw, nb, y = (
        io["xb"], io["qkvw"], io["qkvb"], io["projw"], io["projb"],
        io["nw"], io["nb"], io["y"])

    ctx = ExitStack()
    with ctx:
        # ---------------- pools ----------------
        # PSUM: sp ring 3x(128,1024) [6 banks] + pv 2x(128,512) [2 banks]
        left = ctx.enter_context(tc.tile_pool(name="left", bufs=1))
        psum_sp = ctx.enter_context(tc.tile_pool(name="psum_sp", bufs=3, space="PSUM"))
        psum_pv = ctx.enter_context(tc.tile_pool(name="psum_pv", bufs=2, space="PSUM"))

        right_ctx = ExitStack()
        xf_pool = right_ctx.enter_context(
            tc.tile_pool(name="xf_pool", bufs=1, side="right"))
        wstg_pool = right_ctx.enter_context(
            tc.tile_pool(name="wstg_pool", bufs=4, side="right"))
        scr_pool = right_ctx.enter_context(
            tc.tile_pool(name="scr_pool", bufs=2, side="right"))

        # ---------------- persistent tiles ----------------
        # fp8 activations/weights for DoubleRow GEMMs, chunk-major layouts
        xn8 = left.tile([128, T * N], f8, name="xn8", tag="xn8")
        ksb = [left.tile([128, N], bf16, name=f"ksb{t}", tag=f"ksb{t}") for t in range(T)]
        qsb = [left.tile([128, NQ], bf16, name=f"qsb{t}", tag=f"qsb{t}") for t in range(T)]
        wTq8 = left.tile([128, T * 1536], f8, name="wTq8", tag="wTq8")
        wTp8 = left.tile([128, T * C], f8, name="wTp8", tag="wTp8")
        vb_bc = left.tile([128, C], f32, name="vb_bc", tag="vb_bc")
        ones_row = left.tile([1, D], f32, name="ones_row", tag="ones_row")
        qb = [left.tile([128, 1], f32, name=f"qb{i}", tag=f"qb{i}") for i in range(8)]
        pb = [left.tile([128, 1], f32, name=f"pb{i}", tag=f"pb{i}") for i in range(T)]
        nwt = [left.tile([128, 1], f32, name=f"nwt{t}", tag=f"nwt{t}") for t in range(T)]
        nbt = [left.tile([128, 1], f32, name=f"nbt{t}", tag=f"nbt{t}") for t in range(T)]
        stat = [left.tile([128, 2], f32, name=f"stat{t}", tag=f"stat{t}") for t in range(T)]
        gstat = [left.tile([128, 2], f32, name=f"gstat{t}", tag=f"gstat{t}") for t in range(T)]
        wu = left.tile([128, 512], bf16, name="wu", tag="wu")

        # ---------------- warmup: trip the HAM clock gate ----------------
        nc.vector.memset(wu[:], 0.125)
        wu_ps = psum_pv.tile([128, 512], f32, name="wu_ps", tag="pv")
        # preload the exp activation table while idle
        wu_exp = scr_pool.tile([1, 8], f32, name="wu_exp", tag="wu_exp")
        nc.scalar.activation(wu_exp[0:1, :], wu[0:1, 0:8], Act.Exp)

        def warm(n):
            for _ in range(n):
                nc.tensor.matmul(wu_ps[:], wu[:, 0:128], wu[:],
                                 start=True, stop=True)
        warm(N_WARM1)

        # ---------------- input DMAs ----------------
        xf = [xf_pool.tile([128, N], f32, name=f"xf{t}", tag=f"xf{t}") for t in range(T)]
        for t in range(T):
            for c4 in range(4):   # split across DMA queues
                nc.sync.dma_start(
                    xf[t][:, 1024 * c4:1024 * (c4 + 1)],
                    xb[128 * t:128 * (t + 1), 1024 * c4:1024 * (c4 + 1)])
            nc.sync.dma_start(nwt[t][:, 0:1], nw[128 * t:128 * (t + 1)])
            nc.sync.dma_start(nbt[t][:, 0:1], nb[128 * t:128 * (t + 1)])
            nc.sync.dma_start(pb[t][:, 0:1], projb[128 * t:128 * (t + 1)])
        for i in range(8):
            nc.sync.dma_start(qb[i][:, 0:1], qkvb[128 * i:128 * (i + 1)])
        # v bias broadcast to 128 partitions (stride-0 partition read)
        nc.gpsimd.dma_start(
            out=vb_bc[:],
            in_=bass.AP(tensor=qkvb.tensor, offset=1024, ap=[[0, 128], [1, C]]))
        nc.vector.memset(ones_row[0:1, :], 1.0)

        # weights: natural-layout contiguous DMA, cast to bf16, transpose
        # 128x128 blocks on the PE (identity trick) into wTq/wTp.
        ident = left.tile([128, 128], bf16, name="ident", tag="ident")
        nc.sync.dma_start(ident[:], io["cid"][:, :])
        ind = left.tile([128, 2], f32, name="ind", tag="ind")
        nc.sync.dma_start(ind[:], io["cind"][:, :])
        indT = left.tile([2, 128], f32, name="indT", tag="indT")
        nc.sync.dma_start(indT[0:2, :], io["cindT"][:, :])
        for i in range(12):   # qkv_w row-tiles
            wstg = wstg_pool.tile([128, C], f32, name="wstg", tag="wstg")
            nc.sync.dma_start(wstg[:], qkvw[128 * i:128 * (i + 1), :])
            wbf = wstg_pool.tile([128, C], bf16, name="wbf", tag="wbf")
            nc.vector.tensor_copy(wbf[:], wstg[:])
            for j in range(T):
                tp = psum_sp.tile([128, 1024], bf16, name="tp", tag="sp")
                nc.tensor.transpose(tp[:, 0:128], wbf[:, 128 * j:128 * (j + 1)], ident[:])
                nc.vector.tensor_copy(
                    wTq8[:, 1536 * j + 128 * i:1536 * j + 128 * (i + 1)], tp[:, 0:128])
        for i in range(4):    # proj_w row-tiles
            wstg = wstg_pool.tile([128, C], f32, name="wstg", tag="wstg")
            nc.sync.dma_start(wstg[:], projw[128 * i:128 * (i + 1), :])
            wbf = wstg_pool.tile([128, C], bf16, name="wbf", tag="wbf")
            nc.vector.tensor_copy(wbf[:], wstg[:])
            for j in range(T):
                tp = psum_sp.tile([128, 1024], bf16, name="tp", tag="sp")
                nc.tensor.transpose(tp[:, 0:128], wbf[:, 128 * j:128 * (j + 1)], ident[:])
                nc.vector.tensor_copy(
                    wTp8[:, C * j + 128 * i:C * j + 128 * (i + 1)], tp[:, 0:128])

        # ---------------- phase 1: group stats + normalize ----------------
        for t in range(T):
            nc.vector.tensor_reduce(
                out=stat[t][:, 0:1], in_=xf[t][:], axis=mybir.AxisListType.X, op=Alu.add)
            sq_scr = scr_pool.tile([128, N], bf16, name="sq_scr", tag="sq_scr")
            nc.scalar.activation(
                sq_scr[:], xf[t][:], Act.Square, accum_out=stat[t][:, 1:2])
            # group-reduce over partitions via indicator matmuls
            gg_ps = psum_pv.tile([128, 512], f32, name="gg_ps", tag="pv")
            nc.tensor.matmul(gg_ps[0:2, 0:2], ind[:, 0:2], stat[t][:, 0:2],
                             start=True, stop=True)
            gg_sb = left.tile([2, 2], f32, name=f"gg_sb{t}", tag=f"gg_sb{t}")
            nc.vector.tensor_copy(gg_sb[0:2, :], gg_ps[0:2, 0:2])
            gb_ps = psum_pv.tile([128, 512], f32, name="gb_ps", tag="pv")
            nc.tensor.matmul(gb_ps[:, 0:2], indT[0:2, :], gg_sb[0:2, :],
                             start=True, stop=True)
            nc.vector.tensor_copy(gstat[t][:, 0:2], gb_ps[:, 0:2])
            # mean/var/rstd -> per-channel affine a,b
            mean_t = left.tile([128, 1], f32, name=f"mean{t}", tag=f"mean{t}")
            e2_t = left.tile([128, 1], f32, name=f"e2{t}", tag=f"e2{t}")
            var_t = left.tile([128, 1], f32, name=f"var{t}", tag=f"var{t}")
            std_t = left.tile([128, 1], f32, name=f"std{t}", tag=f"std{t}")
            a_t = left.tile([128, 1], f32, name=f"a{t}", tag=f"a{t}")
            b_t = left.tile([128, 1], f32, name=f"b{t}", tag=f"b{t}")
            inv = 1.0 / GELEM
            nc.vector.tensor_scalar(mean_t[:], gstat[t][:, 0:1], inv, None, Alu.mult)
            nc.vector.tensor_scalar(e2_t[:], gstat[t][:, 1:2], inv, None, Alu.mult)
            nc.vector.scalar_tensor_tensor(
                var_t[:], mean_t[:], -1.0, mean_t[:], Alu.mult, Alu.mult)
            nc.vector.scalar_tensor_tensor(
                var_t[:], e2_t[:], EPS, var_t[:], Alu.add, Alu.add)
            nc.scalar.activation(std_t[:], var_t[:], Act.Sqrt)
            nc.vector.reciprocal(a_t[:], std_t[:])
            nc.vector.tensor_tensor(a_t[:], a_t[:], nwt[t][:], Alu.mult)
            nc.vector.tensor_tensor(b_t[:], mean_t[:], a_t[:], Alu.mult)
            nc.vector.tensor_tensor(b_t[:], nbt[t][:], b_t[:], Alu.subtract)
            # normalize + cast to fp8 (alternate engines to halve the chain)
            if t % 2 == 0:
                nc.scalar.activation(
                    xn8[:, N * t:N * (t + 1)], xf[t][:], Act.Identity,
                    bias=b_t[:, 0:1], scale=a_t[:, 0:1])
            else:
                nc.vector.tensor_scalar(
                    xn8[:, N * t:N * (t + 1)], xf[t][:],
                    a_t[:, 0:1], b_t[:, 0:1], Alu.mult, Alu.add)
            warm(N_WARM2)
        warm(N_WARM3)

        right_ctx.close()

        # ---------------- mid pools (reuse xf space) ----------------
        mid = ctx.enter_context(tc.tile_pool(name="mid", bufs=1))
        ps8_pool = ctx.enter_context(tc.tile_pool(name="ps8_pool", bufs=6))
        rec_pool = ctx.enter_context(tc.tile_pool(name="rec_pool", bufs=2))
        yo_pool = ctx.enter_context(tc.tile_pool(name="yo_pool", bufs=2))

        # fp8e5 v^T for DoubleRow PV: per m-pair m2 and head h, 160 cols =
        # two 80-wide blocks (64 v + ones col @64 + 15 zero pad), one per
        # m-tile of the pair
        vT8 = mid.tile([128, (MT // 2) * 1280], f8, name="vT8", tag="vT8")
        attn8 = mid.tile([128, T * NQ], f8, name="attn8", tag="attn8")
        xres = [mid.tile([128, NQ], f32, name=f"xres{t}", tag=f"xres{t}") for t in range(T)]
        for t in range(T):
            nc.sync.dma_start(xres[t][:], xb[128 * t:128 * (t + 1), 0:NQ])

        # ones + zero-pad columns of the augmented v^T (denominator trick)
        v80 = vT8[:].rearrange("p (n x) -> p n x", x=80)
        nc.vector.memset(v80[:, :, 64:65], 1.0)
        nc.vector.memset(v80[:, :, 65:80], 0.0)

        # ---------------- phase 3: projections (fp8 DoubleRow) ----------
        DR = mybir.MatmulPerfMode.DoubleRow

        def wq_pair(pr, lo, hi):
            return wTq8[:, 3072 * pr:3072 * (pr + 1)].rearrange(
                "p (j x) -> p j x", j=2)[:, :, lo:hi]

        def xn_pair(pr, lo, hi):
            return xn8[:, 2 * N * pr:2 * N * (pr + 1)].rearrange(
                "p (j x) -> p j x", j=2)[:, :, lo:hi]

        # q: out rows 0..511 of qkv, only NQ query columns (2 windows per tile)
        for i in range(T):
            qp = psum_sp.tile([128, 1024], f32, name="qp", tag="sp")
            for w in range(W):
                for pr in range(2):
                    nc.tensor.matmul(
                        qp[:, 512 * w:512 * w + 512],
                        wq_pair(pr, 128 * i, 128 * i + 128),
                        xn_pair(pr, 512 * w, 512 * w + 512),
                        start=(pr == 0), stop=(pr == 1), perf_mode=DR)
            if i % 2 == 0:
                nc.scalar.add(qsb[i][:], qp[:], qb[i][:, 0:1])
            else:
                nc.vector.tensor_scalar(
                    qsb[i][:], qp[:], qb[i][:, 0:1], None, Alu.add)
        # k: out rows 512..1023, all N columns (2 windows per psum tile)
        for i in range(T):
            for w2 in range(4):
                kp = psum_sp.tile([128, 1024], f32, name="kp", tag="sp")
                for w in (2 * w2, 2 * w2 + 1):
                    for pr in range(2):
                        nc.tensor.matmul(
                            kp[:, 512 * (w % 2):512 * (w % 2) + 512],
                            wq_pair(pr, 512 + 128 * i, 512 + 128 * i + 128),
                            xn_pair(pr, 512 * w, 512 * w + 512),
                            start=(pr == 0), stop=(pr == 1), perf_mode=DR)
                if w2 % 2 == 0:
                    nc.scalar.add(
                        ksb[i][:, 1024 * w2:1024 * (w2 + 1)], kp[:],
                        qb[4 + i][:, 0:1])
                else:
                    nc.vector.tensor_scalar(
                        ksb[i][:, 1024 * w2:1024 * (w2 + 1)], kp[:],
                        qb[4 + i][:, 0:1], None, Alu.add)
        # vT: (m, 512) per m-tile, two m-tiles per psum tile, strided into
        # the 80-column augmented fp8 layout
        for m2 in range(MT // 2):
            vp = psum_sp.tile([128, 1024], f32, name="vp", tag="sp")
            for mh in range(2):
                mt = 2 * m2 + mh
                for pr in range(2):
                    nc.tensor.matmul(
                        vp[:, 512 * mh:512 * mh + 512],
                        xn_pair(pr, 128 * mt, 128 * mt + 128),
                        wq_pair(pr, 1024, 1536),
                        start=(pr == 0), stop=(pr == 1), perf_mode=DR)
            vbv = vb_bc[:].rearrange("p (h x) -> p h x", x=64)
            vdst = vT8[:, 1280 * m2:1280 * (m2 + 1)].rearrange(
                "p (h j x) -> p h j x", h=8, j=2, x=80)
            for mh in range(2):
                dst = vdst[:, :, mh, 0:64]
                src = vp[:, 512 * mh:512 * mh + 512].rearrange(
                    "p (h x) -> p h x", x=64)
                nc.vector.tensor_tensor(dst, src, vbv, Alu.add)

        # ---------------- phase 4+5: attention, deferred tails, proj ----
        # tails are emitted in stages inside the NEXT unit's m-loop so the
        # PE stream never waits on the reciprocal chain
        def tail_stage1(tw, thp, tpv0, tpv1):
            # denominators stacked on partition 0 + one reciprocal
            dnq = rec_pool.tile([1, 1024], f32, name="dnq", tag="dnq")
            nc.scalar.copy(dnq[0:1, 0:512], tpv0[64:65, :])
            nc.scalar.copy(dnq[0:1, 512:1024], tpv1[64:65, :])
            rcq = rec_pool.tile([1, 1024], f32, name="rcq", tag="rcq")
            rscr = rec_pool.tile([1, 1024], f32, name="rscr", tag="rscr")
            nc.vector.reciprocal_approx_accurate(
                rcq[0:1, :], dnq[0:1, :], rscr[0:1, :])
            return rcq

        def tail_stage2(rcq):
            # partition-broadcast of the reciprocals via two PE matmuls
            # (emitted mid-next-unit, so the reciprocal chain is already done)
            bc = psum_sp.tile([128, 1024], f32, name="bc", tag="sp")
            nc.tensor.matmul(
                bc[0:64, 0:512], ones_row[0:1, 0:D],
                rcq[0:1, 0:512], start=True, stop=True)
            nc.tensor.matmul(
                bc[64:128, 0:512], ones_row[0:1, 0:D],
                rcq[0:1, 512:1024], start=True, stop=True)
            bcs0 = rec_pool.tile([64, 512], f32, name="bcs0", tag="bcs0")
            bcs1 = rec_pool.tile([64, 512], f32, name="bcs1", tag="bcs1")
            nc.scalar.copy(bcs0[0:64, :], bc[0:64, 0:512])
            nc.scalar.copy(bcs1[0:64, :], bc[64:128, 0:512])
            return bcs0, bcs1

        def tail_stage3(tw, thp, tpv0, tpv1, bcs0, bcs1):
            nc.vector.tensor_tensor(
                attn8[0:64, NQ * thp + 512 * tw:NQ * thp + 512 * tw + 512],
                tpv0[0:64, :], bcs0[0:64, :], Alu.mult)
            nc.vector.tensor_tensor(
                attn8[64:128, NQ * thp + 512 * tw:NQ * thp + 512 * tw + 512],
                tpv1[0:64, :], bcs1[0:64, :], Alu.mult)

        def emit_proj(pw):
            # proj + residual for window pw (fp8 DoubleRow over attn8)
            for i in range(T):
                py = psum_sp.tile([128, 1024], f32, name="py", tag="sp")
                for pr in range(2):
                    rhs = attn8[:, 2 * NQ * pr:2 * NQ * (pr + 1)].rearrange(
                        "p (j x) -> p j x", j=2)[:, :, 512 * pw:512 * pw + 512]
                    lhs = wTp8[:, 1024 * pr:1024 * (pr + 1)].rearrange(
                        "p (j x) -> p j x", j=2)[:, :, 128 * i:128 * i + 128]
                    nc.tensor.matmul(
                        py[:, 0:512], lhs, rhs,
                        start=(pr == 0), stop=(pr == 1), perf_mode=DR)
                yo = yo_pool.tile([128, 512], f32, name="yo", tag="yo")
                nc.vector.scalar_tensor_tensor(
                    yo[:], py[:, 0:512], pb[i][:, 0:1],
                    xres[i][:, 512 * pw:512 * pw + 512], Alu.add, Alu.add)
                nc.sync.dma_start(
                    y[128 * i:128 * i + 128, 512 * pw:512 * pw + 512], yo[:])

        pend = []             # (m2, ps8, hp, pv0, pv1) across units

        def flush_pv():
            m2, ps8t, fhp, fpv0, fpv1 = pend.pop(0)
            st, sto = (m2 == 0), (m2 == MT // 2 - 1)
            vb = 1280 * m2
            for h, pv in ((0, fpv0), (1, fpv1)):
                lhs = vT8[:, vb + 160 * (2 * fhp + h):
                          vb + 160 * (2 * fhp + h) + 160].rearrange(
                    "p (j x) -> p j x", j=2)
                rhs = ps8t[:, 1024 * h:1024 * h + 1024].rearrange(
                    "p (j x) -> p j x", j=2)
                nc.tensor.matmul(
                    pv[0:80, :], lhs, rhs, start=st, stop=sto,
                    perf_mode=DR)

        deferred = None       # (w, hp, pv0, pv1) of the previous unit
        proj_due = None       # window whose proj should be emitted next
        for w in range(W):
            for hp in range(4):
                # drain the previous unit's leftover PV flushes first: the
                # engines finish its last exps while these stream
                while pend:
                    flush_pv()
                h0, h1 = 2 * hp, 2 * hp + 1
                pv0 = psum_pv.tile([128, 512], f32, name="pv0", tag="pv")
                pv1 = psum_pv.tile([128, 512], f32, name="pv1", tag="pv")

                ps8 = None
                for m in range(MT):
                    sp = psum_sp.tile([128, 1024], f32, name="sp", tag="sp")
                    # S pair: head-even on PE rows 0:63, head-odd on rows
                    # 64:127 (concurrent row groups, shared LDW window)
                    nc.tensor.matmul(
                        sp[:, 0:512],
                        ksb[hp][0:64, 128 * m:128 * m + 128],
                        qsb[hp][0:64, 512 * w:512 * w + 512],
                        start=True, stop=True)
                    nc.tensor.matmul(
                        sp[:, 512:1024],
                        ksb[hp][64:128, 128 * m:128 * m + 128],
                        qsb[hp][64:128, 512 * w:512 * w + 512],
                        start=True, stop=True)
                    m2, mh = divmod(m, 2)
                    if mh == 0:
                        ps8 = ps8_pool.tile([128, 2048], f8, name="ps8", tag="ps8")
                    # exp of this m-tile for both heads into the fp8 pair
                    # tile: cols [1024h + 512mh : +512] = (head h, m-half mh)
                    outv = ps8[:].rearrange(
                        "p (H x) -> p H x", H=2)[:, :, 512 * mh:512 * mh + 512]
                    inv = sp[:].rearrange("p (h x) -> p h x", h=2)
                    # ACT handles even m-tiles plus one odd (engine balance);
                    # DVE does the rest via the int8 Schraudolph bit trick
                    if mh == 0 or m == 13:
                        nc.scalar.activation(outv, inv, Act.Exp, scale=0.125)
                    else:
                        nc.vector.tensor_scalar(
                            outv.bitcast(i8), inv, SA5, SB5, Alu.mult, Alu.add)
                    if mh == 1:
                        pend.append((m2, ps8, hp, pv0, pv1))
                        if len(pend) >= 5:
                            flush_pv()
                    # previous unit's tail stages + any due proj, overlapped
                    # with this unit's m-loop (keeps the PE stream dense)
                    if deferred is not None:
                        if m == 2:
                            t_rcq = tail_stage1(*deferred)
                        elif m == 5:
                            t_bcs = tail_stage2(t_rcq)
                        elif m == 8:
                            tail_stage3(*deferred, *t_bcs)
                            deferred = None
                    elif m == 10 and proj_due is not None:
                        emit_proj(proj_due)
                        proj_due = None
                deferred = (w, hp, pv0, pv1)
                if hp == 3:
                    proj_due = w
        while pend:
            flush_pv()
        t_rcq = tail_stage1(*deferred)
        t_bcs = tail_stage2(t_rcq)
        tail_stage3(*deferred, *t_bcs)
        emit_proj(proj_due)


def _build():
    import concourse.tile as tile
    from concourse import bacc, mybir

    nc = bacc.Bacc("TRN2", target_bir_lowering=False, debug=False)
    f32 = mybir.dt.float32
    io = {
        "xb": nc.dram_tensor("xb", [C, N], f32, kind="ExternalInput").ap(),
        "qkvw": nc.dram_tensor("qkvw", [3 * C, C], f32, kind="ExternalInput").ap(),
        "qkvb": nc.dram_tensor("qkvb", [3 * C], f32, kind="ExternalInput").ap(),
        "projw": nc.dram_tensor("projw", [C, C], f32, kind="ExternalInput").ap(),
        "projb": nc.dram_tensor("projb", [C], f32, kind="ExternalInput").ap(),
        "nw": nc.dram_tensor("nw", [C], f32, kind="ExternalInput").ap(),
        "nb": nc.dram_tensor("nb", [C], f32, kind="ExternalInput").ap(),
        "cid": nc.dram_tensor("cid", [128, 128], mybir.dt.bfloat16,
                              kind="ExternalInput").ap(),
        "cind": nc.dram_tensor("cind", [128, 2], f32, kind="ExternalInput").ap(),
        "cindT": nc.dram_tensor("cindT", [2, 128], f32, kind="ExternalInput").ap(),
        "y": nc.dram_tensor("y", [C, NQ], f32, kind="ExternalOutput").ap(),
    }
    with tile.TileContext(nc) as tc:
        _emit(tc, io)
    nc.compile()
    return nc


def get_compiled():
    global _COMPILED
    if _COMPILED is None:
        _COMPILED = _build()
    return _COMPILED


def make_in_maps(x, norm_w, norm_b, qkv_w, qkv_b, proj_w, proj_b):
    import ml_dtypes

    xf = np.ascontiguousarray(np.asarray(x, np.float32)).reshape(2, C, N)
    ind = np.zeros((128, 2), np.float32)
    ind[0:64, 0] = 1.0
    ind[64:128, 1] = 1.0
    shared = {
        "cid": np.eye(128, dtype=ml_dtypes.bfloat16),
        "cind": ind,
        "cindT": np.ascontiguousarray(ind.T),
        "qkvw": np.ascontiguousarray(np.asarray(qkv_w, np.float32)),
        "qkvb": np.ascontiguousarray(np.asarray(qkv_b, np.float32)),
        "projw": np.ascontiguousarray(np.asarray(proj_w, np.float32)),
        "projb": np.ascontiguousarray(np.asarray(proj_b, np.float32)),
        "nw": np.ascontiguousarray(np.asarray(norm_w, np.float32)),
        "nb": np.ascontiguousarray(np.asarray(norm_b, np.float32)),
    }
    in_maps = []
    for core in range(8):
        bi, qs = core // 4, core % 4
        # rotate so this core's queries are always columns [0:NQ)
        xroll = np.concatenate(
            [xf[bi][:, qs * NQ:], xf[bi][:, :qs * NQ]], axis=1)
        m = dict(shared)
        m["xb"] = np.ascontiguousarray(xroll)
        in_maps.append(m)
    return in_maps


def assemble(results, x):
    y = np.zeros((2, C, N), np.float32)
    for core in range(8):
        bi, qs = core // 4, core % 4
        y[bi][:, qs * NQ:(qs + 1) * NQ] = results[core]["y"]
    return y.reshape(x.shape)


def kernel(x, norm_w, norm_b, qkv_w, qkv_b, proj_w, proj_b, **_ignored):
    from concourse import bass_utils

    nc = get_compiled()
    in_maps = make_in_maps(x, norm_w, norm_b, qkv_w, qkv_b, proj_w, proj_b)
    res = bass_utils.run_bass_kernel_spmd(nc, in_maps, core_ids=list(range(8)))
    return assemble(res.results, np.asarray(x))


# revision 34
# speedup vs baseline: 2.1056x; 1.0243x over previous
"""Trainium2 Bass kernel for nn_AttentionBlock (GroupNorm + MHA + proj + residual).

Full inputs in, full output out. Sharding: 8 cores = 2 batches x 4 query-slices.
Each core: GroupNorm over its batch image, q projection for its 1024 queries,
k/v projections over all 4096 keys, per-head attention (S^T = k^T q, softmax
along the PSUM partition axis via an appended ones-column in the PV matmul),
output projection and residual for its query slice.

v2 performance structure:
 - Warmup matmuls trip the PE HAM clock gate early (else everything runs at
   1.2 GHz instead of 2.4 GHz).
 - Heads are processed in pairs: head-even uses PE rows 0:63, head-odd rows
   64:127 (tile_position row groups) so the two S matmuls per key-tile run
   concurrently in the PE array and LDWEIGHTS overlaps streaming.
 - softmax exp alternates between ScalarE (real exp, even key-tiles) and
   VectorE (Schraudolph bit-trick exp -> bf16 bit pattern via an int16
   round, odd key-tiles), halving the exp bottleneck.
 - PV matmuls lag S by 2 key-tiles (3-deep PSUM ring) so the exp latency is
   off the PE critical path.

All matmuls run in bf16 with fp32 PSUM accumulation; softmax logits stay fp32.
"""
import numpy as np

C = 512          # channels
N = 4096         # pixels (64*64)
NQ = 1024        # queries per core
H = 8            # heads
D = 64           # head dim
T = 4            # 128-channel chunks
W = NQ // 512    # query windows of 512
MT = N // 128    # key m-tiles of 128
NGROUPS = 8
EPS = 1e-5
GELEM = (C // NGROUPS) * N   # elements per norm group

# Schraudolph fast-exp (fp8e5m2 bits via int8 round-to-nearest):
#   bits = round(raw * SA5 + SB5);  bitcast(int8->fp8e5) ~ exp(0.125*raw)
# SA5 = 0.125 * log2(e) * 4 ; SB5 = 15*4 - 0.21875 (max rel err 11.7%,
# same order as the direct e5m2 quantization of a true exp)
SA5 = 0.7213475204444817
SB5 = 59.78125

N_WARM1 = 60     # warmup MMs before transposes (covers input DMA)
N_WARM2 = 20     # warmup MMs per groupnorm chunk
N_WARM3 = 80     # warmup MMs after groupnorm emission

_COMPILED = None


def _emit(tc, io):
    import concourse.bass as bass
    from concourse import mybir
    from contextlib import ExitStack

    nc = tc.nc
    f32 = mybir.dt.float32
    bf16 = mybir.dt.bfloat16
    i8 = mybir.dt.int8
    f8 = mybir.dt.float8e5
    Alu = mybir.AluOpType
    Act = mybir.ActivationFunctionType

    xb, qkvw, qkvb, projw, projb, nw, nb, y = (
        io["xb"], io["qkvw"], io["qkvb"], io["projw"], io["projb"],
        io["nw"], io["nb"], io["y"])

    ctx = ExitStack()
    with ctx:
        # ---------------- pools ----------------
        # PSUM: sp ring 3x(128,1024) [6 banks] + pv 2x(128,512) [2 banks]
        left = ctx.enter_context(tc.tile_pool(name="left", bufs=1))
        psum_sp = ctx.enter_context(tc.tile_pool(name="psum_sp", bufs=3, space="PSUM"))
        psum_pv = ctx.enter_context(tc.tile_pool(name="psum_pv", bufs=2, space="PSUM"))

        right_ctx = ExitStack()
        xf_pool = right_ctx.enter_context(
            tc.tile_pool(name="xf_pool", bufs=1, side="right"))
        wstg_pool = right_ctx.enter_context(
            tc.tile_pool(name="wstg_pool", bufs=4, side="right"))
        scr_pool = right_ctx.enter_context(
            tc.tile_pool(name="scr_pool", bufs=2, side="right"))

        # ---------------- persistent tiles ----------------
        # fp8 activations/weights for DoubleRow GEMMs, chunk-major layouts
        xn8 = left.tile([128, T * N], f8, name="xn8", tag="xn8")
        ksb = [left.tile([128, N], bf16, name=f"ksb{t}", tag=f"ksb{t}") for t in range(T)]
        qsb = [left.tile([128, NQ], bf16, name=f"qsb{t}", tag=f"qsb{t}") for t in range(T)]
        wTq8 = left.tile([128, T * 1536], f8, name="wTq8", tag="wTq8")
        wTp8 = left.tile([128, T * C], f8, name="wTp8", tag="wTp8")
        vb_bc = left.tile([128, C], f32, name="vb_bc", tag="vb_bc")
        ones_row = left.tile([1, D], f32, name="ones_row", tag="ones_row")
        qb = [left.tile([128, 1], f32, name=f"qb{i}", tag=f"qb{i}") for i in range(8)]
        pb = [left.tile([128, 1], f32, name=f"pb{i}", tag=f"pb{i}") for i in range(T)]
        nwt = [left.tile([128, 1], f32, name=f"nwt{t}", tag=f"nwt{t}") for t in range(T)]
        nbt = [left.tile([128, 1], f32, name=f"nbt{t}", tag=f"nbt{t}") for t in range(T)]
        stat = [left.tile([128, 2], f32, name=f"stat{t}", tag=f"stat{t}") for t in range(T)]
        gstat = [left.tile([128, 2], f32, name=f"gstat{t}", tag=f"gstat{t}") for t in range(T)]
        wu = left.tile([128, 512], bf16, name="wu", tag="wu")

        # ---------------- warmup: trip the HAM clock gate ----------------
        nc.vector.memset(wu[:], 0.125)
        wu_ps = psum_pv.tile([128, 512], f32, name="wu_ps", tag="pv")
        # preload the exp activation table while idle
        wu_exp = scr_pool.tile([1, 8], f32, name="wu_exp", tag="wu_exp")
        nc.scalar.activation(wu_exp[0:1, :], wu[0:1, 0:8], Act.Exp)

        def warm(n):
            for _ in range(n):
                nc.tensor.matmul(wu_ps[:], wu[:, 0:128], wu[:],
                                 start=True, stop=True)
        warm(N_WARM1)

        # ---------------- input DMAs ----------------
        xf = [xf_pool.tile([128, N], f32, name=f"xf{t}", tag=f"xf{t}") for t in range(T)]
        for t in range(T):
            for c4 in range(4):   # split across DMA queues
                nc.sync.dma_start(
                    xf[t][:, 1024 * c4:1024 * (c4 + 1)],
                    xb[128 * t:128 * (t + 1), 1024 * c4:1024 * (c4 + 1)])
            nc.sync.dma_start(nwt[t][:, 0:1], nw[128 * t:128 * (t + 1)])
            nc.sync.dma_start(nbt[t][:, 0:1], nb[128 * t:128 * (t + 1)])
            nc.sync.dma_start(pb[t][:, 0:1], projb[128 * t:128 * (t + 1)])
        for i in range(8):
            nc.sync.dma_start(qb[i][:, 0:1], qkvb[128 * i:128 * (i + 1)])
        # v bias broadcast to 128 partitions (stride-0 partition read)
        nc.gpsimd.dma_start(
            out=vb_bc[:],
            in_=bass.AP(tensor=qkvb.tensor, offset=1024, ap=[[0, 128], [1, C]]))
        nc.vector.memset(ones_row[0:1, :], 1.0)

        # weights: natural-layout contiguous DMA, cast to bf16, transpose
        # 128x128 blocks on the PE (identity trick) into wTq/wTp.
        ident = left.tile([128, 128], bf16, name="ident", tag="ident")
        nc.sync.dma_start(ident[:], io["cid"][:, :])
        ind = left.tile([128, 2], f32, name="ind", tag="ind")
        nc.sync.dma_start(ind[:], io["cind"][:, :])
        indT = left.tile([2, 128], f32, name="indT", tag="indT")
        nc.sync.dma_start(indT[0:2, :], io["cindT"][:, :])
        for i in range(12):   # qkv_w row-tiles
            wstg = wstg_pool.tile([128, C], f32, name="wstg", tag="wstg")
            nc.sync.dma_start(wstg[:], qkvw[128 * i:128 * (i + 1), :])
            wbf = wstg_pool.tile([128, C], bf16, name="wbf", tag="wbf")
            nc.vector.tensor_copy(wbf[:], wstg[:])
            for j in range(T):
                tp = psum_sp.tile([128, 1024], bf16, name="tp", tag="sp")
                nc.tensor.transpose(tp[:, 0:128], wbf[:, 128 * j:128 * (j + 1)], ident[:])
                nc.vector.tensor_copy(
                    wTq8[:, 1536 * j + 128 * i:1536 * j + 128 * (i + 1)], tp[:, 0:128])
        for i in range(4):    # proj_w row-tiles
            wstg = wstg_pool.tile([128, C], f32, name="wstg", tag="wstg")
            nc.sync.dma_start(wstg[:], projw[128 * i:128 * (i + 1), :])
            wbf = wstg_pool.tile([128, C], bf16, name="wbf", tag="wbf")
            nc.vector.tensor_copy(wbf[:], wstg[:])
            for j in range(T):
                tp = psum_sp.tile([128, 1024], bf16, name="tp", tag="sp")
                nc.tensor.transpose(tp[:, 0:128], wbf[:, 128 * j:128 * (j + 1)], ident[:])
                nc.vector.tensor_copy(
                    wTp8[:, C * j + 128 * i:C * j + 128 * (i + 1)], tp[:, 0:128])

        # ---------------- phase 1: group stats + normalize ----------------
        for t in range(T):
            nc.vector.tensor_reduce(
                out=stat[t][:, 0:1], in_=xf[t][:], axis=mybir.AxisListType.X, op=Alu.add)
            sq_scr = scr_pool.tile([128, N], bf16, name="sq_scr", tag="sq_scr")
            nc.scalar.activation(
                sq_scr[:], xf[t][:], Act.Square, accum_out=stat[t][:, 1:2])
            # group-reduce over partitions via indicator matmuls
            gg_ps = psum_pv.tile([128, 512], f32, name="gg_ps", tag="pv")
            nc.tensor.matmul(gg_ps[0:2, 0:2], ind[:, 0:2], stat[t][:, 0:2],
                             start=True, stop=True)
            gg_sb = left.tile([2, 2], f32, name=f"gg_sb{t}", tag=f"gg_sb{t}")
            nc.vector.tensor_copy(gg_sb[0:2, :], gg_ps[0:2, 0:2])
            gb_ps = psum_pv.tile([128, 512], f32, name="gb_ps", tag="pv")
            nc.tensor.matmul(gb_ps[:, 0:2], indT[0:2, :], gg_sb[0:2, :],
                             start=True, stop=True)
            nc.vector.tensor_copy(gstat[t][:, 0:2], gb_ps[:, 0:2])
            # mean/var/rstd -> per-channel affine a,b
            mean_t = left.tile([128, 1], f32, name=f"mean{t}", tag=f"mean{t}")
            e2_t = left.tile([128, 1], f32, name=f"e2{t}", tag=f"e2{t}")
            var_t = left.tile([128, 1], f32, name=f"var{t}", tag=f"var{t}")
            std_t = left.tile([128, 1], f32, name=f"std{t}", tag=f"std{t}")
            a_t = left.tile([128, 1], f32, name=f"a{t}", tag=f"a{t}")
            b_t = left.tile([128, 1], f32, name=f"b{t}", tag=f"b{t}")
            inv = 1.0 / GELEM
            nc.vector.tensor_scalar(mean_t[:], gstat[t][:, 0:1], inv, None, Alu.mult)
            nc.vector.tensor_scalar(e2_t[:], gstat[t][:, 1:2], inv, None, Alu.mult)
            nc.vector.scalar_tensor_tensor(
                var_t[:], mean_t[:], -1.0, mean_t[:], Alu.mult, Alu.mult)
            nc.vector.scalar_tensor_tensor(
                var_t[:], e2_t[:], EPS, var_t[:], Alu.add, Alu.add)
            nc.scalar.activation(std_t[:], var_t[:], Act.Sqrt)
            nc.vector.reciprocal(a_t[:], std_t[:])
            nc.vector.tensor_tensor(a_t[:], a_t[:], nwt[t][:], Alu.mult)
            nc.vector.tensor_tensor(b_t[:], mean_t[:], a_t[:], Alu.mult)
            nc.vector.tensor_tensor(b_t[:], nbt[t][:], b_t[:], Alu.subtract)
            # normalize + cast to fp8 (alternate engines to halve the chain)
            if t % 2 == 0:
                nc.scalar.activation(
                    xn8[:, N * t:N * (t + 1)], xf[t][:], Act.Identity,
                    bias=b_t[:, 0:1], scale=a_t[:, 0:1])
            else:
                nc.vector.tensor_scalar(
                    xn8[:, N * t:N * (t + 1)], xf[t][:],
                    a_t[:, 0:1], b_t[:, 0:1], Alu.mult, Alu.add)
            warm(N_WARM2)
        warm(N_WARM3)

        right_ctx.close()

        # ---------------- mid pools (reuse xf space) ----------------
        mid = ctx.enter_context(tc.tile_pool(name="mid", bufs=1))
        ps8_pool = ctx.enter_context(tc.tile_pool(name="ps8_pool", bufs=6))
        rec_pool = ctx.enter_context(tc.tile_pool(name="rec_pool", bufs=2))
        yo_pool = ctx.enter_context(tc.tile_pool(name="yo_pool", bufs=2))

        # fp8e5 v^T for DoubleRow PV: per m-pair m2 and head h, 160 cols =
        # two 80-wide blocks (64 v + ones col @64 + 15 zero pad), one per
        # m-tile of the pair
        vT8 = mid.tile([128, (MT // 2) * 1280], f8, name="vT8", tag="vT8")
        attn8 = mid.tile([128, T * NQ], f8, name="attn8", tag="attn8")
        xres = [mid.tile([128, NQ], f32, name=f"xres{t}", tag=f"xres{t}") for t in range(T)]
        for t in range(T):
            nc.sync.dma_start(xres[t][:], xb[128 * t:128 * (t + 1), 0:NQ])

        # ones + zero-pad columns of the augmented v^T (denominator trick)
        v80 = vT8[:].rearrange("p (n x) -> p n x", x=80)
        nc.vector.memset(v80[:, :, 64:65], 1.0)
        nc.vector.memset(v80[:, :, 65:80], 0.0)

        # ---------------- phase 3: projections (fp8 DoubleRow) ----------
        DR = mybir.MatmulPerfMode.DoubleRow

        def wq_pair(pr, lo, hi):
            return wTq8[:, 3072 * pr:3072 * (pr + 1)].rearrange(
                "p (j x) -> p j x", j=2)[:, :, lo:hi]

        def xn_pair(pr, lo, hi):
            return xn8[:, 2 * N * pr:2 * N * (pr + 1)].rearrange(
                "p (j x) -> p j x", j=2)[:, :, lo:hi]

        # q: out rows 0..511 of qkv, only NQ query columns (2 windows per tile)
        for i in range(T):
            qp = psum_sp.tile([128, 1024], f32, name="qp", tag="sp")
            for w in range(W):
                for pr in range(2):
                    nc.tensor.matmul(
                        qp[:, 512 * w:512 * w + 512],
                        wq_pair(pr, 128 * i, 128 * i + 128),
                        xn_pair(pr, 512 * w, 512 * w + 512),
                        start=(pr == 0), stop=(pr == 1), perf_mode=DR)
            if i % 2 == 0:
                nc.scalar.add(qsb[i][:], qp[:], qb[i][:, 0:1])
            else:
                nc.vector.tensor_scalar(
                    qsb[i][:], qp[:], qb[i][:, 0:1], None, Alu.add)
        # k: out rows 512..1023, all N columns (2 windows per psum tile)
        for i in range(T):
            for w2 in range(4):
                kp = psum_sp.tile([128, 1024], f32, name="kp", tag="sp")
                for w in (2 * w2, 2 * w2 + 1):
                    for pr in range(2):
                        nc.tensor.matmul(
                            kp[:, 512 * (w % 2):512 * (w % 2) + 512],
                            wq_pair(pr, 512 + 128 * i, 512 + 128 * i + 128),
                            xn_pair(pr, 512 * w, 512 * w + 512),
                            start=(pr == 0), stop=(pr == 1), perf_mode=DR)
                if w2 % 2 == 0:
                    nc.scalar.add(
                        ksb[i][:, 1024 * w2:1024 * (w2 + 1)], kp[:],
                        qb[4 + i][:, 0:1])
                else:
                    nc.vector.tensor_scalar(
                        ksb[i][:, 1024 * w2:1024 * (w2 + 1)], kp[:],
                        qb[4 + i][:, 0:1], None, Alu.add)
        # vT: (m, 512) per m-tile, two m-tiles per psum tile, strided into
        # the 80-column augmented fp8 layout
        for m2 in range(MT // 2):
            vp = psum_sp.tile([128, 1024], f32, name="vp", tag="sp")
            for mh in range(2):
                mt = 2 * m2 + mh
                for pr in range(2):
                    nc.tensor.matmul(
                        vp[:, 512 * mh:512 * mh + 512],
                        xn_pair(pr, 128 * mt, 128 * mt + 128),
                        wq_pair(pr, 1024, 1536),
                        start=(pr == 0), stop=(pr == 1), perf_mode=DR)
            vbv = vb_bc[:].rearrange("p (h x) -> p h x", x=64)
            vdst = vT8[:, 1280 * m2:1280 * (m2 + 1)].rearrange(
                "p (h j x) -> p h j x", h=8, j=2, x=80)
            for mh in range(2):
                dst = vdst[:, :, mh, 0:64]
                src = vp[:, 512 * mh:512 * mh + 512].rearrange(
                    "p (h x) -> p h x", x=64)
                nc.vector.tensor_tensor(dst, src, vbv, Alu.add)

        # ---------------- phase 4+5: attention, deferred tails, proj ----
        # tails are emitted in stages inside the NEXT unit's m-loop so the
        # PE stream never waits on the reciprocal chain
        def tail_stage1(tw, thp, tpv0, tpv1):
            # denominators stacked on partition 0 + one reciprocal
            dnq = rec_pool.tile([1, 1024], f32, name="dnq", tag="dnq")
            nc.scalar.copy(dnq[0:1, 0:512], tpv0[64:65, :])
            nc.scalar.copy(dnq[0:1, 512:1024], tpv1[64:65, :])
            rcq = rec_pool.tile([1, 1024], f32, name="rcq", tag="rcq")
            rscr = rec_pool.tile([1, 1024], f32, name="rscr", tag="rscr")
            nc.vector.reciprocal_approx_accurate(
                rcq[0:1, :], dnq[0:1, :], rscr[0:1, :])
            return rcq

        def tail_stage2(rcq):
            # partition-broadcast of the reciprocals via two PE matmuls
            # (emitted mid-next-unit, so the reciprocal chain is already done)
            bc = psum_sp.tile([128, 1024], f32, name="bc", tag="sp")
            nc.tensor.matmul(
                bc[0:64, 0:512], ones_row[0:1, 0:D],
                rcq[0:1, 0:512], start=True, stop=True)
            nc.tensor.matmul(
                bc[64:128, 0:512], ones_row[0:1, 0:D],
                rcq[0:1, 512:1024], start=True, stop=True)
            bcs0 = rec_pool.tile([64, 512], f32, name="bcs0", tag="bcs0")
            bcs1 = rec_pool.tile([64, 512], f32, name="bcs1", tag="bcs1")
            nc.scalar.copy(bcs0[0:64, :], bc[0:64, 0:512])
            nc.scalar.copy(bcs1[0:64, :], bc[64:128, 0:512])
            return bcs0, bcs1

        def tail_stage3(tw, thp, tpv0, tpv1, bcs0, bcs1):
            nc.vector.tensor_tensor(
                attn8[0:64, NQ * thp + 512 * tw:NQ * thp + 512 * tw + 512],
                tpv0[0:64, :], bcs0[0:64, :], Alu.mult)
            nc.vector.tensor_tensor(
                attn8[64:128, NQ * thp + 512 * tw:NQ * thp + 512 * tw + 512],
                tpv1[0:64, :], bcs1[0:64, :], Alu.mult)

        def emit_proj(pw):
            # proj + residual for window pw (fp8 DoubleRow over attn8)
            for i in range(T):
                py = psum_sp.tile([128, 1024], f32, name="py", tag="sp")
                for pr in range(2):
                    rhs = attn8[:, 2 * NQ * pr:2 * NQ * (pr + 1)].rearrange(
                        "p (j x) -> p j x", j=2)[:, :, 512 * pw:512 * pw + 512]
                    lhs = wTp8[:, 1024 * pr:1024 * (pr + 1)].rearrange(
                        "p (j x) -> p j x", j=2)[:, :, 128 * i:128 * i + 128]
                    nc.tensor.matmul(
                        py[:, 0:512], lhs, rhs,
                        start=(pr == 0), stop=(pr == 1), perf_mode=DR)
                yo = yo_pool.tile([128, 512], f32, name="yo", tag="yo")
                nc.vector.scalar_tensor_tensor(
                    yo[:], py[:, 0:512], pb[i][:, 0:1],
                    xres[i][:, 512 * pw:512 * pw + 512], Alu.add, Alu.add)
                nc.sync.dma_start(
                    y[128 * i:128 * i + 128, 512 * pw:512 * pw + 512], yo[:])

        pend = []             # (m2, ps8, hp, pv0, pv1) across units

        def flush_pv():
            m2, ps8t, fhp, fpv0, fpv1 = pend.pop(0)
            st, sto = (m2 == 0), (m2 == MT // 2 - 1)
            vb = 1280 * m2
            for h, pv in ((0, fpv0), (1, fpv1)):
                lhs = vT8[:, vb + 160 * (2 * fhp + h):
                          vb + 160 * (2 * fhp + h) + 160].rearrange(
                    "p (j x) -> p j x", j=2)
                rhs = ps8t[:, 1024 * h:1024 * h + 1024].rearrange(
                    "p (j x) -> p j x", j=2)
                nc.tensor.matmul(
                    pv[0:80, :], lhs, rhs, start=st, stop=sto,
                    perf_mode=DR)

        deferred = None       # (w, hp, pv0, pv1) of the previous unit
        proj_due = None       # window whose proj should be emitted next
        for w in range(W):
            for hp in range(4):
                # drain the previous unit's leftover PV flushes first: the
                # engines finish its last exps while these stream
                while pend:
                    flush_pv()
                h0, h1 = 2 * hp, 2 * hp + 1
                pv0 = psum_pv.tile([128, 512], f32, name="pv0", tag="pv")
                pv1 = psum_pv.tile([128, 512], f32, name="pv1", tag="pv")

                ps8 = None
                for m in range(MT):
                    sp = psum_sp.tile([128, 1024], f32, name="sp", tag="sp")
                    # S pair: head-even on PE rows 0:63, head-odd on rows
                    # 64:127 (concurrent row groups, shared LDW window)
                    nc.tensor.matmul(
                        sp[:, 0:512],
                        ksb[hp][0:64, 128 * m:128 * m + 128],
                        qsb[hp][0:64, 512 * w:512 * w + 512],
                        start=True, stop=True)
                    nc.tensor.matmul(
                        sp[:, 512:1024],
                        ksb[hp][64:128, 128 * m:128 * m + 128],
                        qsb[hp][64:128, 512 * w:512 * w + 512],
                        start=True, stop=True)
                    m2, mh = divmod(m, 2)
                    if mh == 0:
                        ps8 = ps8_pool.tile([128, 2048], f8, name="ps8", tag="ps8")
                    # exp of this m-tile for both heads into the fp8 pair
                    # tile: cols [1024h + 512mh : +512] = (head h, m-half mh)
                    outv = ps8[:].rearrange(
                        "p (H x) -> p H x", H=2)[:, :, 512 * mh:512 * mh + 512]
                    inv = sp[:].rearrange("p (h x) -> p h x", h=2)
                    # ACT handles even m-tiles plus one odd (engine balance);
                    # DVE does the rest via the int8 Schraudolph bit trick
                    if mh == 0 or m == 13:
                        nc.scalar.activation(outv, inv, Act.Exp, scale=0.125)
                    else:
                        nc.vector.tensor_scalar(
                            outv.bitcast(i8), inv, SA5, SB5, Alu.mult, Alu.add)
                    if mh == 1:
                        pend.append((m2, ps8, hp, pv0, pv1))
                        if len(pend) >= 5:
                            flush_pv()
                    # previous unit's tail stages + any due proj, overlapped
                    # with this unit's m-loop (keeps the PE stream dense)
                    if deferred is not None:
                        if m == 2:
                            with tc.high_priority():
                                t_rcq = tail_stage1(*deferred)
                        elif m == 5:
                            with tc.high_priority():
                                t_bcs = tail_stage2(t_rcq)
                        elif m == 8:
                            with tc.high_priority():
                                tail_stage3(*deferred, *t_bcs)
                            deferred = None
                    elif m == 10 and proj_due is not None:
                        with tc.high_priority():
                            emit_proj(proj_due)
                        proj_due = None
                deferred = (w, hp, pv0, pv1)
                if hp == 3:
                    proj_due = w
        while pend:
            flush_pv()
        t_rcq = tail_stage1(*deferred)
        t_bcs = tail_stage2(t_rcq)
        tail_stage3(*deferred, *t_bcs)
        emit_proj(proj_due)


def _build():
    import concourse.tile as tile
    from concourse import bacc, mybir

    nc = bacc.Bacc("TRN2", target_bir_lowering=False, debug=False)
    f32 = mybir.dt.float32
    io = {
        "xb": nc.dram_tensor("xb", [C, N], f32, kind="ExternalInput").ap(),
        "qkvw": nc.dram_tensor("qkvw", [3 * C, C], f32, kind="ExternalInput").ap(),
        "qkvb": nc.dram_tensor("qkvb", [3 * C], f32, kind="ExternalInput").ap(),
        "projw": nc.dram_tensor("projw", [C, C], f32, kind="ExternalInput").ap(),
        "projb": nc.dram_tensor("projb", [C], f32, kind="ExternalInput").ap(),
        "nw": nc.dram_tensor("nw", [C], f32, kind="ExternalInput").ap(),
        "nb": nc.dram_tensor("nb", [C], f32, kind="ExternalInput").ap(),
        "cid": nc.dram_tensor("cid", [128, 128], mybir.dt.bfloat16,
                              kind="ExternalInput").ap(),
        "cind": nc.dram_tensor("cind", [128, 2], f32, kind="ExternalInput").ap(),
        "cindT": nc.dram_tensor("cindT", [2, 128], f32, kind="ExternalInput").ap(),
        "y": nc.dram_tensor("y", [C, NQ], f32, kind="ExternalOutput").ap(),
    }
    with tile.TileContext(nc) as tc:
        _emit(tc, io)
    nc.compile()
    return nc


def get_compiled():
    global _COMPILED
    if _COMPILED is None:
        _COMPILED = _build()
    return _COMPILED


def make_in_maps(x, norm_w, norm_b, qkv_w, qkv_b, proj_w, proj_b):
    import ml_dtypes

    xf = np.ascontiguousarray(np.asarray(x, np.float32)).reshape(2, C, N)
    ind = np.zeros((128, 2), np.float32)
    ind[0:64, 0] = 1.0
    ind[64:128, 1] = 1.0
    shared = {
        "cid": np.eye(128, dtype=ml_dtypes.bfloat16),
        "cind": ind,
        "cindT": np.ascontiguousarray(ind.T),
        "qkvw": np.ascontiguousarray(np.asarray(qkv_w, np.float32)),
        "qkvb": np.ascontiguousarray(np.asarray(qkv_b, np.float32)),
        "projw": np.ascontiguousarray(np.asarray(proj_w, np.float32)),
        "projb": np.ascontiguousarray(np.asarray(proj_b, np.float32)),
        "nw": np.ascontiguousarray(np.asarray(norm_w, np.float32)),
        "nb": np.ascontiguousarray(np.asarray(norm_b, np.float32)),
    }
    in_maps = []
    for core in range(8):
        bi, qs = core // 4, core % 4
        # rotate so this core's queries are always columns [0:NQ)
        xroll = np.concatenate(
            [xf[bi][:, qs * NQ:], xf[bi][:, :qs * NQ]], axis=1)
        m = dict(shared)
        m["xb"] = np.ascontiguousarray(xroll)
        in_maps.append(m)
    return in_maps


def assemble(results, x):
    y = np.zeros((2, C, N), np.float32)
    for core in range(8):
        bi, qs = core // 4, core % 4
        y[bi][:, qs * NQ:(qs + 1) * NQ] = results[core]["y"]
    return y.reshape(x.shape)


def kernel(x, norm_w, norm_b, qkv_w, qkv_b, proj_w, proj_b, **_ignored):
    from concourse import bass_utils

    nc = get_compiled()
    in_maps = make_in_maps(x, norm_w, norm_b, qkv_w, qkv_b, proj_w, proj_b)
    res = bass_utils.run_bass_kernel_spmd(nc, in_maps, core_ids=list(range(8)))
    return assemble(res.results, np.asarray(x))


# revision 35
# speedup vs baseline: 2.1194x; 1.0066x over previous
"""Trainium2 Bass kernel for nn_AttentionBlock (GroupNorm + MHA + proj + residual).

Full inputs in, full output out. Sharding: 8 cores = 2 batches x 4 query-slices.
Each core: GroupNorm over its batch image, q projection for its 1024 queries,
k/v projections over all 4096 keys, per-head attention (S^T = k^T q, softmax
along the PSUM partition axis via an appended ones-column in the PV matmul),
output projection and residual for its query slice.

v2 performance structure:
 - Warmup matmuls trip the PE HAM clock gate early (else everything runs at
   1.2 GHz instead of 2.4 GHz).
 - Heads are processed in pairs: head-even uses PE rows 0:63, head-odd rows
   64:127 (tile_position row groups) so the two S matmuls per key-tile run
   concurrently in the PE array and LDWEIGHTS overlaps streaming.
 - softmax exp alternates between ScalarE (real exp, even key-tiles) and
   VectorE (Schraudolph bit-trick exp -> bf16 bit pattern via an int16
   round, odd key-tiles), halving the exp bottleneck.
 - PV matmuls lag S by 2 key-tiles (3-deep PSUM ring) so the exp latency is
   off the PE critical path.

All matmuls run in bf16 with fp32 PSUM accumulation; softmax logits stay fp32.
"""
import numpy as np

C = 512          # channels
N = 4096         # pixels (64*64)
NQ = 1024        # queries per core
H = 8            # heads
D = 64           # head dim
T = 4            # 128-channel chunks
W = NQ // 512    # query windows of 512
MT = N // 128    # key m-tiles of 128
NGROUPS = 8
EPS = 1e-5
GELEM = (C // NGROUPS) * N   # elements per norm group

# Schraudolph fast-exp (fp8e5m2 bits via int8 round-to-nearest):
#   bits = round(raw * SA5 + SB5);  bitcast(int8->fp8e5) ~ exp(0.125*raw)
# SA5 = 0.125 * log2(e) * 4 ; SB5 = 15*4 - 0.21875 (max rel err 11.7%,
# same order as the direct e5m2 quantization of a true exp)
SA5 = 0.7213475204444817
SB5 = 59.78125

N_WARM1 = 60     # warmup MMs before transposes (covers input DMA)
N_WARM2 = 20     # warmup MMs per groupnorm chunk
N_WARM3 = 80     # warmup MMs after groupnorm emission

_COMPILED = None


def _emit(tc, io):
    import concourse.bass as bass
    from concourse import mybir
    from contextlib import ExitStack

    nc = tc.nc
    f32 = mybir.dt.float32
    bf16 = mybir.dt.bfloat16
    i8 = mybir.dt.int8
    f8 = mybir.dt.float8e5
    Alu = mybir.AluOpType
    Act = mybir.ActivationFunctionType

    xb, qkvw, qkvb, projw, projb, nw, nb, y = (
        io["xb"], io["qkvw"], io["qkvb"], io["projw"], io["projb"],
        io["nw"], io["nb"], io["y"])
    bscr = io["bscr"]

    ctx = ExitStack()
    with ctx:
        # ---------------- pools ----------------
        # PSUM: sp ring 3x(128,1024) [6 banks] + pv 2x(128,512) [2 banks]
        left = ctx.enter_context(tc.tile_pool(name="left", bufs=1))
        psum_sp = ctx.enter_context(tc.tile_pool(name="psum_sp", bufs=3, space="PSUM"))
        psum_pv = ctx.enter_context(tc.tile_pool(name="psum_pv", bufs=2, space="PSUM"))

        right_ctx = ExitStack()
        xf_pool = right_ctx.enter_context(
            tc.tile_pool(name="xf_pool", bufs=1, side="right"))
        wstg_pool = right_ctx.enter_context(
            tc.tile_pool(name="wstg_pool", bufs=4, side="right"))
        scr_pool = right_ctx.enter_context(
            tc.tile_pool(name="scr_pool", bufs=2, side="right"))

        # ---------------- persistent tiles ----------------
        # fp8 activations/weights for DoubleRow GEMMs, chunk-major layouts
        xn8 = left.tile([128, T * N], f8, name="xn8", tag="xn8")
        ksb = [left.tile([128, N], bf16, name=f"ksb{t}", tag=f"ksb{t}") for t in range(T)]
        qsb = [left.tile([128, NQ], bf16, name=f"qsb{t}", tag=f"qsb{t}") for t in range(T)]
        wTq8 = left.tile([128, T * 1536], f8, name="wTq8", tag="wTq8")
        wTp8 = left.tile([128, T * C], f8, name="wTp8", tag="wTp8")
        vb_bc = left.tile([128, C], f32, name="vb_bc", tag="vb_bc")
        ones_row = left.tile([1, D], f32, name="ones_row", tag="ones_row")
        qb = [left.tile([128, 1], f32, name=f"qb{i}", tag=f"qb{i}") for i in range(8)]
        pb = [left.tile([128, 1], f32, name=f"pb{i}", tag=f"pb{i}") for i in range(T)]
        nwt = [left.tile([128, 1], f32, name=f"nwt{t}", tag=f"nwt{t}") for t in range(T)]
        nbt = [left.tile([128, 1], f32, name=f"nbt{t}", tag=f"nbt{t}") for t in range(T)]
        stat = [left.tile([128, 2], f32, name=f"stat{t}", tag=f"stat{t}") for t in range(T)]
        gstat = [left.tile([128, 2], f32, name=f"gstat{t}", tag=f"gstat{t}") for t in range(T)]
        wu = left.tile([128, 512], bf16, name="wu", tag="wu")

        # ---------------- warmup: trip the HAM clock gate ----------------
        nc.vector.memset(wu[:], 0.125)
        wu_ps = psum_pv.tile([128, 512], f32, name="wu_ps", tag="pv")
        # preload the exp activation table while idle
        wu_exp = scr_pool.tile([1, 8], f32, name="wu_exp", tag="wu_exp")
        nc.scalar.activation(wu_exp[0:1, :], wu[0:1, 0:8], Act.Exp)

        def warm(n):
            for _ in range(n):
                nc.tensor.matmul(wu_ps[:], wu[:, 0:128], wu[:],
                                 start=True, stop=True)
        warm(N_WARM1)

        # ---------------- input DMAs ----------------
        xf = [xf_pool.tile([128, N], f32, name=f"xf{t}", tag=f"xf{t}") for t in range(T)]
        for t in range(T):
            for c4 in range(4):   # split across DMA queues
                nc.sync.dma_start(
                    xf[t][:, 1024 * c4:1024 * (c4 + 1)],
                    xb[128 * t:128 * (t + 1), 1024 * c4:1024 * (c4 + 1)])
            nc.sync.dma_start(nwt[t][:, 0:1], nw[128 * t:128 * (t + 1)])
            nc.sync.dma_start(nbt[t][:, 0:1], nb[128 * t:128 * (t + 1)])
            nc.sync.dma_start(pb[t][:, 0:1], projb[128 * t:128 * (t + 1)])
        for i in range(8):
            nc.sync.dma_start(qb[i][:, 0:1], qkvb[128 * i:128 * (i + 1)])
        # v bias broadcast to 128 partitions (stride-0 partition read)
        nc.gpsimd.dma_start(
            out=vb_bc[:],
            in_=bass.AP(tensor=qkvb.tensor, offset=1024, ap=[[0, 128], [1, C]]))
        nc.vector.memset(ones_row[0:1, :], 1.0)

        # weights: natural-layout contiguous DMA, cast to bf16, transpose
        # 128x128 blocks on the PE (identity trick) into wTq/wTp.
        ident = left.tile([128, 128], bf16, name="ident", tag="ident")
        nc.sync.dma_start(ident[:], io["cid"][:, :])
        ind = left.tile([128, 2], f32, name="ind", tag="ind")
        nc.sync.dma_start(ind[:], io["cind"][:, :])
        indT = left.tile([2, 128], f32, name="indT", tag="indT")
        nc.sync.dma_start(indT[0:2, :], io["cindT"][:, :])
        for i in range(12):   # qkv_w row-tiles
            wstg = wstg_pool.tile([128, C], f32, name="wstg", tag="wstg")
            nc.sync.dma_start(wstg[:], qkvw[128 * i:128 * (i + 1), :])
            wbf = wstg_pool.tile([128, C], bf16, name="wbf", tag="wbf")
            nc.vector.tensor_copy(wbf[:], wstg[:])
            for j in range(T):
                tp = psum_sp.tile([128, 1024], bf16, name="tp", tag="sp")
                nc.tensor.transpose(tp[:, 0:128], wbf[:, 128 * j:128 * (j + 1)], ident[:])
                nc.vector.tensor_copy(
                    wTq8[:, 1536 * j + 128 * i:1536 * j + 128 * (i + 1)], tp[:, 0:128])
        for i in range(4):    # proj_w row-tiles
            wstg = wstg_pool.tile([128, C], f32, name="wstg", tag="wstg")
            nc.sync.dma_start(wstg[:], projw[128 * i:128 * (i + 1), :])
            wbf = wstg_pool.tile([128, C], bf16, name="wbf", tag="wbf")
            nc.vector.tensor_copy(wbf[:], wstg[:])
            for j in range(T):
                tp = psum_sp.tile([128, 1024], bf16, name="tp", tag="sp")
                nc.tensor.transpose(tp[:, 0:128], wbf[:, 128 * j:128 * (j + 1)], ident[:])
                nc.vector.tensor_copy(
                    wTp8[:, C * j + 128 * i:C * j + 128 * (i + 1)], tp[:, 0:128])

        # ---------------- phase 1: group stats + normalize ----------------
        for t in range(T):
            nc.vector.tensor_reduce(
                out=stat[t][:, 0:1], in_=xf[t][:], axis=mybir.AxisListType.X, op=Alu.add)
            sq_scr = scr_pool.tile([128, N], bf16, name="sq_scr", tag="sq_scr")
            nc.scalar.activation(
                sq_scr[:], xf[t][:], Act.Square, accum_out=stat[t][:, 1:2])
            # group-reduce over partitions via indicator matmuls
            gg_ps = psum_pv.tile([128, 512], f32, name="gg_ps", tag="pv")
            nc.tensor.matmul(gg_ps[0:2, 0:2], ind[:, 0:2], stat[t][:, 0:2],
                             start=True, stop=True)
            gg_sb = left.tile([2, 2], f32, name=f"gg_sb{t}", tag=f"gg_sb{t}")
            nc.vector.tensor_copy(gg_sb[0:2, :], gg_ps[0:2, 0:2])
            gb_ps = psum_pv.tile([128, 512], f32, name="gb_ps", tag="pv")
            nc.tensor.matmul(gb_ps[:, 0:2], indT[0:2, :], gg_sb[0:2, :],
                             start=True, stop=True)
            nc.vector.tensor_copy(gstat[t][:, 0:2], gb_ps[:, 0:2])
            # mean/var/rstd -> per-channel affine a,b
            mean_t = left.tile([128, 1], f32, name=f"mean{t}", tag=f"mean{t}")
            e2_t = left.tile([128, 1], f32, name=f"e2{t}", tag=f"e2{t}")
            var_t = left.tile([128, 1], f32, name=f"var{t}", tag=f"var{t}")
            std_t = left.tile([128, 1], f32, name=f"std{t}", tag=f"std{t}")
            a_t = left.tile([128, 1], f32, name=f"a{t}", tag=f"a{t}")
            b_t = left.tile([128, 1], f32, name=f"b{t}", tag=f"b{t}")
            inv = 1.0 / GELEM
            nc.vector.tensor_scalar(mean_t[:], gstat[t][:, 0:1], inv, None, Alu.mult)
            nc.vector.tensor_scalar(e2_t[:], gstat[t][:, 1:2], inv, None, Alu.mult)
            nc.vector.scalar_tensor_tensor(
                var_t[:], mean_t[:], -1.0, mean_t[:], Alu.mult, Alu.mult)
            nc.vector.scalar_tensor_tensor(
                var_t[:], e2_t[:], EPS, var_t[:], Alu.add, Alu.add)
            nc.scalar.activation(std_t[:], var_t[:], Act.Sqrt)
            nc.vector.reciprocal(a_t[:], std_t[:])
            nc.vector.tensor_tensor(a_t[:], a_t[:], nwt[t][:], Alu.mult)
            nc.vector.tensor_tensor(b_t[:], mean_t[:], a_t[:], Alu.mult)
            nc.vector.tensor_tensor(b_t[:], nbt[t][:], b_t[:], Alu.subtract)
            # normalize + cast to fp8 (alternate engines to halve the chain)
            if t % 2 == 0:
                nc.scalar.activation(
                    xn8[:, N * t:N * (t + 1)], xf[t][:], Act.Identity,
                    bias=b_t[:, 0:1], scale=a_t[:, 0:1])
            else:
                nc.vector.tensor_scalar(
                    xn8[:, N * t:N * (t + 1)], xf[t][:],
                    a_t[:, 0:1], b_t[:, 0:1], Alu.mult, Alu.add)
            warm(N_WARM2)
        warm(N_WARM3)

        right_ctx.close()

        # ---------------- mid pools (reuse xf space) ----------------
        mid = ctx.enter_context(tc.tile_pool(name="mid", bufs=1))
        ps8_pool = ctx.enter_context(tc.tile_pool(name="ps8_pool", bufs=7))
        rec_pool = ctx.enter_context(tc.tile_pool(name="rec_pool", bufs=2))
        yo_pool = ctx.enter_context(tc.tile_pool(name="yo_pool", bufs=2))

        # fp8e5 v^T for DoubleRow PV: per m-pair m2 and head h, 160 cols =
        # two 80-wide blocks (64 v + ones col @64 + 15 zero pad), one per
        # m-tile of the pair
        vT8 = mid.tile([128, (MT // 2) * 1280], f8, name="vT8", tag="vT8")
        attn8 = mid.tile([128, T * NQ], f8, name="attn8", tag="attn8")
        xres = [mid.tile([128, NQ], f32, name=f"xres{t}", tag=f"xres{t}") for t in range(T)]
        for t in range(T):
            nc.sync.dma_start(xres[t][:], xb[128 * t:128 * (t + 1), 0:NQ])

        # ones + zero-pad columns of the augmented v^T (denominator trick)
        v80 = vT8[:].rearrange("p (n x) -> p n x", x=80)
        nc.vector.memset(v80[:, :, 64:65], 1.0)
        nc.vector.memset(v80[:, :, 65:80], 0.0)

        # ---------------- phase 3: projections (fp8 DoubleRow) ----------
        DR = mybir.MatmulPerfMode.DoubleRow

        def wq_pair(pr, lo, hi):
            return wTq8[:, 3072 * pr:3072 * (pr + 1)].rearrange(
                "p (j x) -> p j x", j=2)[:, :, lo:hi]

        def xn_pair(pr, lo, hi):
            return xn8[:, 2 * N * pr:2 * N * (pr + 1)].rearrange(
                "p (j x) -> p j x", j=2)[:, :, lo:hi]

        # q: out rows 0..511 of qkv, only NQ query columns (2 windows per tile)
        for i in range(T):
            qp = psum_sp.tile([128, 1024], f32, name="qp", tag="sp")
            for w in range(W):
                for pr in range(2):
                    nc.tensor.matmul(
                        qp[:, 512 * w:512 * w + 512],
                        wq_pair(pr, 128 * i, 128 * i + 128),
                        xn_pair(pr, 512 * w, 512 * w + 512),
                        start=(pr == 0), stop=(pr == 1), perf_mode=DR)
            if i % 2 == 0:
                nc.scalar.add(qsb[i][:], qp[:], qb[i][:, 0:1])
            else:
                nc.vector.tensor_scalar(
                    qsb[i][:], qp[:], qb[i][:, 0:1], None, Alu.add)
        # k: out rows 512..1023, all N columns (2 windows per psum tile)
        for i in range(T):
            for w2 in range(4):
                kp = psum_sp.tile([128, 1024], f32, name="kp", tag="sp")
                for w in (2 * w2, 2 * w2 + 1):
                    for pr in range(2):
                        nc.tensor.matmul(
                            kp[:, 512 * (w % 2):512 * (w % 2) + 512],
                            wq_pair(pr, 512 + 128 * i, 512 + 128 * i + 128),
                            xn_pair(pr, 512 * w, 512 * w + 512),
                            start=(pr == 0), stop=(pr == 1), perf_mode=DR)
                if w2 % 2 == 0:
                    nc.scalar.add(
                        ksb[i][:, 1024 * w2:1024 * (w2 + 1)], kp[:],
                        qb[4 + i][:, 0:1])
                else:
                    nc.vector.tensor_scalar(
                        ksb[i][:, 1024 * w2:1024 * (w2 + 1)], kp[:],
                        qb[4 + i][:, 0:1], None, Alu.add)
        # vT: (m, 512) per m-tile, two m-tiles per psum tile, strided into
        # the 80-column augmented fp8 layout
        for m2 in range(MT // 2):
            vp = psum_sp.tile([128, 1024], f32, name="vp", tag="sp")
            for mh in range(2):
                mt = 2 * m2 + mh
                for pr in range(2):
                    nc.tensor.matmul(
                        vp[:, 512 * mh:512 * mh + 512],
                        xn_pair(pr, 128 * mt, 128 * mt + 128),
                        wq_pair(pr, 1024, 1536),
                        start=(pr == 0), stop=(pr == 1), perf_mode=DR)
            vbv = vb_bc[:].rearrange("p (h x) -> p h x", x=64)
            vdst = vT8[:, 1280 * m2:1280 * (m2 + 1)].rearrange(
                "p (h j x) -> p h j x", h=8, j=2, x=80)
            for mh in range(2):
                dst = vdst[:, :, mh, 0:64]
                src = vp[:, 512 * mh:512 * mh + 512].rearrange(
                    "p (h x) -> p h x", x=64)
                nc.vector.tensor_tensor(dst, src, vbv, Alu.add)

        # ---------------- phase 4+5: attention, deferred tails, proj ----
        # tails are emitted in stages inside the NEXT unit's m-loop so the
        # PE stream never waits on the reciprocal chain
        def tail_stage1(tw, thp, tpv0, tpv1):
            # denominators stacked on partition 0 + one reciprocal
            dnq = rec_pool.tile([1, 1024], f32, name="dnq", tag="dnq")
            nc.scalar.copy(dnq[0:1, 0:512], tpv0[64:65, :])
            nc.scalar.copy(dnq[0:1, 512:1024], tpv1[64:65, :])
            rcq = rec_pool.tile([1, 1024], f32, name="rcq", tag="rcq")
            rscr = rec_pool.tile([1, 1024], f32, name="rscr", tag="rscr")
            nc.vector.reciprocal_approx_accurate(
                rcq[0:1, :], dnq[0:1, :], rscr[0:1, :])
            return rcq

        def tail_stage2(rcq, slot):
            # partition-broadcast of the reciprocals via a DRAM round-trip
            # (stride-0 partition read) -- zero PE involvement; all three
            # DMAs share the in-order sync queue
            nc.sync.dma_start(bscr[slot:slot + 1, :], rcq[0:1, :])
            bcs0 = rec_pool.tile([64, 512], f32, name="bcs0", tag="bcs0")
            bcs1 = rec_pool.tile([64, 512], f32, name="bcs1", tag="bcs1")
            nc.sync.dma_start(
                bcs0[0:64, :],
                bass.AP(tensor=bscr.tensor, offset=1024 * slot,
                        ap=[[0, 64], [1, 512]]))
            nc.sync.dma_start(
                bcs1[0:64, :],
                bass.AP(tensor=bscr.tensor, offset=1024 * slot + 512,
                        ap=[[0, 64], [1, 512]]))
            return bcs0, bcs1

        def tail_stage3(tw, thp, tpv0, tpv1, bcs0, bcs1):
            nc.vector.tensor_tensor(
                attn8[0:64, NQ * thp + 512 * tw:NQ * thp + 512 * tw + 512],
                tpv0[0:64, :], bcs0[0:64, :], Alu.mult)
            nc.vector.tensor_tensor(
                attn8[64:128, NQ * thp + 512 * tw:NQ * thp + 512 * tw + 512],
                tpv1[0:64, :], bcs1[0:64, :], Alu.mult)

        def emit_proj(pw):
            # proj + residual for window pw (fp8 DoubleRow over attn8)
            for i in range(T):
                py = psum_sp.tile([128, 1024], f32, name="py", tag="sp")
                for pr in range(2):
                    rhs = attn8[:, 2 * NQ * pr:2 * NQ * (pr + 1)].rearrange(
                        "p (j x) -> p j x", j=2)[:, :, 512 * pw:512 * pw + 512]
                    lhs = wTp8[:, 1024 * pr:1024 * (pr + 1)].rearrange(
                        "p (j x) -> p j x", j=2)[:, :, 128 * i:128 * i + 128]
                    nc.tensor.matmul(
                        py[:, 0:512], lhs, rhs,
                        start=(pr == 0), stop=(pr == 1), perf_mode=DR)
                yo = yo_pool.tile([128, 512], f32, name="yo", tag="yo")
                nc.vector.scalar_tensor_tensor(
                    yo[:], py[:, 0:512], pb[i][:, 0:1],
                    xres[i][:, 512 * pw:512 * pw + 512], Alu.add, Alu.add)
                nc.sync.dma_start(
                    y[128 * i:128 * i + 128, 512 * pw:512 * pw + 512], yo[:])

        pend = []             # (m2, ps8, hp, pv0, pv1) across units

        def flush_pv():
            m2, ps8t, fhp, fpv0, fpv1 = pend.pop(0)
            st, sto = (m2 == 0), (m2 == MT // 2 - 1)
            vb = 1280 * m2
            for h, pv in ((0, fpv0), (1, fpv1)):
                lhs = vT8[:, vb + 160 * (2 * fhp + h):
                          vb + 160 * (2 * fhp + h) + 160].rearrange(
                    "p (j x) -> p j x", j=2)
                rhs = ps8t[:, 1024 * h:1024 * h + 1024].rearrange(
                    "p (j x) -> p j x", j=2)
                nc.tensor.matmul(
                    pv[0:80, :], lhs, rhs, start=st, stop=sto,
                    perf_mode=DR)

        deferred = None       # (w, hp, pv0, pv1) of the previous unit
        proj_due = None       # window whose proj should be emitted next
        for w in range(W):
            for hp in range(4):
                # drain the previous unit's leftover PV flushes first: the
                # engines finish its last exps while these stream
                while pend:
                    flush_pv()
                h0, h1 = 2 * hp, 2 * hp + 1
                pv0 = psum_pv.tile([128, 512], f32, name="pv0", tag="pv")
                pv1 = psum_pv.tile([128, 512], f32, name="pv1", tag="pv")

                ps8 = None
                for m in range(MT):
                    sp = psum_sp.tile([128, 1024], f32, name="sp", tag="sp")
                    # S pair: head-even on PE rows 0:63, head-odd on rows
                    # 64:127 (concurrent row groups, shared LDW window)
                    nc.tensor.matmul(
                        sp[:, 0:512],
                        ksb[hp][0:64, 128 * m:128 * m + 128],
                        qsb[hp][0:64, 512 * w:512 * w + 512],
                        start=True, stop=True)
                    nc.tensor.matmul(
                        sp[:, 512:1024],
                        ksb[hp][64:128, 128 * m:128 * m + 128],
                        qsb[hp][64:128, 512 * w:512 * w + 512],
                        start=True, stop=True)
                    m2, mh = divmod(m, 2)
                    if mh == 0:
                        ps8 = ps8_pool.tile([128, 2048], f8, name="ps8", tag="ps8")
                    # exp of this m-tile for both heads into the fp8 pair
                    # tile: cols [1024h + 512mh : +512] = (head h, m-half mh)
                    outv = ps8[:].rearrange(
                        "p (H x) -> p H x", H=2)[:, :, 512 * mh:512 * mh + 512]
                    inv = sp[:].rearrange("p (h x) -> p h x", h=2)
                    # ACT handles even m-tiles plus one odd (engine balance);
                    # DVE does the rest via the int8 Schraudolph bit trick
                    if mh == 0 or m == 13:
                        nc.scalar.activation(outv, inv, Act.Exp, scale=0.125)
                    else:
                        nc.vector.tensor_scalar(
                            outv.bitcast(i8), inv, SA5, SB5, Alu.mult, Alu.add)
                    if mh == 1:
                        pend.append((m2, ps8, hp, pv0, pv1))
                        if len(pend) >= 6:
                            flush_pv()
                    # previous unit's tail stages + any due proj, overlapped
                    # with this unit's m-loop (keeps the PE stream dense)
                    if deferred is not None:
                        if m == 2:
                            with tc.high_priority():
                                t_rcq = tail_stage1(*deferred)
                        elif m == 5:
                            with tc.high_priority():
                                t_bcs = tail_stage2(
                                    t_rcq, deferred[0] * 4 + deferred[1])
                        elif m == 8:
                            with tc.high_priority():
                                tail_stage3(*deferred, *t_bcs)
                            deferred = None
                    elif m == 10 and proj_due is not None:
                        with tc.high_priority():
                            emit_proj(proj_due)
                        proj_due = None
                deferred = (w, hp, pv0, pv1)
                if hp == 3:
                    proj_due = w
        while pend:
            flush_pv()
        t_rcq = tail_stage1(*deferred)
        t_bcs = tail_stage2(t_rcq, deferred[0] * 4 + deferred[1])
        tail_stage3(*deferred, *t_bcs)
        emit_proj(proj_due)


def _build():
    import concourse.tile as tile
    from concourse import bacc, mybir

    nc = bacc.Bacc("TRN2", target_bir_lowering=False, debug=False)
    f32 = mybir.dt.float32
    io = {
        "xb": nc.dram_tensor("xb", [C, N], f32, kind="ExternalInput").ap(),
        "qkvw": nc.dram_tensor("qkvw", [3 * C, C], f32, kind="ExternalInput").ap(),
        "qkvb": nc.dram_tensor("qkvb", [3 * C], f32, kind="ExternalInput").ap(),
        "projw": nc.dram_tensor("projw", [C, C], f32, kind="ExternalInput").ap(),
        "projb": nc.dram_tensor("projb", [C], f32, kind="ExternalInput").ap(),
        "nw": nc.dram_tensor("nw", [C], f32, kind="ExternalInput").ap(),
        "nb": nc.dram_tensor("nb", [C], f32, kind="ExternalInput").ap(),
        "cid": nc.dram_tensor("cid", [128, 128], mybir.dt.bfloat16,
                              kind="ExternalInput").ap(),
        "cind": nc.dram_tensor("cind", [128, 2], f32, kind="ExternalInput").ap(),
        "cindT": nc.dram_tensor("cindT", [2, 128], f32, kind="ExternalInput").ap(),
        "y": nc.dram_tensor("y", [C, NQ], f32, kind="ExternalOutput").ap(),
        "bscr": nc.dram_tensor("bscr", [8, 1024], f32).ap(),
    }
    with tile.TileContext(nc) as tc:
        _emit(tc, io)
    nc.compile()
    return nc


def get_compiled():
    global _COMPILED
    if _COMPILED is None:
        _COMPILED = _build()
    return _COMPILED


def make_in_maps(x, norm_w, norm_b, qkv_w, qkv_b, proj_w, proj_b):
    import ml_dtypes

    xf = np.ascontiguousarray(np.asarray(x, np.float32)).reshape(2, C, N)
    ind = np.zeros((128, 2), np.float32)
    ind[0:64, 0] = 1.0
    ind[64:128, 1] = 1.0
    shared = {
        "cid": np.eye(128, dtype=ml_dtypes.bfloat16),
        "cind": ind,
        "cindT": np.ascontiguousarray(ind.T),
        "qkvw": np.ascontiguousarray(np.asarray(qkv_w, np.float32)),
        "qkvb": np.ascontiguousarray(np.asarray(qkv_b, np.float32)),
        "projw": np.ascontiguousarray(np.asarray(proj_w, np.float32)),
        "projb": np.ascontiguousarray(np.asarray(proj_b, np.float32)),
        "nw": np.ascontiguousarray(np.asarray(norm_w, np.float32)),
        "nb": np.ascontiguousarray(np.asarray(norm_b, np.float32)),
    }
    in_maps = []
    for core in range(8):
        bi, qs = core // 4, core % 4
        # rotate so this core's queries are always columns [0:NQ)
        xroll = np.concatenate(
            [xf[bi][:, qs * NQ:], xf[bi][:, :qs * NQ]], axis=1)
        m = dict(shared)
        m["xb"] = np.ascontiguousarray(xroll)
        in_maps.append(m)
    return in_maps


def assemble(results, x):
    y = np.zeros((2, C, N), np.float32)
    for core in range(8):
        bi, qs = core // 4, core % 4
        y[bi][:, qs * NQ:(qs + 1) * NQ] = results[core]["y"]
    return y.reshape(x.shape)


def kernel(x, norm_w, norm_b, qkv_w, qkv_b, proj_w, proj_b, **_ignored):
    from concourse import bass_utils

    nc = get_compiled()
    in_maps = make_in_maps(x, norm_w, norm_b, qkv_w, qkv_b, proj_w, proj_b)
    res = bass_utils.run_bass_kernel_spmd(nc, in_maps, core_ids=list(range(8)))
    return assemble(res.results, np.asarray(x))


# revision 36
# speedup vs baseline: 2.1651x; 1.0216x over previous
"""Trainium2 Bass kernel for nn_AttentionBlock (GroupNorm + MHA + proj + residual).

Full inputs in, full output out. Sharding: 8 cores = 2 batches x 4 query-slices.
Each core: GroupNorm over its batch image, q projection for its 1024 queries,
k/v projections over all 4096 keys, per-head attention (S^T = k^T q, softmax
along the PSUM partition axis via an appended ones-column in the PV matmul),
output projection and residual for its query slice.

v2 performance structure:
 - Warmup matmuls trip the PE HAM clock gate early (else everything runs at
   1.2 GHz instead of 2.4 GHz).
 - Heads are processed in pairs: head-even uses PE rows 0:63, head-odd rows
   64:127 (tile_position row groups) so the two S matmuls per key-tile run
   concurrently in the PE array and LDWEIGHTS overlaps streaming.
 - softmax exp alternates between ScalarE (real exp, even key-tiles) and
   VectorE (Schraudolph bit-trick exp -> bf16 bit pattern via an int16
   round, odd key-tiles), halving the exp bottleneck.
 - PV matmuls lag S by 2 key-tiles (3-deep PSUM ring) so the exp latency is
   off the PE critical path.

All matmuls run in bf16 with fp32 PSUM accumulation; softmax logits stay fp32.
"""
import numpy as np

C = 512          # channels
N = 4096         # pixels (64*64)
NQ = 1024        # queries per core
H = 8            # heads
D = 64           # head dim
T = 4            # 128-channel chunks
W = NQ // 512    # query windows of 512
MT = N // 128    # key m-tiles of 128
NGROUPS = 8
EPS = 1e-5
GELEM = (C // NGROUPS) * N   # elements per norm group

# Schraudolph fast-exp (fp8e5m2 bits via int8 round-to-nearest):
#   bits = round(raw * SA5 + SB5);  bitcast(int8->fp8e5) ~ exp(0.125*raw)
# SA5 = 0.125 * log2(e) * 4 ; SB5 = 15*4 - 0.21875 (max rel err 11.7%,
# same order as the direct e5m2 quantization of a true exp)
SA5 = 0.7213475204444817
SB5 = 59.78125

N_WARM1 = 60     # warmup MMs before transposes (covers input DMA)
N_WARM2 = 20     # warmup MMs per groupnorm chunk
N_WARM3 = 80     # warmup MMs after groupnorm emission

_COMPILED = None


def _emit(tc, io):
    import concourse.bass as bass
    from concourse import mybir
    from contextlib import ExitStack

    nc = tc.nc
    f32 = mybir.dt.float32
    bf16 = mybir.dt.bfloat16
    i8 = mybir.dt.int8
    f8 = mybir.dt.float8e5
    Alu = mybir.AluOpType
    Act = mybir.ActivationFunctionType

    xb, qkvw, qkvb, projw, projb, nw, nb, y = (
        io["xb"], io["qkvw"], io["qkvb"], io["projw"], io["projb"],
        io["nw"], io["nb"], io["y"])
    bscr = io["bscr"]

    ctx = ExitStack()
    with ctx:
        # ---------------- pools ----------------
        # PSUM: sp ring 3x(128,1024) [6 banks] + pv 2x(128,512) [2 banks]
        left = ctx.enter_context(tc.tile_pool(name="left", bufs=1))
        psum_sp = ctx.enter_context(tc.tile_pool(name="psum_sp", bufs=3, space="PSUM"))
        psum_pv = ctx.enter_context(tc.tile_pool(name="psum_pv", bufs=2, space="PSUM"))

        right_ctx = ExitStack()
        xf_pool = right_ctx.enter_context(
            tc.tile_pool(name="xf_pool", bufs=1, side="right"))
        wstg_pool = right_ctx.enter_context(
            tc.tile_pool(name="wstg_pool", bufs=4, side="right"))
        scr_pool = right_ctx.enter_context(
            tc.tile_pool(name="scr_pool", bufs=2, side="right"))

        # ---------------- persistent tiles ----------------
        # fp8 activations/weights for DoubleRow GEMMs, chunk-major layouts
        xn8 = left.tile([128, T * N], f8, name="xn8", tag="xn8")
        ksb = [left.tile([128, N], bf16, name=f"ksb{t}", tag=f"ksb{t}") for t in range(T)]
        qsb = [left.tile([128, NQ], bf16, name=f"qsb{t}", tag=f"qsb{t}") for t in range(T)]
        wTq8 = left.tile([128, T * 1536], f8, name="wTq8", tag="wTq8")
        wTp8 = left.tile([128, T * C], f8, name="wTp8", tag="wTp8")
        vb_bc = left.tile([128, C], f32, name="vb_bc", tag="vb_bc")
        ones_row = left.tile([1, D], f32, name="ones_row", tag="ones_row")
        qb = [left.tile([128, 1], f32, name=f"qb{i}", tag=f"qb{i}") for i in range(8)]
        pb = [left.tile([128, 1], f32, name=f"pb{i}", tag=f"pb{i}") for i in range(T)]
        nwt = [left.tile([128, 1], f32, name=f"nwt{t}", tag=f"nwt{t}") for t in range(T)]
        nbt = [left.tile([128, 1], f32, name=f"nbt{t}", tag=f"nbt{t}") for t in range(T)]
        stat = [left.tile([128, 2], f32, name=f"stat{t}", tag=f"stat{t}") for t in range(T)]
        gstat = [left.tile([128, 2], f32, name=f"gstat{t}", tag=f"gstat{t}") for t in range(T)]
        wu = left.tile([128, 512], bf16, name="wu", tag="wu")

        # ---------------- warmup: trip the HAM clock gate ----------------
        nc.vector.memset(wu[:], 0.125)
        wu_ps = psum_pv.tile([128, 512], f32, name="wu_ps", tag="pv")
        # preload the exp activation table while idle
        wu_exp = scr_pool.tile([1, 8], f32, name="wu_exp", tag="wu_exp")
        nc.scalar.activation(wu_exp[0:1, :], wu[0:1, 0:8], Act.Exp)

        def warm(n):
            for _ in range(n):
                nc.tensor.matmul(wu_ps[:], wu[:, 0:128], wu[:],
                                 start=True, stop=True)
        warm(N_WARM1)

        # ---------------- input DMAs ----------------
        xf = [xf_pool.tile([128, N], f32, name=f"xf{t}", tag=f"xf{t}") for t in range(T)]
        for t in range(T):
            for c4 in range(4):   # split across DMA queues
                nc.sync.dma_start(
                    xf[t][:, 1024 * c4:1024 * (c4 + 1)],
                    xb[128 * t:128 * (t + 1), 1024 * c4:1024 * (c4 + 1)])
            nc.sync.dma_start(nwt[t][:, 0:1], nw[128 * t:128 * (t + 1)])
            nc.sync.dma_start(nbt[t][:, 0:1], nb[128 * t:128 * (t + 1)])
            nc.sync.dma_start(pb[t][:, 0:1], projb[128 * t:128 * (t + 1)])
        for i in range(8):
            nc.sync.dma_start(qb[i][:, 0:1], qkvb[128 * i:128 * (i + 1)])
        # v bias broadcast to 128 partitions (stride-0 partition read)
        nc.gpsimd.dma_start(
            out=vb_bc[:],
            in_=bass.AP(tensor=qkvb.tensor, offset=1024, ap=[[0, 128], [1, C]]))
        nc.vector.memset(ones_row[0:1, :], 1.0)

        # weights: natural-layout contiguous DMA, cast to bf16, transpose
        # 128x128 blocks on the PE (identity trick) into wTq/wTp.
        ident = left.tile([128, 128], bf16, name="ident", tag="ident")
        nc.sync.dma_start(ident[:], io["cid"][:, :])
        ind = left.tile([128, 2], f32, name="ind", tag="ind")
        nc.sync.dma_start(ind[:], io["cind"][:, :])
        indT = left.tile([2, 128], f32, name="indT", tag="indT")
        nc.sync.dma_start(indT[0:2, :], io["cindT"][:, :])
        for i in range(12):   # qkv_w row-tiles
            wstg = wstg_pool.tile([128, C], f32, name="wstg", tag="wstg")
            nc.sync.dma_start(wstg[:], qkvw[128 * i:128 * (i + 1), :])
            wbf = wstg_pool.tile([128, C], bf16, name="wbf", tag="wbf")
            nc.vector.tensor_copy(wbf[:], wstg[:])
            for j in range(T):
                tp = psum_sp.tile([128, 1024], bf16, name="tp", tag="sp")
                nc.tensor.transpose(tp[:, 0:128], wbf[:, 128 * j:128 * (j + 1)], ident[:])
                nc.vector.tensor_copy(
                    wTq8[:, 1536 * j + 128 * i:1536 * j + 128 * (i + 1)], tp[:, 0:128])
        for i in range(4):    # proj_w row-tiles
            wstg = wstg_pool.tile([128, C], f32, name="wstg", tag="wstg")
            nc.sync.dma_start(wstg[:], projw[128 * i:128 * (i + 1), :])
            wbf = wstg_pool.tile([128, C], bf16, name="wbf", tag="wbf")
            nc.vector.tensor_copy(wbf[:], wstg[:])
            for j in range(T):
                tp = psum_sp.tile([128, 1024], bf16, name="tp", tag="sp")
                nc.tensor.transpose(tp[:, 0:128], wbf[:, 128 * j:128 * (j + 1)], ident[:])
                nc.vector.tensor_copy(
                    wTp8[:, C * j + 128 * i:C * j + 128 * (i + 1)], tp[:, 0:128])

        # ---------------- phase 1: group stats + normalize ----------------
        for t in range(T):
            sm_scr = scr_pool.tile([128, N], bf16, name="sm_scr", tag="sq_scr")
            nc.scalar.activation(
                sm_scr[:], xf[t][:], Act.Identity, accum_out=stat[t][:, 0:1])
            sq_scr = scr_pool.tile([128, N], bf16, name="sq_scr", tag="sq_scr")
            nc.scalar.activation(
                sq_scr[:], xf[t][:], Act.Square, accum_out=stat[t][:, 1:2])
            # group-reduce over partitions via indicator matmuls
            gg_ps = psum_pv.tile([128, 512], f32, name="gg_ps", tag="pv")
            nc.tensor.matmul(gg_ps[0:2, 0:2], ind[:, 0:2], stat[t][:, 0:2],
                             start=True, stop=True)
            gg_sb = left.tile([2, 2], f32, name=f"gg_sb{t}", tag=f"gg_sb{t}")
            nc.vector.tensor_copy(gg_sb[0:2, :], gg_ps[0:2, 0:2])
            gb_ps = psum_pv.tile([128, 512], f32, name="gb_ps", tag="pv")
            nc.tensor.matmul(gb_ps[:, 0:2], indT[0:2, :], gg_sb[0:2, :],
                             start=True, stop=True)
            nc.vector.tensor_copy(gstat[t][:, 0:2], gb_ps[:, 0:2])
            # mean/var/rstd -> per-channel affine a,b
            mean_t = left.tile([128, 1], f32, name=f"mean{t}", tag=f"mean{t}")
            e2_t = left.tile([128, 1], f32, name=f"e2{t}", tag=f"e2{t}")
            var_t = left.tile([128, 1], f32, name=f"var{t}", tag=f"var{t}")
            std_t = left.tile([128, 1], f32, name=f"std{t}", tag=f"std{t}")
            a_t = left.tile([128, 1], f32, name=f"a{t}", tag=f"a{t}")
            b_t = left.tile([128, 1], f32, name=f"b{t}", tag=f"b{t}")
            inv = 1.0 / GELEM
            nc.vector.tensor_scalar(mean_t[:], gstat[t][:, 0:1], inv, None, Alu.mult)
            nc.vector.tensor_scalar(e2_t[:], gstat[t][:, 1:2], inv, None, Alu.mult)
            nc.vector.scalar_tensor_tensor(
                var_t[:], mean_t[:], -1.0, mean_t[:], Alu.mult, Alu.mult)
            nc.vector.scalar_tensor_tensor(
                var_t[:], e2_t[:], EPS, var_t[:], Alu.add, Alu.add)
            nc.scalar.activation(std_t[:], var_t[:], Act.Sqrt)
            nc.vector.reciprocal(a_t[:], std_t[:])
            nc.vector.tensor_tensor(a_t[:], a_t[:], nwt[t][:], Alu.mult)
            nc.vector.tensor_tensor(b_t[:], mean_t[:], a_t[:], Alu.mult)
            nc.vector.tensor_tensor(b_t[:], nbt[t][:], b_t[:], Alu.subtract)
            # normalize + cast to fp8 (alternate engines to halve the chain)
            if t % 2 == 0:
                nc.scalar.activation(
                    xn8[:, N * t:N * (t + 1)], xf[t][:], Act.Identity,
                    bias=b_t[:, 0:1], scale=a_t[:, 0:1])
            else:
                nc.vector.tensor_scalar(
                    xn8[:, N * t:N * (t + 1)], xf[t][:],
                    a_t[:, 0:1], b_t[:, 0:1], Alu.mult, Alu.add)
            warm(N_WARM2)
        warm(N_WARM3)

        right_ctx.close()

        # ---------------- mid pools (reuse xf space) ----------------
        mid = ctx.enter_context(tc.tile_pool(name="mid", bufs=1))
        ps8_pool = ctx.enter_context(tc.tile_pool(name="ps8_pool", bufs=8))
        rec_pool = ctx.enter_context(tc.tile_pool(name="rec_pool", bufs=2))
        yo_pool = ctx.enter_context(tc.tile_pool(name="yo_pool", bufs=2))

        # fp8e5 v^T for DoubleRow PV: per m-pair m2 and head h, 160 cols =
        # two 80-wide blocks (64 v + ones col @64 + 15 zero pad), one per
        # m-tile of the pair
        vT8 = mid.tile([128, (MT // 2) * 1280], f8, name="vT8", tag="vT8")
        attn8 = mid.tile([128, T * NQ], f8, name="attn8", tag="attn8")
        xres = [mid.tile([128, NQ], f32, name=f"xres{t}", tag=f"xres{t}") for t in range(T)]
        for t in range(T):
            nc.sync.dma_start(xres[t][:], xb[128 * t:128 * (t + 1), 0:NQ])

        # ones + zero-pad columns of the augmented v^T (denominator trick)
        v80 = vT8[:].rearrange("p (n x) -> p n x", x=80)
        nc.vector.memset(v80[:, :, 64:65], 1.0)
        nc.vector.memset(v80[:, :, 65:80], 0.0)

        # ---------------- phase 3: projections (fp8 DoubleRow) ----------
        DR = mybir.MatmulPerfMode.DoubleRow

        def wq_pair(pr, lo, hi):
            return wTq8[:, 3072 * pr:3072 * (pr + 1)].rearrange(
                "p (j x) -> p j x", j=2)[:, :, lo:hi]

        def xn_pair(pr, lo, hi):
            return xn8[:, 2 * N * pr:2 * N * (pr + 1)].rearrange(
                "p (j x) -> p j x", j=2)[:, :, lo:hi]

        # q: out rows 0..511 of qkv, only NQ query columns (2 windows per tile)
        for i in range(T):
            qp = psum_sp.tile([128, 1024], f32, name="qp", tag="sp")
            for w in range(W):
                for pr in range(2):
                    nc.tensor.matmul(
                        qp[:, 512 * w:512 * w + 512],
                        wq_pair(pr, 128 * i, 128 * i + 128),
                        xn_pair(pr, 512 * w, 512 * w + 512),
                        start=(pr == 0), stop=(pr == 1), perf_mode=DR)
            if i % 2 == 0:
                nc.scalar.add(qsb[i][:], qp[:], qb[i][:, 0:1])
            else:
                nc.vector.tensor_scalar(
                    qsb[i][:], qp[:], qb[i][:, 0:1], None, Alu.add)
        # k: out rows 512..1023, all N columns (2 windows per psum tile)
        for i in range(T):
            for w2 in range(4):
                kp = psum_sp.tile([128, 1024], f32, name="kp", tag="sp")
                for w in (2 * w2, 2 * w2 + 1):
                    for pr in range(2):
                        nc.tensor.matmul(
                            kp[:, 512 * (w % 2):512 * (w % 2) + 512],
                            wq_pair(pr, 512 + 128 * i, 512 + 128 * i + 128),
                            xn_pair(pr, 512 * w, 512 * w + 512),
                            start=(pr == 0), stop=(pr == 1), perf_mode=DR)
                if w2 % 2 == 0:
                    nc.scalar.add(
                        ksb[i][:, 1024 * w2:1024 * (w2 + 1)], kp[:],
                        qb[4 + i][:, 0:1])
                else:
                    nc.vector.tensor_scalar(
                        ksb[i][:, 1024 * w2:1024 * (w2 + 1)], kp[:],
                        qb[4 + i][:, 0:1], None, Alu.add)
        # vT: (m, 512) per m-tile, two m-tiles per psum tile, strided into
        # the 80-column augmented fp8 layout
        for m2 in range(MT // 2):
            vp = psum_sp.tile([128, 1024], f32, name="vp", tag="sp")
            for mh in range(2):
                mt = 2 * m2 + mh
                for pr in range(2):
                    nc.tensor.matmul(
                        vp[:, 512 * mh:512 * mh + 512],
                        xn_pair(pr, 128 * mt, 128 * mt + 128),
                        wq_pair(pr, 1024, 1536),
                        start=(pr == 0), stop=(pr == 1), perf_mode=DR)
            vbv = vb_bc[:].rearrange("p (h x) -> p h x", x=64)
            vdst = vT8[:, 1280 * m2:1280 * (m2 + 1)].rearrange(
                "p (h j x) -> p h j x", h=8, j=2, x=80)
            for mh in range(2):
                dst = vdst[:, :, mh, 0:64]
                src = vp[:, 512 * mh:512 * mh + 512].rearrange(
                    "p (h x) -> p h x", x=64)
                nc.vector.tensor_tensor(dst, src, vbv, Alu.add)

        # ---------------- phase 4+5: attention, deferred tails, proj ----
        # tails are emitted in stages inside the NEXT unit's m-loop so the
        # PE stream never waits on the reciprocal chain
        def tail_stage1(tw, thp, tpv0, tpv1):
            # denominators stacked on partition 0 + one reciprocal
            dnq = rec_pool.tile([1, 1024], f32, name="dnq", tag="dnq")
            nc.scalar.copy(dnq[0:1, 0:512], tpv0[64:65, :])
            nc.scalar.copy(dnq[0:1, 512:1024], tpv1[64:65, :])
            rcq = rec_pool.tile([1, 1024], f32, name="rcq", tag="rcq")
            rscr = rec_pool.tile([1, 1024], f32, name="rscr", tag="rscr")
            nc.vector.reciprocal_approx_accurate(
                rcq[0:1, :], dnq[0:1, :], rscr[0:1, :])
            return rcq

        def tail_stage2_pe(rcq):
            # PE broadcast variant for the final tail (shorter serial chain
            # than the DRAM round-trip when nothing overlaps it)
            bc = psum_sp.tile([128, 1024], f32, name="bc", tag="sp")
            nc.tensor.matmul(
                bc[0:64, 0:512], ones_row[0:1, 0:D],
                rcq[0:1, 0:512], start=True, stop=True)
            nc.tensor.matmul(
                bc[64:128, 0:512], ones_row[0:1, 0:D],
                rcq[0:1, 512:1024], start=True, stop=True)
            bcs0 = rec_pool.tile([64, 512], f32, name="bcs0", tag="bcs0")
            bcs1 = rec_pool.tile([64, 512], f32, name="bcs1", tag="bcs1")
            nc.scalar.copy(bcs0[0:64, :], bc[0:64, 0:512])
            nc.scalar.copy(bcs1[0:64, :], bc[64:128, 0:512])
            return bcs0, bcs1

        def tail_stage2(rcq, slot):
            # partition-broadcast of the reciprocals via a DRAM round-trip
            # (stride-0 partition read) -- zero PE involvement; all three
            # DMAs share the in-order sync queue
            nc.sync.dma_start(bscr[slot:slot + 1, :], rcq[0:1, :])
            bcs0 = rec_pool.tile([64, 512], f32, name="bcs0", tag="bcs0")
            bcs1 = rec_pool.tile([64, 512], f32, name="bcs1", tag="bcs1")
            nc.sync.dma_start(
                bcs0[0:64, :],
                bass.AP(tensor=bscr.tensor, offset=1024 * slot,
                        ap=[[0, 64], [1, 512]]))
            nc.sync.dma_start(
                bcs1[0:64, :],
                bass.AP(tensor=bscr.tensor, offset=1024 * slot + 512,
                        ap=[[0, 64], [1, 512]]))
            return bcs0, bcs1

        def tail_stage3(tw, thp, tpv0, tpv1, bcs0, bcs1):
            nc.vector.tensor_tensor(
                attn8[0:64, NQ * thp + 512 * tw:NQ * thp + 512 * tw + 512],
                tpv0[0:64, :], bcs0[0:64, :], Alu.mult)
            nc.vector.tensor_tensor(
                attn8[64:128, NQ * thp + 512 * tw:NQ * thp + 512 * tw + 512],
                tpv1[0:64, :], bcs1[0:64, :], Alu.mult)

        def emit_proj(pw):
            # proj + residual for window pw (fp8 DoubleRow over attn8)
            for i in range(T):
                py = psum_sp.tile([128, 1024], f32, name="py", tag="sp")
                for pr in range(2):
                    rhs = attn8[:, 2 * NQ * pr:2 * NQ * (pr + 1)].rearrange(
                        "p (j x) -> p j x", j=2)[:, :, 512 * pw:512 * pw + 512]
                    lhs = wTp8[:, 1024 * pr:1024 * (pr + 1)].rearrange(
                        "p (j x) -> p j x", j=2)[:, :, 128 * i:128 * i + 128]
                    nc.tensor.matmul(
                        py[:, 0:512], lhs, rhs,
                        start=(pr == 0), stop=(pr == 1), perf_mode=DR)
                yo = yo_pool.tile([128, 512], f32, name="yo", tag="yo")
                nc.vector.scalar_tensor_tensor(
                    yo[:], py[:, 0:512], pb[i][:, 0:1],
                    xres[i][:, 512 * pw:512 * pw + 512], Alu.add, Alu.add)
                nc.sync.dma_start(
                    y[128 * i:128 * i + 128, 512 * pw:512 * pw + 512], yo[:])

        pend = []             # (m2, ps8, hp, pv0, pv1) across units

        def flush_pv():
            m2, ps8t, fhp, fpv0, fpv1 = pend.pop(0)
            st, sto = (m2 == 0), (m2 == MT // 2 - 1)
            vb = 1280 * m2
            for h, pv in ((0, fpv0), (1, fpv1)):
                lhs = vT8[:, vb + 160 * (2 * fhp + h):
                          vb + 160 * (2 * fhp + h) + 160].rearrange(
                    "p (j x) -> p j x", j=2)
                rhs = ps8t[:, 1024 * h:1024 * h + 1024].rearrange(
                    "p (j x) -> p j x", j=2)
                nc.tensor.matmul(
                    pv[0:80, :], lhs, rhs, start=st, stop=sto,
                    perf_mode=DR)

        deferred = None       # (w, hp, pv0, pv1) of the previous unit
        proj_due = None       # window whose proj should be emitted next
        for w in range(W):
            for hp in range(4):
                # drain the previous unit's leftover PV flushes first: the
                # engines finish its last exps while these stream
                while pend:
                    flush_pv()
                h0, h1 = 2 * hp, 2 * hp + 1
                pv0 = psum_pv.tile([128, 512], f32, name="pv0", tag="pv")
                pv1 = psum_pv.tile([128, 512], f32, name="pv1", tag="pv")

                ps8 = None
                for m in range(MT):
                    sp = psum_sp.tile([128, 1024], f32, name="sp", tag="sp")
                    # S pair: head-even on PE rows 0:63, head-odd on rows
                    # 64:127 (concurrent row groups, shared LDW window)
                    nc.tensor.matmul(
                        sp[:, 0:512],
                        ksb[hp][0:64, 128 * m:128 * m + 128],
                        qsb[hp][0:64, 512 * w:512 * w + 512],
                        start=True, stop=True)
                    nc.tensor.matmul(
                        sp[:, 512:1024],
                        ksb[hp][64:128, 128 * m:128 * m + 128],
                        qsb[hp][64:128, 512 * w:512 * w + 512],
                        start=True, stop=True)
                    m2, mh = divmod(m, 2)
                    if mh == 0:
                        ps8 = ps8_pool.tile([128, 2048], f8, name="ps8", tag="ps8")
                    # exp of this m-tile for both heads into the fp8 pair
                    # tile: cols [1024h + 512mh : +512] = (head h, m-half mh)
                    outv = ps8[:].rearrange(
                        "p (H x) -> p H x", H=2)[:, :, 512 * mh:512 * mh + 512]
                    inv = sp[:].rearrange("p (h x) -> p h x", h=2)
                    # ACT handles even m-tiles plus one odd (engine balance);
                    # DVE does the rest via the int8 Schraudolph bit trick
                    if mh == 0 or m == 13:
                        nc.scalar.activation(outv, inv, Act.Exp, scale=0.125)
                    else:
                        nc.vector.tensor_scalar(
                            outv.bitcast(i8), inv, SA5, SB5, Alu.mult, Alu.add)
                    if mh == 1:
                        pend.append((m2, ps8, hp, pv0, pv1))
                        if len(pend) >= 7:
                            flush_pv()
                    # previous unit's tail stages + any due proj, overlapped
                    # with this unit's m-loop (keeps the PE stream dense)
                    if deferred is not None:
                        if m == 2:
                            with tc.high_priority():
                                t_rcq = tail_stage1(*deferred)
                        elif m == 5:
                            with tc.high_priority():
                                t_bcs = tail_stage2(
                                    t_rcq, deferred[0] * 4 + deferred[1])
                        elif m == 8:
                            with tc.high_priority():
                                tail_stage3(*deferred, *t_bcs)
                            deferred = None
                    elif m == 10 and proj_due is not None:
                        with tc.high_priority():
                            emit_proj(proj_due)
                        proj_due = None
                deferred = (w, hp, pv0, pv1)
                if hp == 3:
                    proj_due = w
        while pend:
            flush_pv()
        t_rcq = tail_stage1(*deferred)
        t_bcs = tail_stage2_pe(t_rcq)
        tail_stage3(*deferred, *t_bcs)
        emit_proj(proj_due)


def _build():
    import concourse.tile as tile
    from concourse import bacc, mybir

    nc = bacc.Bacc("TRN2", target_bir_lowering=False, debug=False)
    f32 = mybir.dt.float32
    io = {
        "xb": nc.dram_tensor("xb", [C, N], f32, kind="ExternalInput").ap(),
        "qkvw": nc.dram_tensor("qkvw", [3 * C, C], f32, kind="ExternalInput").ap(),
        "qkvb": nc.dram_tensor("qkvb", [3 * C], f32, kind="ExternalInput").ap(),
        "projw": nc.dram_tensor("projw", [C, C], f32, kind="ExternalInput").ap(),
        "projb": nc.dram_tensor("projb", [C], f32, kind="ExternalInput").ap(),
        "nw": nc.dram_tensor("nw", [C], f32, kind="ExternalInput").ap(),
        "nb": nc.dram_tensor("nb", [C], f32, kind="ExternalInput").ap(),
        "cid": nc.dram_tensor("cid", [128, 128], mybir.dt.bfloat16,
                              kind="ExternalInput").ap(),
        "cind": nc.dram_tensor("cind", [128, 2], f32, kind="ExternalInput").ap(),
        "cindT": nc.dram_tensor("cindT", [2, 128], f32, kind="ExternalInput").ap(),
        "y": nc.dram_tensor("y", [C, NQ], f32, kind="ExternalOutput").ap(),
        "bscr": nc.dram_tensor("bscr", [8, 1024], f32).ap(),
    }
    with tile.TileContext(nc) as tc:
        _emit(tc, io)
    nc.compile()
    return nc


def get_compiled():
    global _COMPILED
    if _COMPILED is None:
        _COMPILED = _build()
    return _COMPILED


def make_in_maps(x, norm_w, norm_b, qkv_w, qkv_b, proj_w, proj_b):
    import ml_dtypes

    xf = np.ascontiguousarray(np.asarray(x, np.float32)).reshape(2, C, N)
    ind = np.zeros((128, 2), np.float32)
    ind[0:64, 0] = 1.0
    ind[64:128, 1] = 1.0
    shared = {
        "cid": np.eye(128, dtype=ml_dtypes.bfloat16),
        "cind": ind,
        "cindT": np.ascontiguousarray(ind.T),
        "qkvw": np.ascontiguousarray(np.asarray(qkv_w, np.float32)),
        "qkvb": np.ascontiguousarray(np.asarray(qkv_b, np.float32)),
        "projw": np.ascontiguousarray(np.asarray(proj_w, np.float32)),
        "projb": np.ascontiguousarray(np.asarray(proj_b, np.float32)),
        "nw": np.ascontiguousarray(np.asarray(norm_w, np.float32)),
        "nb": np.ascontiguousarray(np.asarray(norm_b, np.float32)),
    }
    in_maps = []
    for core in range(8):
        bi, qs = core // 4, core % 4
        # rotate so this core's queries are always columns [0:NQ)
        xroll = np.concatenate(
            [xf[bi][:, qs * NQ:], xf[bi][:, :qs * NQ]], axis=1)
        m = dict(shared)
        m["xb"] = np.ascontiguousarray(xroll)
        in_maps.append(m)
    return in_maps


def assemble(results, x):
    y = np.zeros((2, C, N), np.float32)
    for core in range(8):
        bi, qs = core // 4, core % 4
        y[bi][:, qs * NQ:(qs + 1) * NQ] = results[core]["y"]
    return y.reshape(x.shape)


def kernel(x, norm_w, norm_b, qkv_w, qkv_b, proj_w, proj_b, **_ignored):
    from concourse import bass_utils

    nc = get_compiled()
    in_maps = make_in_maps(x, norm_w, norm_b, qkv_w, qkv_b, proj_w, proj_b)
    res = bass_utils.run_bass_kernel_spmd(nc, in_maps, core_ids=list(range(8)))
    return assemble(res.results, np.asarray(x))


# revision 38
# speedup vs baseline: 2.1761x; 1.0051x over previous
"""Trainium2 Bass kernel for nn_AttentionBlock (GroupNorm + MHA + proj + residual).

Full inputs in, full output out. Sharding: 8 cores = 2 batches x 4 query-slices.
Each core: GroupNorm over its batch image, q projection for its 1024 queries,
k/v projections over all 4096 keys, per-head attention (S^T = k^T q, softmax
along the PSUM partition axis via an appended ones-column in the PV matmul),
output projection and residual for its query slice.

Performance structure (measured 358us on HW vs 776us for the plain
flash-style schedule):
 - Warmup matmuls trip the PE HAM clock gate early (else everything runs at
   1.2 GHz instead of 2.4 GHz) and cover the DMA/groupnorm prolog.
 - qkv/proj GEMMs run as fp8e5m2 DoubleRow matmuls (contraction chunk pairs,
   ~1.45x); S matmuls stay bf16 (k/q in bf16), issued as head pairs on PE
   row groups 0:63 / 64:127.
 - softmax exp alternates between ScalarE (real exp) and VectorE (Schraudolph
   bit-trick: int8 round -> fp8e5m2 bit pattern), splitting the 33M-logit
   exp load across both engines; outputs land strided in a per-key-pair fp8
   tile that feeds DoubleRow PV matmuls (ones-column augmented 80-col v^T
   blocks keep the softmax denominator free).
 - PV lags S by 6 key-pairs (big fp8 ring) and leftover PV flushes drain at
   the next unit's start so the PE always has queued work while the engines
   catch up; per-head tails (denominator reciprocal + DRAM-round-trip
   partition broadcast + scale) are emitted mid-next-unit at high priority,
   entirely off the PE critical path.

Logits accumulate in fp32 PSUM; groupnorm statistics are fp32.
"""
import numpy as np

C = 512          # channels
N = 4096         # pixels (64*64)
NQ = 1024        # queries per core
H = 8            # heads
D = 64           # head dim
T = 4            # 128-channel chunks
W = NQ // 512    # query windows of 512
MT = N // 128    # key m-tiles of 128
NGROUPS = 8
EPS = 1e-5
GELEM = (C // NGROUPS) * N   # elements per norm group

# Schraudolph fast-exp (fp8e5m2 bits via int8 round-to-nearest):
#   bits = round(raw * SA5 + SB5);  bitcast(int8->fp8e5) ~ exp(0.125*raw)
# SA5 = 0.125 * log2(e) * 4 ; SB5 = 15*4 - 0.21875 (max rel err 11.7%,
# same order as the direct e5m2 quantization of a true exp)
SA5 = 0.7213475204444817
SB5 = 59.78125

N_WARM1 = 60     # warmup MMs before transposes (covers input DMA)
N_WARM2 = 20     # warmup MMs per groupnorm chunk
N_WARM3 = 30     # warmup MMs after groupnorm emission

_COMPILED = None


def _emit(tc, io):
    import concourse.bass as bass
    from concourse import mybir
    from contextlib import ExitStack

    nc = tc.nc
    f32 = mybir.dt.float32
    bf16 = mybir.dt.bfloat16
    i8 = mybir.dt.int8
    f8 = mybir.dt.float8e5
    Alu = mybir.AluOpType
    Act = mybir.ActivationFunctionType

    xb, qkvw, qkvb, projw, projb, nw, nb, y = (
        io["xb"], io["qkvw"], io["qkvb"], io["projw"], io["projb"],
        io["nw"], io["nb"], io["y"])
    bscr = io["bscr"]

    ctx = ExitStack()
    with ctx:
        # ---------------- pools ----------------
        # PSUM: sp ring 3x(128,1024) [6 banks] + pv 2x(128,512) [2 banks]
        left = ctx.enter_context(tc.tile_pool(name="left", bufs=1))
        psum_sp = ctx.enter_context(tc.tile_pool(name="psum_sp", bufs=3, space="PSUM"))
        psum_pv = ctx.enter_context(tc.tile_pool(name="psum_pv", bufs=2, space="PSUM"))

        right_ctx = ExitStack()
        xf_pool = right_ctx.enter_context(
            tc.tile_pool(name="xf_pool", bufs=1, side="right"))
        wstg_pool = right_ctx.enter_context(
            tc.tile_pool(name="wstg_pool", bufs=4, side="right"))
        scr_pool = right_ctx.enter_context(
            tc.tile_pool(name="scr_pool", bufs=2, side="right"))

        # ---------------- persistent tiles ----------------
        # fp8 activations/weights for DoubleRow GEMMs, chunk-major layouts
        xn8 = left.tile([128, T * N], f8, name="xn8", tag="xn8")
        ksb = [left.tile([128, N], bf16, name=f"ksb{t}", tag=f"ksb{t}") for t in range(T)]
        qsb = [left.tile([128, NQ], bf16, name=f"qsb{t}", tag=f"qsb{t}") for t in range(T)]
        wTq8 = left.tile([128, T * 1536], f8, name="wTq8", tag="wTq8")
        wTp8 = left.tile([128, T * C], f8, name="wTp8", tag="wTp8")
        vb_bc = left.tile([128, C], f32, name="vb_bc", tag="vb_bc")
        ones_row = left.tile([1, D], f32, name="ones_row", tag="ones_row")
        qb = [left.tile([128, 1], f32, name=f"qb{i}", tag=f"qb{i}") for i in range(8)]
        pb = [left.tile([128, 1], f32, name=f"pb{i}", tag=f"pb{i}") for i in range(T)]
        nwt = [left.tile([128, 1], f32, name=f"nwt{t}", tag=f"nwt{t}") for t in range(T)]
        nbt = [left.tile([128, 1], f32, name=f"nbt{t}", tag=f"nbt{t}") for t in range(T)]
        stat = [left.tile([128, 2], f32, name=f"stat{t}", tag=f"stat{t}") for t in range(T)]
        gstat = [left.tile([128, 2], f32, name=f"gstat{t}", tag=f"gstat{t}") for t in range(T)]
        wu = left.tile([128, 512], bf16, name="wu", tag="wu")

        # ---------------- warmup: trip the HAM clock gate ----------------
        nc.vector.memset(wu[:], 0.125)
        wu_ps = psum_pv.tile([128, 512], f32, name="wu_ps", tag="pv")
        # preload the exp activation table while idle
        wu_exp = scr_pool.tile([1, 8], f32, name="wu_exp", tag="wu_exp")
        nc.scalar.activation(wu_exp[0:1, :], wu[0:1, 0:8], Act.Exp)

        def warm(n):
            for _ in range(n):
                nc.tensor.matmul(wu_ps[:], wu[:, 0:128], wu[:],
                                 start=True, stop=True)
        warm(N_WARM1)

        # ---------------- input DMAs ----------------
        xf = [xf_pool.tile([128, N], f32, name=f"xf{t}", tag=f"xf{t}") for t in range(T)]
        for t in range(T):
            for c4 in range(4):   # split across DMA queues
                nc.sync.dma_start(
                    xf[t][:, 1024 * c4:1024 * (c4 + 1)],
                    xb[128 * t:128 * (t + 1), 1024 * c4:1024 * (c4 + 1)])
            nc.sync.dma_start(nwt[t][:, 0:1], nw[128 * t:128 * (t + 1)])
            nc.sync.dma_start(nbt[t][:, 0:1], nb[128 * t:128 * (t + 1)])
            nc.sync.dma_start(pb[t][:, 0:1], projb[128 * t:128 * (t + 1)])
        for i in range(8):
            nc.sync.dma_start(qb[i][:, 0:1], qkvb[128 * i:128 * (i + 1)])
        # v bias broadcast to 128 partitions (stride-0 partition read)
        nc.gpsimd.dma_start(
            out=vb_bc[:],
            in_=bass.AP(tensor=qkvb.tensor, offset=1024, ap=[[0, 128], [1, C]]))
        nc.vector.memset(ones_row[0:1, :], 1.0)

        # weights: natural-layout contiguous DMA, cast to bf16, transpose
        # 128x128 blocks on the PE (identity trick) into wTq/wTp.
        ident = left.tile([128, 128], bf16, name="ident", tag="ident")
        nc.sync.dma_start(ident[:], io["cid"][:, :])
        ind = left.tile([128, 2], f32, name="ind", tag="ind")
        nc.sync.dma_start(ind[:], io["cind"][:, :])
        indT = left.tile([2, 128], f32, name="indT", tag="indT")
        nc.sync.dma_start(indT[0:2, :], io["cindT"][:, :])
        for i in range(12):   # qkv_w row-tiles
            wstg = wstg_pool.tile([128, C], f32, name="wstg", tag="wstg")
            nc.sync.dma_start(wstg[:], qkvw[128 * i:128 * (i + 1), :])
            wbf = wstg_pool.tile([128, C], bf16, name="wbf", tag="wbf")
            nc.vector.tensor_copy(wbf[:], wstg[:])
            for j in range(T):
                tp = psum_sp.tile([128, 1024], bf16, name="tp", tag="sp")
                nc.tensor.transpose(tp[:, 0:128], wbf[:, 128 * j:128 * (j + 1)], ident[:])
                nc.vector.tensor_copy(
                    wTq8[:, 1536 * j + 128 * i:1536 * j + 128 * (i + 1)], tp[:, 0:128])
        for i in range(4):    # proj_w row-tiles
            wstg = wstg_pool.tile([128, C], f32, name="wstg", tag="wstg")
            nc.sync.dma_start(wstg[:], projw[128 * i:128 * (i + 1), :])
            wbf = wstg_pool.tile([128, C], bf16, name="wbf", tag="wbf")
            nc.vector.tensor_copy(wbf[:], wstg[:])
            for j in range(T):
                tp = psum_sp.tile([128, 1024], bf16, name="tp", tag="sp")
                nc.tensor.transpose(tp[:, 0:128], wbf[:, 128 * j:128 * (j + 1)], ident[:])
                nc.vector.tensor_copy(
                    wTp8[:, C * j + 128 * i:C * j + 128 * (i + 1)], tp[:, 0:128])

        # ---------------- phase 1: group stats + normalize ----------------
        for t in range(T):
            sm_scr = scr_pool.tile([128, N], bf16, name="sm_scr", tag="sq_scr")
            nc.scalar.activation(
                sm_scr[:], xf[t][:], Act.Identity, accum_out=stat[t][:, 0:1])
            sq_scr = scr_pool.tile([128, N], bf16, name="sq_scr", tag="sq_scr")
            nc.scalar.activation(
                sq_scr[:], xf[t][:], Act.Square, accum_out=stat[t][:, 1:2])
            # group-reduce over partitions via indicator matmuls
            gg_ps = psum_pv.tile([128, 512], f32, name="gg_ps", tag="pv")
            nc.tensor.matmul(gg_ps[0:2, 0:2], ind[:, 0:2], stat[t][:, 0:2],
                             start=True, stop=True)
            gg_sb = left.tile([2, 2], f32, name=f"gg_sb{t}", tag=f"gg_sb{t}")
            nc.vector.tensor_copy(gg_sb[0:2, :], gg_ps[0:2, 0:2])
            gb_ps = psum_pv.tile([128, 512], f32, name="gb_ps", tag="pv")
            nc.tensor.matmul(gb_ps[:, 0:2], indT[0:2, :], gg_sb[0:2, :],
                             start=True, stop=True)
            nc.vector.tensor_copy(gstat[t][:, 0:2], gb_ps[:, 0:2])
            # mean/var/rstd -> per-channel affine a,b
            mean_t = left.tile([128, 1], f32, name=f"mean{t}", tag=f"mean{t}")
            e2_t = left.tile([128, 1], f32, name=f"e2{t}", tag=f"e2{t}")
            var_t = left.tile([128, 1], f32, name=f"var{t}", tag=f"var{t}")
            std_t = left.tile([128, 1], f32, name=f"std{t}", tag=f"std{t}")
            a_t = left.tile([128, 1], f32, name=f"a{t}", tag=f"a{t}")
            b_t = left.tile([128, 1], f32, name=f"b{t}", tag=f"b{t}")
            inv = 1.0 / GELEM
            nc.vector.tensor_scalar(mean_t[:], gstat[t][:, 0:1], inv, None, Alu.mult)
            nc.vector.tensor_scalar(e2_t[:], gstat[t][:, 1:2], inv, None, Alu.mult)
            nc.vector.scalar_tensor_tensor(
                var_t[:], mean_t[:], -1.0, mean_t[:], Alu.mult, Alu.mult)
            nc.vector.scalar_tensor_tensor(
                var_t[:], e2_t[:], EPS, var_t[:], Alu.add, Alu.add)
            nc.scalar.activation(std_t[:], var_t[:], Act.Sqrt)
            nc.vector.reciprocal(a_t[:], std_t[:])
            nc.vector.tensor_tensor(a_t[:], a_t[:], nwt[t][:], Alu.mult)
            nc.vector.tensor_tensor(b_t[:], mean_t[:], a_t[:], Alu.mult)
            nc.vector.tensor_tensor(b_t[:], nbt[t][:], b_t[:], Alu.subtract)
            # normalize + cast to fp8 (alternate engines to halve the chain)
            if t % 2 == 0:
                nc.scalar.activation(
                    xn8[:, N * t:N * (t + 1)], xf[t][:], Act.Identity,
                    bias=b_t[:, 0:1], scale=a_t[:, 0:1])
            else:
                nc.vector.tensor_scalar(
                    xn8[:, N * t:N * (t + 1)], xf[t][:],
                    a_t[:, 0:1], b_t[:, 0:1], Alu.mult, Alu.add)
            warm(N_WARM2)
        warm(N_WARM3)

        right_ctx.close()

        # ---------------- mid pools (reuse xf space) ----------------
        mid = ctx.enter_context(tc.tile_pool(name="mid", bufs=1))
        ps8_pool = ctx.enter_context(tc.tile_pool(name="ps8_pool", bufs=8))
        rec_pool = ctx.enter_context(tc.tile_pool(name="rec_pool", bufs=2))
        yo_pool = ctx.enter_context(tc.tile_pool(name="yo_pool", bufs=2))

        # fp8e5 v^T for DoubleRow PV: per m-pair m2 and head h, 160 cols =
        # two 80-wide blocks (64 v + ones col @64 + 15 zero pad), one per
        # m-tile of the pair
        vT8 = mid.tile([128, (MT // 2) * 1280], f8, name="vT8", tag="vT8")
        attn8 = mid.tile([128, T * NQ], f8, name="attn8", tag="attn8")
        xres = [mid.tile([128, NQ], f32, name=f"xres{t}", tag=f"xres{t}") for t in range(T)]
        for t in range(T):
            nc.sync.dma_start(xres[t][:], xb[128 * t:128 * (t + 1), 0:NQ])

        # ones + zero-pad columns of the augmented v^T (denominator trick)
        v80 = vT8[:].rearrange("p (n x) -> p n x", x=80)
        nc.vector.memset(v80[:, :, 64:65], 1.0)
        nc.vector.memset(v80[:, :, 65:80], 0.0)

        # ---------------- phase 3: projections (fp8 DoubleRow) ----------
        DR = mybir.MatmulPerfMode.DoubleRow

        def wq_pair(pr, lo, hi):
            return wTq8[:, 3072 * pr:3072 * (pr + 1)].rearrange(
                "p (j x) -> p j x", j=2)[:, :, lo:hi]

        def xn_pair(pr, lo, hi):
            return xn8[:, 2 * N * pr:2 * N * (pr + 1)].rearrange(
                "p (j x) -> p j x", j=2)[:, :, lo:hi]

        # q: out rows 0..511 of qkv, only NQ query columns (2 windows per tile)
        for i in range(T):
            qp = psum_sp.tile([128, 1024], f32, name="qp", tag="sp")
            for w in range(W):
                for pr in range(2):
                    nc.tensor.matmul(
                        qp[:, 512 * w:512 * w + 512],
                        wq_pair(pr, 128 * i, 128 * i + 128),
                        xn_pair(pr, 512 * w, 512 * w + 512),
                        start=(pr == 0), stop=(pr == 1), perf_mode=DR)
            if i % 2 == 0:
                nc.scalar.add(qsb[i][:], qp[:], qb[i][:, 0:1])
            else:
                nc.vector.tensor_scalar(
                    qsb[i][:], qp[:], qb[i][:, 0:1], None, Alu.add)
        # k: out rows 512..1023, all N columns (2 windows per psum tile)
        for i in range(T):
            for w2 in range(4):
                kp = psum_sp.tile([128, 1024], f32, name="kp", tag="sp")
                for w in (2 * w2, 2 * w2 + 1):
                    for pr in range(2):
                        nc.tensor.matmul(
                            kp[:, 512 * (w % 2):512 * (w % 2) + 512],
                            wq_pair(pr, 512 + 128 * i, 512 + 128 * i + 128),
                            xn_pair(pr, 512 * w, 512 * w + 512),
                            start=(pr == 0), stop=(pr == 1), perf_mode=DR)
                if w2 % 2 == 0:
                    nc.scalar.add(
                        ksb[i][:, 1024 * w2:1024 * (w2 + 1)], kp[:],
                        qb[4 + i][:, 0:1])
                else:
                    nc.vector.tensor_scalar(
                        ksb[i][:, 1024 * w2:1024 * (w2 + 1)], kp[:],
                        qb[4 + i][:, 0:1], None, Alu.add)
        # vT: (m, 512) per m-tile, two m-tiles per psum tile, strided into
        # the 80-column augmented fp8 layout
        for m2 in range(MT // 2):
            vp = psum_sp.tile([128, 1024], f32, name="vp", tag="sp")
            for mh in range(2):
                mt = 2 * m2 + mh
                for pr in range(2):
                    nc.tensor.matmul(
                        vp[:, 512 * mh:512 * mh + 512],
                        xn_pair(pr, 128 * mt, 128 * mt + 128),
                        wq_pair(pr, 1024, 1536),
                        start=(pr == 0), stop=(pr == 1), perf_mode=DR)
            vbv = vb_bc[:].rearrange("p (h x) -> p h x", x=64)
            vdst = vT8[:, 1280 * m2:1280 * (m2 + 1)].rearrange(
                "p (h j x) -> p h j x", h=8, j=2, x=80)
            for mh in range(2):
                dst = vdst[:, :, mh, 0:64]
                src = vp[:, 512 * mh:512 * mh + 512].rearrange(
                    "p (h x) -> p h x", x=64)
                nc.vector.tensor_tensor(dst, src, vbv, Alu.add)

        # ---------------- phase 4+5: attention, deferred tails, proj ----
        # tails are emitted in stages inside the NEXT unit's m-loop so the
        # PE stream never waits on the reciprocal chain
        def tail_stage1(tw, thp, tpv0, tpv1):
            # denominators stacked on partition 0 + one reciprocal
            dnq = rec_pool.tile([1, 1024], f32, name="dnq", tag="dnq")
            nc.scalar.copy(dnq[0:1, 0:512], tpv0[64:65, :])
            nc.scalar.copy(dnq[0:1, 512:1024], tpv1[64:65, :])
            return tail_rec(dnq)

        REC_FAST = True

        def tail_rec(dnq):
            rcq = rec_pool.tile([1, 1024], f32, name="rcq", tag="rcq")
            if REC_FAST:
                nc.vector.reciprocal_approx_fast(rcq[0:1, :], dnq[0:1, :])
            else:
                rscr = rec_pool.tile([1, 1024], f32, name="rscr", tag="rscr")
                nc.vector.reciprocal_approx_accurate(
                    rcq[0:1, :], dnq[0:1, :], rscr[0:1, :])
            return rcq

        def tail_stage2_pe(rcq):
            # PE broadcast variant for the final tail (shorter serial chain
            # than the DRAM round-trip when nothing overlaps it)
            bc = psum_sp.tile([128, 1024], f32, name="bc", tag="sp")
            nc.tensor.matmul(
                bc[0:64, 0:512], ones_row[0:1, 0:D],
                rcq[0:1, 0:512], start=True, stop=True)
            nc.tensor.matmul(
                bc[64:128, 0:512], ones_row[0:1, 0:D],
                rcq[0:1, 512:1024], start=True, stop=True)
            bcs0 = rec_pool.tile([64, 512], f32, name="bcs0", tag="bcs0")
            bcs1 = rec_pool.tile([64, 512], f32, name="bcs1", tag="bcs1")
            nc.scalar.copy(bcs0[0:64, :], bc[0:64, 0:512])
            nc.scalar.copy(bcs1[0:64, :], bc[64:128, 0:512])
            return bcs0, bcs1

        def tail_stage2(rcq, slot):
            # partition-broadcast of the reciprocals via a DRAM round-trip
            # (stride-0 partition read) -- zero PE involvement; all three
            # DMAs share the in-order sync queue
            nc.sync.dma_start(bscr[slot:slot + 1, :], rcq[0:1, :])
            bcs0 = rec_pool.tile([64, 512], f32, name="bcs0", tag="bcs0")
            bcs1 = rec_pool.tile([64, 512], f32, name="bcs1", tag="bcs1")
            nc.sync.dma_start(
                bcs0[0:64, :],
                bass.AP(tensor=bscr.tensor, offset=1024 * slot,
                        ap=[[0, 64], [1, 512]]))
            nc.sync.dma_start(
                bcs1[0:64, :],
                bass.AP(tensor=bscr.tensor, offset=1024 * slot + 512,
                        ap=[[0, 64], [1, 512]]))
            return bcs0, bcs1

        def tail_stage3(tw, thp, tpv0, tpv1, bcs0, bcs1):
            nc.vector.tensor_tensor(
                attn8[0:64, NQ * thp + 512 * tw:NQ * thp + 512 * tw + 512],
                tpv0[0:64, :], bcs0[0:64, :], Alu.mult)
            nc.vector.tensor_tensor(
                attn8[64:128, NQ * thp + 512 * tw:NQ * thp + 512 * tw + 512],
                tpv1[0:64, :], bcs1[0:64, :], Alu.mult)

        def emit_proj(pw):
            # proj + residual for window pw (fp8 DoubleRow over attn8)
            for i in range(T):
                py = psum_sp.tile([128, 1024], f32, name="py", tag="sp")
                for pr in range(2):
                    rhs = attn8[:, 2 * NQ * pr:2 * NQ * (pr + 1)].rearrange(
                        "p (j x) -> p j x", j=2)[:, :, 512 * pw:512 * pw + 512]
                    lhs = wTp8[:, 1024 * pr:1024 * (pr + 1)].rearrange(
                        "p (j x) -> p j x", j=2)[:, :, 128 * i:128 * i + 128]
                    nc.tensor.matmul(
                        py[:, 0:512], lhs, rhs,
                        start=(pr == 0), stop=(pr == 1), perf_mode=DR)
                yo = yo_pool.tile([128, 512], f32, name="yo", tag="yo")
                nc.vector.scalar_tensor_tensor(
                    yo[:], py[:, 0:512], pb[i][:, 0:1],
                    xres[i][:, 512 * pw:512 * pw + 512], Alu.add, Alu.add)
                nc.sync.dma_start(
                    y[128 * i:128 * i + 128, 512 * pw:512 * pw + 512], yo[:])

        pend = []             # (m2, ps8, hp, pv0, pv1) across units

        def flush_pv():
            m2, ps8t, fhp, fpv0, fpv1 = pend.pop(0)
            st, sto = (m2 == 0), (m2 == MT // 2 - 1)
            vb = 1280 * m2
            for h, pv in ((0, fpv0), (1, fpv1)):
                lhs = vT8[:, vb + 160 * (2 * fhp + h):
                          vb + 160 * (2 * fhp + h) + 160].rearrange(
                    "p (j x) -> p j x", j=2)
                rhs = ps8t[:, 1024 * h:1024 * h + 1024].rearrange(
                    "p (j x) -> p j x", j=2)
                nc.tensor.matmul(
                    pv[0:80, :], lhs, rhs, start=st, stop=sto,
                    perf_mode=DR)

        deferred = None       # (w, hp, pv0, pv1) of the previous unit
        proj_due = None       # window whose proj should be emitted next
        for w in range(W):
            for hp in range(4):
                # drain the previous unit's leftover PV flushes first: the
                # engines finish its last exps while these stream
                while pend:
                    flush_pv()
                h0, h1 = 2 * hp, 2 * hp + 1
                pv0 = psum_pv.tile([128, 512], f32, name="pv0", tag="pv")
                pv1 = psum_pv.tile([128, 512], f32, name="pv1", tag="pv")

                ps8 = None
                for m in range(MT):
                    sp = psum_sp.tile([128, 1024], f32, name="sp", tag="sp")
                    # S pair: head-even on PE rows 0:63, head-odd on rows
                    # 64:127 (concurrent row groups, shared LDW window)
                    nc.tensor.matmul(
                        sp[:, 0:512],
                        ksb[hp][0:64, 128 * m:128 * m + 128],
                        qsb[hp][0:64, 512 * w:512 * w + 512],
                        start=True, stop=True)
                    nc.tensor.matmul(
                        sp[:, 512:1024],
                        ksb[hp][64:128, 128 * m:128 * m + 128],
                        qsb[hp][64:128, 512 * w:512 * w + 512],
                        start=True, stop=True)
                    m2, mh = divmod(m, 2)
                    if mh == 0:
                        ps8 = ps8_pool.tile([128, 2048], f8, name="ps8", tag="ps8")
                    # exp of this m-tile for both heads into the fp8 pair
                    # tile: cols [1024h + 512mh : +512] = (head h, m-half mh)
                    outv = ps8[:].rearrange(
                        "p (H x) -> p H x", H=2)[:, :, 512 * mh:512 * mh + 512]
                    inv = sp[:].rearrange("p (h x) -> p h x", h=2)
                    # ACT handles even m-tiles plus one odd (engine balance);
                    # DVE does the rest via the int8 Schraudolph bit trick
                    if mh == 0 or m == 13:
                        nc.scalar.activation(outv, inv, Act.Exp, scale=0.125)
                    else:
                        nc.vector.tensor_scalar(
                            outv.bitcast(i8), inv, SA5, SB5, Alu.mult, Alu.add)
                    if mh == 1:
                        pend.append((m2, ps8, hp, pv0, pv1))
                        if len(pend) >= 7:
                            flush_pv()
                    # previous unit's tail stages + any due proj, overlapped
                    # with this unit's m-loop (keeps the PE stream dense)
                    if deferred is not None:
                        if m == 2:
                            with tc.high_priority():
                                t_rcq = tail_stage1(*deferred)
                        elif m == 5:
                            with tc.high_priority():
                                t_bcs = tail_stage2(
                                    t_rcq, deferred[0] * 4 + deferred[1])
                        elif m == 8:
                            with tc.high_priority():
                                tail_stage3(*deferred, *t_bcs)
                            deferred = None
                    elif m == 10 and proj_due is not None:
                        with tc.high_priority():
                            emit_proj(proj_due)
                        proj_due = None
                deferred = (w, hp, pv0, pv1)
                if hp == 3:
                    proj_due = w
        while pend:
            flush_pv()
        t_rcq = tail_stage1(*deferred)
        t_bcs = tail_stage2_pe(t_rcq)
        tail_stage3(*deferred, *t_bcs)
        emit_proj(proj_due)


def _build():
    import concourse.tile as tile
    from concourse import bacc, mybir

    nc = bacc.Bacc("TRN2", target_bir_lowering=False, debug=False)
    f32 = mybir.dt.float32
    io = {
        "xb": nc.dram_tensor("xb", [C, N], f32, kind="ExternalInput").ap(),
        "qkvw": nc.dram_tensor("qkvw", [3 * C, C], f32, kind="ExternalInput").ap(),
        "qkvb": nc.dram_tensor("qkvb", [3 * C], f32, kind="ExternalInput").ap(),
        "projw": nc.dram_tensor("projw", [C, C], f32, kind="ExternalInput").ap(),
        "projb": nc.dram_tensor("projb", [C], f32, kind="ExternalInput").ap(),
        "nw": nc.dram_tensor("nw", [C], f32, kind="ExternalInput").ap(),
        "nb": nc.dram_tensor("nb", [C], f32, kind="ExternalInput").ap(),
        "cid": nc.dram_tensor("cid", [128, 128], mybir.dt.bfloat16,
                              kind="ExternalInput").ap(),
        "cind": nc.dram_tensor("cind", [128, 2], f32, kind="ExternalInput").ap(),
        "cindT": nc.dram_tensor("cindT", [2, 128], f32, kind="ExternalInput").ap(),
        "y": nc.dram_tensor("y", [C, NQ], f32, kind="ExternalOutput").ap(),
        "bscr": nc.dram_tensor("bscr", [8, 1024], f32).ap(),
    }
    with tile.TileContext(nc) as tc:
        _emit(tc, io)
    nc.compile()
    return nc


def get_compiled():
    global _COMPILED
    if _COMPILED is None:
        _COMPILED = _build()
    return _COMPILED


def make_in_maps(x, norm_w, norm_b, qkv_w, qkv_b, proj_w, proj_b):
    import ml_dtypes

    xf = np.ascontiguousarray(np.asarray(x, np.float32)).reshape(2, C, N)
    ind = np.zeros((128, 2), np.float32)
    ind[0:64, 0] = 1.0
    ind[64:128, 1] = 1.0
    shared = {
        "cid": np.eye(128, dtype=ml_dtypes.bfloat16),
        "cind": ind,
        "cindT": np.ascontiguousarray(ind.T),
        "qkvw": np.ascontiguousarray(np.asarray(qkv_w, np.float32)),
        "qkvb": np.ascontiguousarray(np.asarray(qkv_b, np.float32)),
        "projw": np.ascontiguousarray(np.asarray(proj_w, np.float32)),
        "projb": np.ascontiguousarray(np.asarray(proj_b, np.float32)),
        "nw": np.ascontiguousarray(np.asarray(norm_w, np.float32)),
        "nb": np.ascontiguousarray(np.asarray(norm_b, np.float32)),
    }
    in_maps = []
    for core in range(8):
        bi, qs = core // 4, core % 4
        # rotate so this core's queries are always columns [0:NQ)
        xroll = np.concatenate(
            [xf[bi][:, qs * NQ:], xf[bi][:, :qs * NQ]], axis=1)
        m = dict(shared)
        m["xb"] = np.ascontiguousarray(xroll)
        in_maps.append(m)
    return in_maps


def assemble(results, x):
    y = np.zeros((2, C, N), np.float32)
    for core in range(8):
        bi, qs = core // 4, core % 4
        y[bi][:, qs * NQ:(qs + 1) * NQ] = results[core]["y"]
    return y.reshape(x.shape)


def kernel(x, norm_w, norm_b, qkv_w, qkv_b, proj_w, proj_b, **_ignored):
    from concourse import bass_utils

    nc = get_compiled()
    in_maps = make_in_maps(x, norm_w, norm_b, qkv_w, qkv_b, proj_w, proj_b)
    res = bass_utils.run_bass_kernel_spmd(nc, in_maps, core_ids=list(range(8)))
    return assemble(res.results, np.asarray(x))


# revision 39
# speedup vs baseline: 2.1831x; 1.0032x over previous
"""Trainium2 Bass kernel for nn_AttentionBlock (GroupNorm + MHA + proj + residual).

Full inputs in, full output out. Sharding: 8 cores = 2 batches x 4 query-slices.
Each core: GroupNorm over its batch image, q projection for its 1024 queries,
k/v projections over all 4096 keys, per-head attention (S^T = k^T q, softmax
along the PSUM partition axis via an appended ones-column in the PV matmul),
output projection and residual for its query slice.

Performance structure (measured 358us on HW vs 776us for the plain
flash-style schedule):
 - Warmup matmuls trip the PE HAM clock gate early (else everything runs at
   1.2 GHz instead of 2.4 GHz) and cover the DMA/groupnorm prolog.
 - qkv/proj GEMMs run as fp8e5m2 DoubleRow matmuls (contraction chunk pairs,
   ~1.45x); S matmuls stay bf16 (k/q in bf16), issued as head pairs on PE
   row groups 0:63 / 64:127.
 - softmax exp alternates between ScalarE (real exp) and VectorE (Schraudolph
   bit-trick: int8 round -> fp8e5m2 bit pattern), splitting the 33M-logit
   exp load across both engines; outputs land strided in a per-key-pair fp8
   tile that feeds DoubleRow PV matmuls (ones-column augmented 80-col v^T
   blocks keep the softmax denominator free).
 - PV lags S by 6 key-pairs (big fp8 ring) and leftover PV flushes drain at
   the next unit's start so the PE always has queued work while the engines
   catch up; per-head tails (denominator reciprocal + DRAM-round-trip
   partition broadcast + scale) are emitted mid-next-unit at high priority,
   entirely off the PE critical path.

Logits accumulate in fp32 PSUM; groupnorm statistics are fp32.
"""
import numpy as np

C = 512          # channels
N = 4096         # pixels (64*64)
NQ = 1024        # queries per core
H = 8            # heads
D = 64           # head dim
T = 4            # 128-channel chunks
W = NQ // 512    # query windows of 512
MT = N // 128    # key m-tiles of 128
NGROUPS = 8
EPS = 1e-5
GELEM = (C // NGROUPS) * N   # elements per norm group

# Schraudolph fast-exp (fp8e5m2 bits via int8 round-to-nearest):
#   bits = round(raw * SA5 + SB5);  bitcast(int8->fp8e5) ~ exp(0.125*raw)
# SA5 = 0.125 * log2(e) * 4 ; SB5 = 15*4 - 0.21875 (max rel err 11.7%,
# same order as the direct e5m2 quantization of a true exp)
SA5 = 0.7213475204444817
SB5 = 59.78125

N_WARM1 = 60     # warmup MMs before transposes (covers input DMA)
N_WARM2 = 20     # warmup MMs per groupnorm chunk
N_WARM3 = 80     # warmup MMs after groupnorm emission

_COMPILED = None


def _emit(tc, io):
    import concourse.bass as bass
    from concourse import mybir
    from contextlib import ExitStack

    nc = tc.nc
    f32 = mybir.dt.float32
    bf16 = mybir.dt.bfloat16
    i8 = mybir.dt.int8
    f8 = mybir.dt.float8e5
    Alu = mybir.AluOpType
    Act = mybir.ActivationFunctionType

    xb, qkvw, qkvb, projw, projb, nw, nb, y = (
        io["xb"], io["qkvw"], io["qkvb"], io["projw"], io["projb"],
        io["nw"], io["nb"], io["y"])
    bscr = io["bscr"]

    ctx = ExitStack()
    with ctx:
        # ---------------- pools ----------------
        # PSUM: sp ring 3x(128,1024) [6 banks] + pv 2x(128,512) [2 banks]
        left = ctx.enter_context(tc.tile_pool(name="left", bufs=1))
        psum_sp = ctx.enter_context(tc.tile_pool(name="psum_sp", bufs=3, space="PSUM"))
        psum_pv = ctx.enter_context(tc.tile_pool(name="psum_pv", bufs=2, space="PSUM"))

        right_ctx = ExitStack()
        xf_pool = right_ctx.enter_context(
            tc.tile_pool(name="xf_pool", bufs=1, side="right"))
        wstg_pool = right_ctx.enter_context(
            tc.tile_pool(name="wstg_pool", bufs=4, side="right"))
        scr_pool = right_ctx.enter_context(
            tc.tile_pool(name="scr_pool", bufs=2, side="right"))

        # ---------------- persistent tiles ----------------
        # fp8 activations/weights for DoubleRow GEMMs, chunk-major layouts
        xn8 = left.tile([128, T * N], f8, name="xn8", tag="xn8")
        ksb = [left.tile([128, N], bf16, name=f"ksb{t}", tag=f"ksb{t}") for t in range(T)]
        qsb = [left.tile([128, NQ], bf16, name=f"qsb{t}", tag=f"qsb{t}") for t in range(T)]
        wTq8 = left.tile([128, T * 1536], f8, name="wTq8", tag="wTq8")
        wTp8 = left.tile([128, T * C], f8, name="wTp8", tag="wTp8")
        vb_bc = left.tile([128, C], f32, name="vb_bc", tag="vb_bc")
        ones_row = left.tile([1, D], f32, name="ones_row", tag="ones_row")
        qb = [left.tile([128, 1], f32, name=f"qb{i}", tag=f"qb{i}") for i in range(8)]
        pb = [left.tile([128, 1], f32, name=f"pb{i}", tag=f"pb{i}") for i in range(T)]
        nwt = [left.tile([128, 1], f32, name=f"nwt{t}", tag=f"nwt{t}") for t in range(T)]
        nbt = [left.tile([128, 1], f32, name=f"nbt{t}", tag=f"nbt{t}") for t in range(T)]
        stat = [left.tile([128, 2], f32, name=f"stat{t}", tag=f"stat{t}") for t in range(T)]
        gstat = [left.tile([128, 2], f32, name=f"gstat{t}", tag=f"gstat{t}") for t in range(T)]
        wu = left.tile([128, 512], bf16, name="wu", tag="wu")

        # ---------------- warmup: trip the HAM clock gate ----------------
        nc.vector.memset(wu[:], 0.125)
        wu_ps = psum_pv.tile([128, 512], f32, name="wu_ps", tag="pv")
        # preload the exp activation table while idle
        wu_exp = scr_pool.tile([1, 8], f32, name="wu_exp", tag="wu_exp")
        nc.scalar.activation(wu_exp[0:1, :], wu[0:1, 0:8], Act.Exp)

        def warm(n):
            for _ in range(n):
                nc.tensor.matmul(wu_ps[:], wu[:, 0:128], wu[:],
                                 start=True, stop=True)
        warm(N_WARM1)

        # ---------------- input DMAs ----------------
        xf = [xf_pool.tile([128, N], f32, name=f"xf{t}", tag=f"xf{t}") for t in range(T)]
        for t in range(T):
            for c4 in range(4):   # split across DMA queues
                nc.sync.dma_start(
                    xf[t][:, 1024 * c4:1024 * (c4 + 1)],
                    xb[128 * t:128 * (t + 1), 1024 * c4:1024 * (c4 + 1)])
            nc.sync.dma_start(nwt[t][:, 0:1], nw[128 * t:128 * (t + 1)])
            nc.sync.dma_start(nbt[t][:, 0:1], nb[128 * t:128 * (t + 1)])
            nc.sync.dma_start(pb[t][:, 0:1], projb[128 * t:128 * (t + 1)])
        for i in range(8):
            nc.sync.dma_start(qb[i][:, 0:1], qkvb[128 * i:128 * (i + 1)])
        # v bias broadcast to 128 partitions (stride-0 partition read)
        nc.gpsimd.dma_start(
            out=vb_bc[:],
            in_=bass.AP(tensor=qkvb.tensor, offset=1024, ap=[[0, 128], [1, C]]))
        nc.vector.memset(ones_row[0:1, :], 1.0)

        # weights: natural-layout contiguous DMA, cast to bf16, transpose
        # 128x128 blocks on the PE (identity trick) into wTq/wTp.
        ident = left.tile([128, 128], bf16, name="ident", tag="ident")
        nc.sync.dma_start(ident[:], io["cid"][:, :])
        ind = left.tile([128, 2], f32, name="ind", tag="ind")
        nc.sync.dma_start(ind[:], io["cind"][:, :])
        indT = left.tile([2, 128], f32, name="indT", tag="indT")
        nc.sync.dma_start(indT[0:2, :], io["cindT"][:, :])
        for i in range(12):   # qkv_w row-tiles
            wstg = wstg_pool.tile([128, C], f32, name="wstg", tag="wstg")
            nc.sync.dma_start(wstg[:], qkvw[128 * i:128 * (i + 1), :])
            wbf = wstg_pool.tile([128, C], bf16, name="wbf", tag="wbf")
            nc.vector.tensor_copy(wbf[:], wstg[:])
            for j in range(T):
                tp = psum_sp.tile([128, 1024], bf16, name="tp", tag="sp")
                nc.tensor.transpose(tp[:, 0:128], wbf[:, 128 * j:128 * (j + 1)], ident[:])
                nc.vector.tensor_copy(
                    wTq8[:, 1536 * j + 128 * i:1536 * j + 128 * (i + 1)], tp[:, 0:128])
        for i in range(4):    # proj_w row-tiles
            wstg = wstg_pool.tile([128, C], f32, name="wstg", tag="wstg")
            nc.sync.dma_start(wstg[:], projw[128 * i:128 * (i + 1), :])
            wbf = wstg_pool.tile([128, C], bf16, name="wbf", tag="wbf")
            nc.vector.tensor_copy(wbf[:], wstg[:])
            for j in range(T):
                tp = psum_sp.tile([128, 1024], bf16, name="tp", tag="sp")
                nc.tensor.transpose(tp[:, 0:128], wbf[:, 128 * j:128 * (j + 1)], ident[:])
                nc.vector.tensor_copy(
                    wTp8[:, C * j + 128 * i:C * j + 128 * (i + 1)], tp[:, 0:128])

        # ---------------- phase 1: group stats + normalize ----------------
        for t in range(T):
            sm_scr = scr_pool.tile([128, N], bf16, name="sm_scr", tag="sq_scr")
            nc.scalar.activation(
                sm_scr[:], xf[t][:], Act.Identity, accum_out=stat[t][:, 0:1])
            sq_scr = scr_pool.tile([128, N], bf16, name="sq_scr", tag="sq_scr")
            nc.scalar.activation(
                sq_scr[:], xf[t][:], Act.Square, accum_out=stat[t][:, 1:2])
            # group-reduce over partitions via indicator matmuls
            gg_ps = psum_pv.tile([128, 512], f32, name="gg_ps", tag="pv")
            nc.tensor.matmul(gg_ps[0:2, 0:2], ind[:, 0:2], stat[t][:, 0:2],
                             start=True, stop=True)
            gg_sb = left.tile([2, 2], f32, name=f"gg_sb{t}", tag=f"gg_sb{t}")
            nc.vector.tensor_copy(gg_sb[0:2, :], gg_ps[0:2, 0:2])
            gb_ps = psum_pv.tile([128, 512], f32, name="gb_ps", tag="pv")
            nc.tensor.matmul(gb_ps[:, 0:2], indT[0:2, :], gg_sb[0:2, :],
                             start=True, stop=True)
            nc.vector.tensor_copy(gstat[t][:, 0:2], gb_ps[:, 0:2])
            # mean/var/rstd -> per-channel affine a,b
            mean_t = left.tile([128, 1], f32, name=f"mean{t}", tag=f"mean{t}")
            e2_t = left.tile([128, 1], f32, name=f"e2{t}", tag=f"e2{t}")
            var_t = left.tile([128, 1], f32, name=f"var{t}", tag=f"var{t}")
            std_t = left.tile([128, 1], f32, name=f"std{t}", tag=f"std{t}")
            a_t = left.tile([128, 1], f32, name=f"a{t}", tag=f"a{t}")
            b_t = left.tile([128, 1], f32, name=f"b{t}", tag=f"b{t}")
            inv = 1.0 / GELEM
            nc.vector.tensor_scalar(mean_t[:], gstat[t][:, 0:1], inv, None, Alu.mult)
            nc.vector.tensor_scalar(e2_t[:], gstat[t][:, 1:2], inv, None, Alu.mult)
            nc.vector.scalar_tensor_tensor(
                var_t[:], mean_t[:], -1.0, mean_t[:], Alu.mult, Alu.mult)
            nc.vector.scalar_tensor_tensor(
                var_t[:], e2_t[:], EPS, var_t[:], Alu.add, Alu.add)
            nc.scalar.activation(std_t[:], var_t[:], Act.Sqrt)
            nc.vector.reciprocal(a_t[:], std_t[:])
            nc.vector.tensor_tensor(a_t[:], a_t[:], nwt[t][:], Alu.mult)
            nc.vector.tensor_tensor(b_t[:], mean_t[:], a_t[:], Alu.mult)
            nc.vector.tensor_tensor(b_t[:], nbt[t][:], b_t[:], Alu.subtract)
            # normalize + cast to fp8 (alternate engines to halve the chain)
            if t % 2 == 0:
                nc.scalar.activation(
                    xn8[:, N * t:N * (t + 1)], xf[t][:], Act.Identity,
                    bias=b_t[:, 0:1], scale=a_t[:, 0:1])
            else:
                nc.vector.tensor_scalar(
                    xn8[:, N * t:N * (t + 1)], xf[t][:],
                    a_t[:, 0:1], b_t[:, 0:1], Alu.mult, Alu.add)
            warm(N_WARM2)
        warm(N_WARM3)

        right_ctx.close()

        # ---------------- mid pools (reuse xf space) ----------------
        mid = ctx.enter_context(tc.tile_pool(name="mid", bufs=1))
        ps8_pool = ctx.enter_context(tc.tile_pool(name="ps8_pool", bufs=8))
        rec_pool = ctx.enter_context(tc.tile_pool(name="rec_pool", bufs=2))
        yo_pool = ctx.enter_context(tc.tile_pool(name="yo_pool", bufs=2))

        # fp8e5 v^T for DoubleRow PV: per m-pair m2 and head h, 160 cols =
        # two 80-wide blocks (64 v + ones col @64 + 15 zero pad), one per
        # m-tile of the pair
        vT8 = mid.tile([128, (MT // 2) * 1280], f8, name="vT8", tag="vT8")
        attn8 = mid.tile([128, T * NQ], f8, name="attn8", tag="attn8")
        xres = [mid.tile([128, NQ], f32, name=f"xres{t}", tag=f"xres{t}") for t in range(T)]
        for t in range(T):
            nc.sync.dma_start(xres[t][:], xb[128 * t:128 * (t + 1), 0:NQ])

        # ones + zero-pad columns of the augmented v^T (denominator trick)
        v80 = vT8[:].rearrange("p (n x) -> p n x", x=80)
        nc.vector.memset(v80[:, :, 64:65], 1.0)
        nc.vector.memset(v80[:, :, 65:80], 0.0)

        # ---------------- phase 3: projections (fp8 DoubleRow) ----------
        DR = mybir.MatmulPerfMode.DoubleRow

        def wq_pair(pr, lo, hi):
            return wTq8[:, 3072 * pr:3072 * (pr + 1)].rearrange(
                "p (j x) -> p j x", j=2)[:, :, lo:hi]

        def xn_pair(pr, lo, hi):
            return xn8[:, 2 * N * pr:2 * N * (pr + 1)].rearrange(
                "p (j x) -> p j x", j=2)[:, :, lo:hi]

        # q: out rows 0..511 of qkv, only NQ query columns (2 windows per tile)
        for i in range(T):
            qp = psum_sp.tile([128, 1024], f32, name="qp", tag="sp")
            for w in range(W):
                for pr in range(2):
                    nc.tensor.matmul(
                        qp[:, 512 * w:512 * w + 512],
                        wq_pair(pr, 128 * i, 128 * i + 128),
                        xn_pair(pr, 512 * w, 512 * w + 512),
                        start=(pr == 0), stop=(pr == 1), perf_mode=DR)
            if i % 2 == 0:
                nc.scalar.add(qsb[i][:], qp[:], qb[i][:, 0:1])
            else:
                nc.vector.tensor_scalar(
                    qsb[i][:], qp[:], qb[i][:, 0:1], None, Alu.add)
        # k: out rows 512..1023, all N columns (2 windows per psum tile)
        for i in range(T):
            for w2 in range(4):
                kp = psum_sp.tile([128, 1024], f32, name="kp", tag="sp")
                for w in (2 * w2, 2 * w2 + 1):
                    for pr in range(2):
                        nc.tensor.matmul(
                            kp[:, 512 * (w % 2):512 * (w % 2) + 512],
                            wq_pair(pr, 512 + 128 * i, 512 + 128 * i + 128),
                            xn_pair(pr, 512 * w, 512 * w + 512),
                            start=(pr == 0), stop=(pr == 1), perf_mode=DR)
                if w2 % 2 == 0:
                    nc.scalar.add(
                        ksb[i][:, 1024 * w2:1024 * (w2 + 1)], kp[:],
                        qb[4 + i][:, 0:1])
                else:
                    nc.vector.tensor_scalar(
                        ksb[i][:, 1024 * w2:1024 * (w2 + 1)], kp[:],
                        qb[4 + i][:, 0:1], None, Alu.add)
        # vT: (m, 512) per m-tile, two m-tiles per psum tile, strided into
        # the 80-column augmented fp8 layout
        for m2 in range(MT // 2):
            vp = psum_sp.tile([128, 1024], f32, name="vp", tag="sp")
            for mh in range(2):
                mt = 2 * m2 + mh
                for pr in range(2):
                    nc.tensor.matmul(
                        vp[:, 512 * mh:512 * mh + 512],
                        xn_pair(pr, 128 * mt, 128 * mt + 128),
                        wq_pair(pr, 1024, 1536),
                        start=(pr == 0), stop=(pr == 1), perf_mode=DR)
            warm(2)
            vbv = vb_bc[:].rearrange("p (h x) -> p h x", x=64)
            vdst = vT8[:, 1280 * m2:1280 * (m2 + 1)].rearrange(
                "p (h j x) -> p h j x", h=8, j=2, x=80)
            for mh in range(2):
                dst = vdst[:, :, mh, 0:64]
                src = vp[:, 512 * mh:512 * mh + 512].rearrange(
                    "p (h x) -> p h x", x=64)
                nc.vector.tensor_tensor(dst, src, vbv, Alu.add)

        # ---------------- phase 4+5: attention, deferred tails, proj ----
        # tails are emitted in stages inside the NEXT unit's m-loop so the
        # PE stream never waits on the reciprocal chain
        def tail_stage1(tw, thp, tpv0, tpv1):
            # denominators stacked on partition 0 + one reciprocal
            dnq = rec_pool.tile([1, 1024], f32, name="dnq", tag="dnq")
            nc.scalar.copy(dnq[0:1, 0:512], tpv0[64:65, :])
            nc.scalar.copy(dnq[0:1, 512:1024], tpv1[64:65, :])
            return tail_rec(dnq)

        REC_FAST = True

        def tail_rec(dnq):
            rcq = rec_pool.tile([1, 1024], f32, name="rcq", tag="rcq")
            if REC_FAST:
                nc.vector.reciprocal_approx_fast(rcq[0:1, :], dnq[0:1, :])
            else:
                rscr = rec_pool.tile([1, 1024], f32, name="rscr", tag="rscr")
                nc.vector.reciprocal_approx_accurate(
                    rcq[0:1, :], dnq[0:1, :], rscr[0:1, :])
            return rcq

        def tail_stage2_pe(rcq):
            # PE broadcast variant for the final tail (shorter serial chain
            # than the DRAM round-trip when nothing overlaps it)
            bc = psum_sp.tile([128, 1024], f32, name="bc", tag="sp")
            nc.tensor.matmul(
                bc[0:64, 0:512], ones_row[0:1, 0:D],
                rcq[0:1, 0:512], start=True, stop=True)
            nc.tensor.matmul(
                bc[64:128, 0:512], ones_row[0:1, 0:D],
                rcq[0:1, 512:1024], start=True, stop=True)
            bcs0 = rec_pool.tile([64, 512], f32, name="bcs0", tag="bcs0")
            bcs1 = rec_pool.tile([64, 512], f32, name="bcs1", tag="bcs1")
            nc.scalar.copy(bcs0[0:64, :], bc[0:64, 0:512])
            nc.scalar.copy(bcs1[0:64, :], bc[64:128, 0:512])
            return bcs0, bcs1

        def tail_stage2(rcq, slot):
            # partition-broadcast of the reciprocals via a DRAM round-trip
            # (stride-0 partition read) -- zero PE involvement; all three
            # DMAs share the in-order sync queue
            nc.sync.dma_start(bscr[slot:slot + 1, :], rcq[0:1, :])
            bcs0 = rec_pool.tile([64, 512], f32, name="bcs0", tag="bcs0")
            bcs1 = rec_pool.tile([64, 512], f32, name="bcs1", tag="bcs1")
            nc.sync.dma_start(
                bcs0[0:64, :],
                bass.AP(tensor=bscr.tensor, offset=1024 * slot,
                        ap=[[0, 64], [1, 512]]))
            nc.sync.dma_start(
                bcs1[0:64, :],
                bass.AP(tensor=bscr.tensor, offset=1024 * slot + 512,
                        ap=[[0, 64], [1, 512]]))
            return bcs0, bcs1

        def tail_stage3(tw, thp, tpv0, tpv1, bcs0, bcs1):
            nc.vector.tensor_tensor(
                attn8[0:64, NQ * thp + 512 * tw:NQ * thp + 512 * tw + 512],
                tpv0[0:64, :], bcs0[0:64, :], Alu.mult)
            nc.vector.tensor_tensor(
                attn8[64:128, NQ * thp + 512 * tw:NQ * thp + 512 * tw + 512],
                tpv1[0:64, :], bcs1[0:64, :], Alu.mult)

        def emit_proj(pw):
            # proj + residual for window pw (fp8 DoubleRow over attn8)
            for i in range(T):
                py = psum_sp.tile([128, 1024], f32, name="py", tag="sp")
                for pr in range(2):
                    rhs = attn8[:, 2 * NQ * pr:2 * NQ * (pr + 1)].rearrange(
                        "p (j x) -> p j x", j=2)[:, :, 512 * pw:512 * pw + 512]
                    lhs = wTp8[:, 1024 * pr:1024 * (pr + 1)].rearrange(
                        "p (j x) -> p j x", j=2)[:, :, 128 * i:128 * i + 128]
                    nc.tensor.matmul(
                        py[:, 0:512], lhs, rhs,
                        start=(pr == 0), stop=(pr == 1), perf_mode=DR)
                yo = yo_pool.tile([128, 512], f32, name="yo", tag="yo")
                nc.vector.scalar_tensor_tensor(
                    yo[:], py[:, 0:512], pb[i][:, 0:1],
                    xres[i][:, 512 * pw:512 * pw + 512], Alu.add, Alu.add)
                nc.sync.dma_start(
                    y[128 * i:128 * i + 128, 512 * pw:512 * pw + 512], yo[:])

        pend = []             # (m2, ps8, hp, pv0, pv1) across units

        def flush_pv():
            m2, ps8t, fhp, fpv0, fpv1 = pend.pop(0)
            st, sto = (m2 == 0), (m2 == MT // 2 - 1)
            vb = 1280 * m2
            for h, pv in ((0, fpv0), (1, fpv1)):
                lhs = vT8[:, vb + 160 * (2 * fhp + h):
                          vb + 160 * (2 * fhp + h) + 160].rearrange(
                    "p (j x) -> p j x", j=2)
                rhs = ps8t[:, 1024 * h:1024 * h + 1024].rearrange(
                    "p (j x) -> p j x", j=2)
                nc.tensor.matmul(
                    pv[0:80, :], lhs, rhs, start=st, stop=sto,
                    perf_mode=DR)

        deferred = None       # (w, hp, pv0, pv1) of the previous unit
        proj_due = None       # window whose proj should be emitted next
        for w in range(W):
            for hp in range(4):
                # drain the previous unit's leftover PV flushes first: the
                # engines finish its last exps while these stream
                while pend:
                    flush_pv()
                h0, h1 = 2 * hp, 2 * hp + 1
                pv0 = psum_pv.tile([128, 512], f32, name="pv0", tag="pv")
                pv1 = psum_pv.tile([128, 512], f32, name="pv1", tag="pv")

                ps8 = None
                for m in range(MT):
                    sp = psum_sp.tile([128, 1024], f32, name="sp", tag="sp")
                    # S pair: head-even on PE rows 0:63, head-odd on rows
                    # 64:127 (concurrent row groups, shared LDW window)
                    nc.tensor.matmul(
                        sp[:, 0:512],
                        ksb[hp][0:64, 128 * m:128 * m + 128],
                        qsb[hp][0:64, 512 * w:512 * w + 512],
                        start=True, stop=True)
                    nc.tensor.matmul(
                        sp[:, 512:1024],
                        ksb[hp][64:128, 128 * m:128 * m + 128],
                        qsb[hp][64:128, 512 * w:512 * w + 512],
                        start=True, stop=True)
                    m2, mh = divmod(m, 2)
                    if mh == 0:
                        ps8 = ps8_pool.tile([128, 2048], f8, name="ps8", tag="ps8")
                    # exp of this m-tile for both heads into the fp8 pair
                    # tile: cols [1024h + 512mh : +512] = (head h, m-half mh)
                    outv = ps8[:].rearrange(
                        "p (H x) -> p H x", H=2)[:, :, 512 * mh:512 * mh + 512]
                    inv = sp[:].rearrange("p (h x) -> p h x", h=2)
                    # ACT handles even m-tiles plus one odd (engine balance);
                    # DVE does the rest via the int8 Schraudolph bit trick
                    if mh == 0 or m == 13:
                        nc.scalar.activation(outv, inv, Act.Exp, scale=0.125)
                    else:
                        nc.vector.tensor_scalar(
                            outv.bitcast(i8), inv, SA5, SB5, Alu.mult, Alu.add)
                    if mh == 1:
                        pend.append((m2, ps8, hp, pv0, pv1))
                        if len(pend) >= 7:
                            flush_pv()
                    # previous unit's tail stages + any due proj, overlapped
                    # with this unit's m-loop (keeps the PE stream dense)
                    if deferred is not None:
                        if m == 2:
                            with tc.high_priority():
                                t_rcq = tail_stage1(*deferred)
                        elif m == 5:
                            with tc.high_priority():
                                t_bcs = tail_stage2(
                                    t_rcq, deferred[0] * 4 + deferred[1])
                        elif m == 8:
                            with tc.high_priority():
                                tail_stage3(*deferred, *t_bcs)
                            deferred = None
                    elif m == 10 and proj_due is not None:
                        with tc.high_priority():
                            emit_proj(proj_due)
                        proj_due = None
                deferred = (w, hp, pv0, pv1)
                if hp == 3:
                    proj_due = w
        while pend:
            flush_pv()
        t_rcq = tail_stage1(*deferred)
        t_bcs = tail_stage2_pe(t_rcq)
        tail_stage3(*deferred, *t_bcs)
        emit_proj(proj_due)


def _build():
    import concourse.tile as tile
    from concourse import bacc, mybir

    nc = bacc.Bacc("TRN2", target_bir_lowering=False, debug=False)
    f32 = mybir.dt.float32
    io = {
        "xb": nc.dram_tensor("xb", [C, N], f32, kind="ExternalInput").ap(),
        "qkvw": nc.dram_tensor("qkvw", [3 * C, C], f32, kind="ExternalInput").ap(),
        "qkvb": nc.dram_tensor("qkvb", [3 * C], f32, kind="ExternalInput").ap(),
        "projw": nc.dram_tensor("projw", [C, C], f32, kind="ExternalInput").ap(),
        "projb": nc.dram_tensor("projb", [C], f32, kind="ExternalInput").ap(),
        "nw": nc.dram_tensor("nw", [C], f32, kind="ExternalInput").ap(),
        "nb": nc.dram_tensor("nb", [C], f32, kind="ExternalInput").ap(),
        "cid": nc.dram_tensor("cid", [128, 128], mybir.dt.bfloat16,
                              kind="ExternalInput").ap(),
        "cind": nc.dram_tensor("cind", [128, 2], f32, kind="ExternalInput").ap(),
        "cindT": nc.dram_tensor("cindT", [2, 128], f32, kind="ExternalInput").ap(),
        "y": nc.dram_tensor("y", [C, NQ], f32, kind="ExternalOutput").ap(),
        "bscr": nc.dram_tensor("bscr", [8, 1024], f32).ap(),
    }
    with tile.TileContext(nc) as tc:
        _emit(tc, io)
    nc.compile()
    return nc


def get_compiled():
    global _COMPILED
    if _COMPILED is None:
        _COMPILED = _build()
    return _COMPILED


def make_in_maps(x, norm_w, norm_b, qkv_w, qkv_b, proj_w, proj_b):
    import ml_dtypes

    xf = np.ascontiguousarray(np.asarray(x, np.float32)).reshape(2, C, N)
    ind = np.zeros((128, 2), np.float32)
    ind[0:64, 0] = 1.0
    ind[64:128, 1] = 1.0
    shared = {
        "cid": np.eye(128, dtype=ml_dtypes.bfloat16),
        "cind": ind,
        "cindT": np.ascontiguousarray(ind.T),
        "qkvw": np.ascontiguousarray(np.asarray(qkv_w, np.float32)),
        "qkvb": np.ascontiguousarray(np.asarray(qkv_b, np.float32)),
        "projw": np.ascontiguousarray(np.asarray(proj_w, np.float32)),
        "projb": np.ascontiguousarray(np.asarray(proj_b, np.float32)),
        "nw": np.ascontiguousarray(np.asarray(norm_w, np.float32)),
        "nb": np.ascontiguousarray(np.asarray(norm_b, np.float32)),
    }
    in_maps = []
    for core in range(8):
        bi, qs = core // 4, core % 4
        # rotate so this core's queries are always columns [0:NQ)
        xroll = np.concatenate(
            [xf[bi][:, qs * NQ:], xf[bi][:, :qs * NQ]], axis=1)
        m = dict(shared)
        m["xb"] = np.ascontiguousarray(xroll)
        in_maps.append(m)
    return in_maps


def assemble(results, x):
    y = np.zeros((2, C, N), np.float32)
    for core in range(8):
        bi, qs = core // 4, core % 4
        y[bi][:, qs * NQ:(qs + 1) * NQ] = results[core]["y"]
    return y.reshape(x.shape)


def kernel(x, norm_w, norm_b, qkv_w, qkv_b, proj_w, proj_b, **_ignored):
    from concourse import bass_utils

    nc = get_compiled()
    in_maps = make_in_maps(x, norm_w, norm_b, qkv_w, qkv_b, proj_w, proj_b)
    res = bass_utils.run_bass_kernel_spmd(nc, in_maps, core_ids=list(range(8)))
    return assemble(res.results, np.asarray(x))
